# revision 1
# baseline (speedup 1.0000x reference)
"""Trainium2 Bass kernel for nn_DGCN (gnn_message_passing).

Sharding: 8 shards = (batch b in 0..3, row-half h in 0..1). Each core gets
the full 2048-node K-side tensors of its batch with the node axis ROTATED
by h*1024 so the adjacency diagonal lands at the same tile position on
every core (uniform SPMD program); the core computes rows 0..1023 of the
rotated order, which are exactly rows [h*1024, (h+1)*1024) of the original
order.

Algorithmic restructuring vs the reference:
 - 1x1-conv-over-heads + residual + head-sum collapse to per-head scalars
   c_h = 1 + colsum(mlp_w)[h] folded into q_w on the host (leaky_relu is
   positively homogeneous). The mlp_b term is a constant shift that the
   following LayerNorm cancels.
 - Only diag(relu(adj)) and rowsum(relu(adj)) are ever needed, so the
   [N,N] matrices never touch DRAM; both LayerNorms over the node axis are
   folded into the PE accumulation:
     z_pre = rstdA*(att-muA)*ga + rstdS*(soc-muS)*gs + (ba+bs)
   computed as one PSUM accumulation group per (row-chunk, col-block).
 - LN statistics come from matmuls, not data passes:
     sum(att)  = q . Ksum,  sum(att^2)  = q^T Gram(k) q (and same for soc).
"""

import sys

if '/opt/trn_rl_repo' not in sys.path:
    sys.path.insert(0, '/opt/trn_rl_repo')

from contextlib import ExitStack

import numpy as np

import concourse.bass as bass
import concourse.tile as tile
from concourse import bacc, masks, mybir
from concourse.bass_interp import get_hw_module
from concourse.bass_utils import run_bass_kernel_spmd

FP = mybir.dt.float32
FPR = mybir.dt.float32r
BF = mybir.dt.bfloat16
AL = mybir.AluOpType
AF = mybir.ActivationFunctionType
AX = mybir.AxisListType

B, N, E, G, H = 4, 2048, 64, 64, 4
D = H * G          # 256
HALF = N // 2      # rows per core
NCH = N // 128     # 16 chunks of the full node axis
HCH = HALF // 128  # 8 chunks owned by this core
MB = 512           # column block (one PSUM bank of fp32)
NMB = N // MB      # 4
EPS = 1e-5

_CACHE = {}

USE_FPR = True


def _r(ap):
    return ap.bitcast(FPR) if USE_FPR else ap


_w = _r  # producers feeding fp32r matmuls must write the rounded encoding


def _tp(nc, out_ap, in_ap, ident):
    # transpose identity operand must match the input's contraction dim
    k = in_ap.partition_size()
    nc.tensor.transpose(out_ap, in_ap, ident[0:k, 0:k])


def _ln_rows(nc, pool, t_in, t_out, g_b, b_b, ngr, tag, epsc=None):
    """LayerNorm over 64-wide groups: t_in [128, ngr*64] -> t_out.

    g_b/b_b are [128, 64] partition-broadcast tiles of the gain/bias.
    """
    a3 = t_in[:].rearrange("p (g e) -> p g e", e=64)
    o3 = t_out[:].rearrange("p (g e) -> p g e", e=64)
    sm = pool.tile([128, ngr], FP, tag="ln_sm")
    nc.vector.tensor_reduce(sm[:], a3, AX.X, AL.add)
    sq = pool.tile([128, ngr * 64], FP, tag="ln_sq")
    nc.scalar.square(sq[:], t_in[:])
    sqs = pool.tile([128, ngr], FP, tag="ln_sqs")
    nc.vector.tensor_reduce(sqs[:], sq[:].rearrange("p (g e) -> p g e", e=64), AX.X, AL.add)
    mu = pool.tile([128, ngr], FP, tag="ln_mu")
    nc.vector.tensor_scalar(mu[:], sm[:], 1.0 / 64, None, AL.mult)
    mu2 = pool.tile([128, ngr], FP, tag="ln_mu2")
    nc.vector.tensor_tensor(mu2[:], mu[:], mu[:], AL.mult)
    var = pool.tile([128, ngr], FP, tag="ln_var")
    nc.vector.scalar_tensor_tensor(var[:], sqs[:], 1.0 / 64, mu2[:], AL.mult, AL.subtract)
    sd = pool.tile([128, ngr], FP, tag="ln_sd")
    nc.scalar.activation(sd[:], var[:], AF.Sqrt, bias=epsc[:, 0:1])
    rs = pool.tile([128, ngr], FP, tag="ln_rs")
    nc.vector.reciprocal(rs[:], sd[:])
    mu_b = mu[:].unsqueeze(2).broadcast_to([128, ngr, 64])
    rs_b = rs[:].unsqueeze(2).broadcast_to([128, ngr, 64])
    g3 = g_b[:].unsqueeze(1).broadcast_to([128, ngr, 64])
    b3 = b_b[:].unsqueeze(1).broadcast_to([128, ngr, 64])
    xc = pool.tile([128, ngr * 64], FP, tag="ln_xc")
    xc3 = xc[:].rearrange("p (g e) -> p g e", e=64)
    nc.vector.tensor_tensor(xc3, a3, mu_b, AL.subtract)
    nc.vector.tensor_tensor(xc3, xc3, rs_b, AL.mult)
    nc.vector.tensor_tensor(xc3, xc3, g3, AL.mult)
    nc.vector.tensor_tensor(o3, xc3, b3, AL.add)


def _emit(ctx: ExitStack, tc: tile.TileContext, io: dict):
    nc = tc.nc

    persist = ctx.enter_context(tc.tile_pool(name="persist", bufs=1))
    tp_in = ctx.enter_context(tc.tile_pool(name="tp_in", bufs=3))
    small = ctx.enter_context(tc.tile_pool(name="small", bufs=1))

    # ---- params to SBUF -------------------------------------------------
    def load(name, shape, tag=None):
        t = persist.tile(shape, FP, tag=tag or name)
        nc.sync.dma_start(t[:], io[name][:])
        return t

    fc1s_unused = None
    def loadr(name, shape):
        t = persist.tile(shape, FP, tag=name)
        nc.gpsimd.dma_start(_w(t[:]), io[name][:])
        return t

    fc1s = loadr("fc1s", [64, 16])
    fc2s = loadr("fc2s", [16, 2])
    fc3s = loadr("fc3s", [2, 64])
    wz = loadr("wz", [128, 64])
    wr = loadr("wr", [128, 64])
    wh = loadr("wh", [128, 64])
    qw = loadr("qw", [64, 256])
    kw = loadr("kw", [64, 256])
    w1 = load("w1", [64, 64])
    w2 = load("w2", [64, 64])
    w3 = load("w3", [64, 64])
    b1 = load("b1", [64, 1])
    fc1b = load("fc1b", [16, 1])
    fc2b = load("fc2b", [2, 1])
    fc3b = load("fc3b", [64, 1])
    b2 = load("b2", [64, 1])
    b3 = load("b3", [64, 1])

    ident = persist.tile([128, 128], FP, tag="ident")
    masks.make_identity(nc, ident[:])
    identb = persist.tile([128, 128], BF, tag="identb")
    masks.make_identity(nc, identb[:])
    ones128 = persist.tile([128, 1], BF, tag="ones128")
    nc.gpsimd.memset(ones128[:], 1.0)
    ones64 = persist.tile([64, 1], BF, tag="ones64")
    nc.gpsimd.memset(ones64[:], 1.0)
    epsc = persist.tile([128, 1], FP, tag="epsc")
    nc.gpsimd.memset(epsc[:], EPS)
    onesrow = persist.tile([1, 128], BF, tag="onesrow")
    nc.gpsimd.memset(onesrow[:], 1.0)
    onesrow_f = persist.tile([1, 128], FP, tag="onesrow_f")
    nc.gpsimd.memset(onesrow_f[:], 1.0)

    # [1,64] param rows; broadcast to [128,64] later via K=1 matmuls
    _brows = {}
    _btiles = {}

    def bcast64(name):
        row = persist.tile([1, 64], FP, tag=f"{name}_row")
        nc.sync.dma_start(row[:], io[name][:])
        _brows[name] = row
        t = persist.tile([128, 64], FP, tag=f"{name}_b")
        _btiles[name] = t
        return t

    bngB = bcast64("bng")
    bnbB = bcast64("bnb")
    xngB = bcast64("xng")
    xnbB = bcast64("xnb")
    lngB = bcast64("lng")
    lnbB = bcast64("lnb")

    # ---- big persistent tiles ------------------------------------------
    front_ctx = ExitStack()
    front = front_ctx.enter_context(tc.tile_pool(name="front", bufs=1))
    xT = front.tile([64, N], FP, tag="xT")        # x^T
    c1 = persist.tile([128, N], FP, tag="c1")       # [x3^T ; last^T]
    c2 = front.tile([128, N], FP, tag="c2")       # [(r*last)^T ; x3^T]
    x1T = front.tile([16, N], FP, tag="x1T")
    x2T = front.tile([2, N], FP, tag="x2T")
    hg0T = front.tile([64, N], FP, tag="hg0T")
    lastT0 = front.tile([64, N], FP, tag="lastT0")  # base-0 copy for DVE gate math
    hgR = front.tile([128, N // 2], FP, tag="hgR")   # Hg rows, [p, 16*64]
    hgT = persist.tile([64, N], FP, tag="hgT")
    qT0 = persist.tile([128, HALF], BF, tag="qT0")
    qT1 = persist.tile([128, HALF], BF, tag="qT1")
    kT0 = persist.tile([128, N], BF, tag="kT0")
    kT1 = persist.tile([128, N], BF, tag="kT1")
    k2T0 = persist.tile([128, N], BF, tag="k2T0")
    k2T1 = persist.tile([128, N], BF, tag="k2T1")
    x3gs = persist.tile([97, N], BF, tag="x3gs")    # [x3^T*gs ; ga ; gs ; cbv ; ...; gs@96]
    gtop = persist.tile([128, 256], BF, tag="gtop")
    gbot = persist.tile([128, 256], BF, tag="gbot")
    gs_f = persist.tile([64, 64], BF, tag="gs_f")
    e0 = persist.tile([128, HALF], BF, tag="e0")
    e1 = persist.tile([128, HALF], BF, tag="e1")
    es = persist.tile([64, HALF], BF, tag="es")
    ks0 = persist.tile([128, 1], BF, tag="ks0")
    ks1 = persist.tile([128, 1], BF, tag="ks1")
    xs_f = persist.tile([64, 1], BF, tag="xs_f")
    x3b = persist.tile([64, N], BF, tag="x3b")   # bf16 copy of x3^T for the soc path
    qs0 = persist.tile([128, HALF], BF, tag="qs0")   # rstdA-scaled q rows
    qs1 = persist.tile([128, HALF], BF, tag="qs1")
    x3s = persist.tile([64, HALF], BF, tag="x3s")    # rstdS-scaled x3 rows
    rc32 = persist.tile([128, 4 * HCH], FP, tag="rc32")
    dg8 = persist.tile([128, HCH], FP, tag="dg8")
    st8 = persist.tile([128, 40], FP, tag="st8")
    dl = persist.tile([128, HCH], FP, tag="dl")

    # ================= phase B: load + transpose x, last ================
    with tc.tile_pool(name="tpps", bufs=2, space="PSUM") as tpps:
        for _nm in ("bng", "bnb", "xng", "xnb", "lng", "lnb"):
            _bp = tpps.tile([128, 64], FP, tag="pt")
            nc.tensor.matmul(_bp[:], onesrow_f[:], _brows[_nm][:], start=True, stop=True)
            nc.vector.tensor_copy(_btiles[_nm][:], _bp[:])
        xin_all = front.tile([128, NCH * 64], FP, tag="xin_all")
        nc.sync.dma_start(xin_all[:].rearrange("p (i e) -> p i e", e=64),
                          io["xK"].rearrange("(i p) e -> p i e", p=128))
        lin_all = front.tile([128, NCH * 64], FP, tag="lin_all")
        nc.sync.dma_start(lin_all[:].rearrange("p (i e) -> p i e", e=64),
                          io["lastK"].rearrange("(i p) e -> p i e", p=128))
        for i in range(NCH):
            pt = tpps.tile([64, 128], FP, tag="pt")
            _tp(nc, pt[:], xin_all[:, i * 64:(i + 1) * 64], ident)
            eng = nc.vector if i % 2 == 0 else nc.scalar
            (eng.tensor_copy if i % 2 == 0 else eng.copy)(_w(xT[0:64, i * 128:(i + 1) * 128]), pt[:])
        for i in range(NCH):
            pt = tpps.tile([64, 128], FP, tag="pt")
            _tp(nc, pt[:], lin_all[:, i * 64:(i + 1) * 64], ident)
            eng = nc.vector if i % 2 == 0 else nc.scalar
            (eng.tensor_copy if i % 2 == 0 else eng.copy)(_w(c1[64:128, i * 128:(i + 1) * 128]), pt[:])

        nc.vector.tensor_copy(lastT0[:], c1[64:128, :])

        # ============= phase C: hyper fc stack (per 512 block) ==========
        with tc.tile_pool(name="fps", bufs=3, space="PSUM") as fps:
            for j in range(NMB):
                sl = slice(j * MB, (j + 1) * MB)
                p1 = fps.tile([16, MB], FP, tag="fp")
                nc.tensor.matmul(p1[:], _r(fc1s[:]), _r(xT[:, sl]), start=True, stop=True)
                nc.scalar.activation(_w(x1T[0:16, sl]), p1[:], AF.Sigmoid, bias=fc1b[:, 0:1])
                p2 = fps.tile([2, MB], FP, tag="fp")
                nc.tensor.matmul(p2[:], _r(fc2s[:]), _r(x1T[:, sl]), start=True, stop=True)
                nc.scalar.activation(_w(x2T[0:2, sl]), p2[:], AF.Sigmoid, bias=fc2b[:, 0:1])
                p3 = fps.tile([64, MB], FP, tag="fp")
                nc.tensor.matmul(p3[:], _r(fc3s[:]), _r(x2T[:, sl]), start=True, stop=True)
                nc.scalar.activation(_w(c1[0:64, sl]), p3[:], AF.Identity, bias=fc3b[:, 0:1])
                nc.vector.tensor_copy(_w(c2[64:128, sl]), c1[0:64, sl])

            nc.vector.tensor_copy(x3b[:], c1[0:64, :])

            # ============= phase D: GRU gates (per 512 block) ===========
            for j in range(NMB):
                sl = slice(j * MB, (j + 1) * MB)
                zp = fps.tile([64, MB], FP, tag="fp")
                nc.tensor.matmul(zp[:], _r(wz[:]), _r(c1[:, sl]), start=True, stop=True)
                zt = tp_in.tile([64, MB], FP, tag="zt")
                nc.scalar.activation(zt[:], zp[:], AF.Sigmoid)
                rp = fps.tile([64, MB], FP, tag="fp")
                nc.tensor.matmul(rp[:], _r(wr[:]), _r(c1[:, sl]), start=True, stop=True)
                rt = tp_in.tile([64, MB], FP, tag="rt")
                nc.scalar.activation(rt[:], rp[:], AF.Sigmoid)
                nc.vector.tensor_tensor(_w(c2[0:64, sl]), rt[:], lastT0[:, sl], AL.mult)
                hp = fps.tile([64, MB], FP, tag="fp")
                nc.tensor.matmul(hp[:], _r(wh[:]), _r(c2[:, sl]), start=True, stop=True)
                ht = tp_in.tile([64, MB], FP, tag="ht")
                nc.scalar.activation(ht[:], hp[:], AF.Tanh)
                dt_ = tp_in.tile([64, MB], FP, tag="dt_")
                nc.vector.tensor_tensor(dt_[:], ht[:], lastT0[:, sl], AL.subtract)
                nc.vector.tensor_tensor(dt_[:], dt_[:], zt[:], AL.mult)
                nc.vector.tensor_tensor(hg0T[:, sl], dt_[:], lastT0[:, sl], AL.add)

        # ================ phase E: Hg LayerNorm (row-major) =============
        for i in range(NCH):
            pt = tpps.tile([128, 64], FP, tag="pt")
            _tp(nc, pt[:], hg0T[:, i * 128:(i + 1) * 128], ident)
            eng_i = i % 2
            (nc.vector.tensor_copy if eng_i == 0 else nc.scalar.copy)(
                hgR[:, i * 64:(i + 1) * 64], pt[:])
        _ln_rows(nc, small, hgR, hgR, bngB, bnbB, NCH, "lnh", epsc)
        # 'last' output: rotated rows 0..1023 = chunks 0..7
        nc.sync.dma_start(
            io["lastH"].rearrange("(i p) e -> p i e", p=128),
            hgR[:, 0:HCH * 64].rearrange("p (i e) -> p i e", e=64))
        for i in range(NCH):
            pt = tpps.tile([64, 128], FP, tag="pt")
            _tp(nc, pt[:], hgR[:, i * 64:(i + 1) * 64], ident)
            (nc.vector.tensor_copy if i % 2 == 0 else nc.scalar.copy)(
                _w(hgT[:, i * 128:(i + 1) * 128]), pt[:])

        # ================ phase F: q / k + leaky_relu ===================
        front_ctx.close()
        with tc.tile_pool(name="qkps", bufs=2, space="PSUM") as qkps:
            for (dst, wsl, nblocks) in (
                (qT0, slice(0, 128), 2), (qT1, slice(128, 256), 2),
                (kT0, slice(0, 128), 4), (kT1, slice(128, 256), 4),
            ):
                wt = qw if dst in (qT0, qT1) else kw
                for nb in range(nblocks):
                    sl = slice(nb * MB, (nb + 1) * MB)
                    pq = qkps.tile([128, MB], FP, tag="pq")
                    nc.tensor.matmul(pq[:], (wt[:, wsl]), (hgT[:, sl]), start=True, stop=True)
                    nc.scalar.copy(dst[:, sl], pq[:])
                    nc.vector.scalar_tensor_tensor(dst[:, sl], dst[:, sl], 0.01,
                                                   dst[:, sl], AL.mult, AL.max)

        # ============== phase G: scaled copies + Gram ====================
        nc.gpsimd.dma_start(x3gs[64:67, :], io["corr3"][:])
        with tc.tile_pool(name="gbp", bufs=1, space="PSUM") as gbp, \
             tc.tile_pool(name="grow", bufs=1) as grow:
            garow = grow.tile([1, N], BF, tag="garow")
            nc.gpsimd.dma_start(garow[:], io["corr3"][0:1, :])
            gsrow = grow.tile([1, N], BF, tag="gsrow")
            nc.gpsimd.dma_start(gsrow[:], io["corr3"][1:2, :])
            gaPS = gbp.tile([128, N], FP, tag="gb")
            for j in range(NMB):
                sl = slice(j * MB, (j + 1) * MB)
                nc.tensor.matmul(gaPS[:, sl], onesrow[:], garow[:, sl],
                                 start=True, stop=True)
            nc.vector.tensor_tensor(k2T0[:], kT0[:], gaPS[:], AL.mult)
            nc.vector.tensor_tensor(k2T1[:], kT1[:], gaPS[:], AL.mult)
            gsPS = gbp.tile([64, N], FP, tag="gb")
            for j in range(NMB):
                sl = slice(j * MB, (j + 1) * MB)
                nc.tensor.matmul(gsPS[:, sl], onesrow[0:1, 0:64], gsrow[:, sl],
                                 start=True, stop=True)
            nc.vector.tensor_tensor(x3gs[0:64, :], c1[0:64, :], gsPS[:], AL.mult)
        with nc.allow_low_precision(reason="bf16 row-sums feed LN stats via the "
                                     "dL path only; error is attenuated ~1e-4x"):
            nc.vector.tensor_reduce(ks0[:], kT0[:], AX.X, AL.add)
            nc.vector.tensor_reduce(ks1[:], kT1[:], AX.X, AL.add)
            nc.vector.tensor_reduce(xs_f[:], x3b[:], AX.X, AL.add)

        with tc.tile_pool(name="gps", bufs=2, space="PSUM") as gps, \
             tc.tile_pool(name="krb", bufs=2, space="PSUM") as krb, \
             tc.tile_pool(name="krp", bufs=2) as krp, \
             tc.tile_pool(name="utps", bufs=1, space="PSUM") as utps:
            gt_ps = gps.tile([128, 256], FP, tag="g")
            gb_ps = gps.tile([128, 256], FP, tag="g")
            for mi in range(NCH):
                msl = slice(mi * 128, (mi + 1) * 128)
                kr = krp.tile([128, 256], BF, tag="kr")
                pt = krb.tile([128, 128], BF, tag="ptb")
                _tp(nc, pt[:], kT0[:, msl], identb)
                nc.vector.tensor_copy(kr[:, 0:128], pt[:])
                pt = krb.tile([128, 128], BF, tag="ptb")
                _tp(nc, pt[:], kT1[:, msl], identb)
                nc.scalar.copy(kr[:, 128:256], pt[:])
                nc.tensor.matmul(gt_ps[:], kr[:, 0:128], kr[:],
                                 start=(mi == 0), stop=(mi == NCH - 1))
                nc.tensor.matmul(gb_ps[:], kr[:, 128:256], kr[:],
                                 start=(mi == 0), stop=(mi == NCH - 1))
            nc.vector.tensor_copy(gtop[:], gt_ps[:])
            nc.scalar.copy(gbot[:], gb_ps[:])
            gs_ps = gps.tile([64, 64], FP, tag="g")
            for mi in range(NCH):
                msl = slice(mi * 128, (mi + 1) * 128)
                xr = krp.tile([128, 64], BF, tag="xr")
                pt = krb.tile([128, 64], BF, tag="ptb")
                _tp(nc, pt[:], x3b[:, msl], identb)
                nc.vector.tensor_copy(xr[:], pt[:])
                nc.tensor.matmul(gs_ps[:], xr[:], xr[:],
                                 start=(mi == 0), stop=(mi == NCH - 1))
            nc.vector.tensor_copy(gs_f[:], gs_ps[:])

            # u = G q' ; e = u * q'   (sum(att^2) per row via ones-matmul later)
            ut = utps.tile([128, HALF], FP, tag="ut")
            for nb in range(2):
                sl = slice(nb * MB, (nb + 1) * MB)
                nc.tensor.matmul(ut[:, sl], gtop[:, 0:128], qT0[:, sl], start=True, stop=False)
                nc.tensor.matmul(ut[:, sl], gbot[:, 0:128], qT1[:, sl], start=False, stop=True)
            nc.vector.tensor_tensor(e0[:], ut[:], qT0[:], AL.mult)
            ut = utps.tile([128, HALF], FP, tag="ut")
            for nb in range(2):
                sl = slice(nb * MB, (nb + 1) * MB)
                nc.tensor.matmul(ut[:, sl], gtop[:, 128:256], qT0[:, sl], start=True, stop=False)
                nc.tensor.matmul(ut[:, sl], gbot[:, 128:256], qT1[:, sl], start=False, stop=True)
            nc.vector.tensor_tensor(e1[:], ut[:], qT1[:], AL.mult)
            us = utps.tile([64, HALF], FP, tag="ut")
            for nb in range(2):
                sl = slice(nb * MB, (nb + 1) * MB)
                nc.tensor.matmul(us[:, sl], gs_f[:], x3b[:, sl], start=True, stop=True)
            nc.vector.tensor_tensor(es[:], us[:], x3b[:, 0:HALF], AL.mult)

        # ================= phase H: per-chunk LN stats ==================
        with tc.tile_pool(name="sps", bufs=1, space="PSUM") as sps, \
             tc.tile_pool(name="rbp", bufs=2, space="PSUM") as rbp:
            p32 = sps.tile([128, 32], FP, tag="p32")
            for i in range(HCH):
                csl = slice(i * 128, (i + 1) * 128)
                nc.tensor.matmul(p32[:, i:i + 1], qT0[:, csl], ks0[:], start=True, stop=False)
                nc.tensor.matmul(p32[:, i:i + 1], qT1[:, csl], ks1[:], start=False, stop=True)
                nc.tensor.matmul(p32[:, 8 + i:9 + i], x3b[:, csl], xs_f[:],
                                 start=True, stop=True)
                nc.tensor.matmul(p32[:, 16 + i:17 + i], e0[:, csl], ones128[:], start=True, stop=False)
                nc.tensor.matmul(p32[:, 16 + i:17 + i], e1[:, csl], ones128[:], start=False, stop=True)
                nc.tensor.matmul(p32[:, 24 + i:25 + i], es[:, csl], ones64[:],
                                 start=True, stop=True)
            ma = small.tile([128, 8], FP, tag="ma")
            nc.vector.tensor_scalar(ma[:], p32[:, 0:8], 1.0 / N, None, AL.mult)
            ms = small.tile([128, 8], FP, tag="ms")
            nc.vector.tensor_scalar(ms[:], p32[:, 8:16], 1.0 / N, None, AL.mult)
            m2 = small.tile([128, 8], FP, tag="m2")
            nc.vector.tensor_tensor(m2[:], ma[:], ma[:], AL.mult)
            va = small.tile([128, 8], FP, tag="va")
            nc.vector.scalar_tensor_tensor(va[:], p32[:, 16:24], 1.0 / N, m2[:], AL.mult, AL.subtract)
            nc.scalar.activation(va[:], va[:], AF.Sqrt, bias=epsc[:, 0:1])
            nc.vector.reciprocal(st8[:, 0:8], va[:])
            nc.vector.tensor_tensor(m2[:], ms[:], ms[:], AL.mult)
            vs = small.tile([128, 8], FP, tag="vs")
            nc.vector.scalar_tensor_tensor(vs[:], p32[:, 24:32], 1.0 / N, m2[:], AL.mult, AL.subtract)
            nc.scalar.activation(vs[:], vs[:], AF.Sqrt, bias=epsc[:, 0:1])
            nc.vector.reciprocal(st8[:, 8:16], vs[:])
            tri = st8[:, 16:40].rearrange("p (i t) -> p i t", t=3)
            nc.vector.scalar_tensor_tensor(tri[:, :, 0:1], ma[:].unsqueeze(2), -1.0,
                                           st8[:, 0:8].unsqueeze(2), AL.mult, AL.mult)
            nc.vector.scalar_tensor_tensor(tri[:, :, 1:2], ms[:].unsqueeze(2), -1.0,
                                           st8[:, 8:16].unsqueeze(2), AL.mult, AL.mult)
            nc.gpsimd.memset(tri[:, :, 2:3], 1.0)

            # pre-scale q / x3 rows by rstd (relu is positively homogeneous,
            # but the two different row scales force the scaling into lhsT)
            tpa = tpps.tile([8, 128], FP, tag="pt")
            _tp(nc, tpa[:], st8[:, 0:8], ident)
            s8a = small.tile([8, 128], BF, tag="s8a")
            nc.vector.tensor_copy(s8a[:], tpa[:])
            tpb = tpps.tile([8, 128], FP, tag="pt")
            _tp(nc, tpb[:], st8[:, 8:16], ident)
            s8b = small.tile([8, 128], BF, tag="s8b")
            nc.vector.tensor_copy(s8b[:], tpb[:])
            ra_row = small.tile([1, HALF], BF, tag="ra_row")
            nc.sync.dma_start(ra_row[:].rearrange("o (i p) -> o i p", p=128), s8a[:])
            rs_row = small.tile([1, HALF], BF, tag="rs_row")
            nc.sync.dma_start(rs_row[:].rearrange("o (i p) -> o i p", p=128), s8b[:])
            rab = rbp.tile([128, HALF], FP, tag="rb")
            for nb in range(2):
                sl = slice(nb * MB, (nb + 1) * MB)
                nc.tensor.matmul(rab[:, sl], onesrow[:], ra_row[:, sl], start=True, stop=True)
            nc.vector.tensor_tensor(qs0[:], qT0[:], rab[:], AL.mult)
            nc.vector.tensor_tensor(qs1[:], qT1[:], rab[:], AL.mult)
            rsb = rbp.tile([64, HALF], FP, tag="rb")
            for nb in range(2):
                sl = slice(nb * MB, (nb + 1) * MB)
                nc.tensor.matmul(rsb[:, sl], onesrow[0:1, 0:64], rs_row[:, sl], start=True, stop=True)
            nc.vector.tensor_tensor(x3s[:], x3b[:, 0:HALF], rsb[:], AL.mult)

    # ==================== phase I: fused adj chunks =====================
    with tc.tile_pool(name="zps", bufs=6, space="PSUM") as zps, \
         tc.tile_pool(name="pstp", bufs=2, space="PSUM") as pstp, \
         tc.tile_pool(name="cw", bufs=2) as cw, \
         tc.tile_pool(name="scrv", bufs=2) as scrv, \
         tc.tile_pool(name="scra", bufs=2) as scra:
        for i in range(HCH):
            csl = slice(i * 128, (i + 1) * 128)
            ptc = pstp.tile([3, 128], FP, tag="pst")
            _tp(nc, ptc[:], st8[:, 16 + 3 * i:19 + 3 * i], ident)
            lc = cw.tile([67, 128], BF, tag="lc")
            nc.vector.tensor_copy(lc[0:64, :], x3s[:, csl])
            nc.vector.tensor_copy(lc[64:67, :], ptc[:])
            for mb in range(NMB):
                msl = slice(mb * MB, (mb + 1) * MB)
                zpt = zps.tile([128, MB], FP, tag="zpt")
                nc.tensor.matmul(zpt[:], qs0[:, csl], k2T0[:, msl], start=True, stop=False)
                nc.tensor.matmul(zpt[:], qs1[:, csl], k2T1[:, msl], start=False, stop=False)
                nc.tensor.matmul(zpt[:], lc[:], x3gs[0:67, msl], start=False, stop=True)
                if mb % 2 == 0:
                    scr = scrv.tile([128, MB], FP, tag="scr")
                    nc.vector.tensor_scalar(scr[:], zpt[:], 0.0, None, AL.max, AL.add,
                                            accum_out=rc32[:, 4 * i + mb:4 * i + mb + 1])
                else:
                    scr = scra.tile([128, MB], FP, tag="scr2")
                    nc.scalar.activation(scr[:], zpt[:], AF.Relu,
                                         accum_out=rc32[:, 4 * i + mb:4 * i + mb + 1])
                if mb == i // 4:
                    off = (i * 128) % MB
                    dsel = cw.tile([128, 128], FP, tag="dsel")
                    nc.gpsimd.affine_select(
                        out=dsel[:], in_=scr[:, off:off + 128], compare_op=AL.is_equal,
                        fill=0.0, base=0, pattern=[[-1, 128]], channel_multiplier=1)
                    nc.vector.tensor_reduce(dg8[:, i:i + 1], dsel[:], AX.X, AL.add)
        rs8 = small.tile([128, HCH], FP, tag="rs8")
        nc.vector.tensor_reduce(rs8[:], rc32[:].rearrange("p (i m) -> p i m", m=4),
                                AX.X, AL.add)
        nc.vector.reciprocal(rs8[:], rs8[:])
        nc.vector.tensor_tensor(dl[:], dg8[:], rs8[:], AL.mult)

    # ======================= phase J: GCN tail ==========================
    with tc.tile_pool(name="jps", bufs=2, space="PSUM") as jps, \
         tc.tile_pool(name="dlp", bufs=1, space="PSUM") as dlp, \
         tc.tile_pool(name="jw", bufs=2) as jw:
        ptd = jps.tile([HCH, 128], FP, tag="jt")
        _tp(nc, ptd[:], dl[:], ident)
        s8d = small.tile([HCH, 128], BF, tag="s8d")
        nc.vector.tensor_copy(s8d[:], ptd[:])
        dl_row = small.tile([1, HALF], BF, tag="dl_row")
        nc.sync.dma_start(dl_row[:].rearrange("o (i p) -> o i p", p=128), s8d[:])
        dlb = dlp.tile([64, HALF], FP, tag="dlb")
        for nb in range(2):
            sl = slice(nb * MB, (nb + 1) * MB)
            nc.tensor.matmul(dlb[:, sl], onesrow[0:1, 0:64], dl_row[:, sl], start=True, stop=True)
        dls = persist.tile([64, HALF], FP, tag="dls")
        nc.scalar.copy(dls[:], dlb[:])

        xo = persist.tile([128, HCH * 64], FP, tag="xo")
        nc.sync.dma_start(xo[:].rearrange("p (i e) -> p i e", e=64),
                          io["origH"].rearrange("(i p) e -> p i e", p=128))
        _ln_rows(nc, small, xo, xo, xngB, xnbB, HCH, "lnx", epsc)
        fin = persist.tile([128, HCH * 64], FP, tag="fin")
        xot = persist.tile([64, HALF], FP, tag="xot")
        for i in range(HCH):
            pt = jps.tile([64, 128], FP, tag="jt")
            _tp(nc, pt[:], xo[:, i * 64:(i + 1) * 64], ident)
            (nc.vector.tensor_copy if i % 2 == 0 else nc.scalar.copy)(
                xot[:, i * 128:(i + 1) * 128], pt[:])
        hc_a = persist.tile([64, HALF], FP, tag="hc_a")
        hc_b = persist.tile([64, HALF], FP, tag="hc_b")
        hcur = xot
        for (w_, b_, dst) in ((w1, b1, hc_a), (w2, b2, hc_b), (w3, b3, hc_a)):
            for nb in range(2):
                sl = slice(nb * MB, (nb + 1) * MB)
                ph = jps.tile([64, MB], FP, tag="ph")
                nc.tensor.matmul(ph[:], w_[:], hcur[:, sl], start=True, stop=True)
                hn = jw.tile([64, MB], FP, tag="hn")
                nc.vector.tensor_tensor(hn[:], ph[:], dls[:, sl], AL.mult)
                nc.scalar.activation(dst[:, sl], hn[:], AF.Identity, bias=b_[:, 0:1])
            hcur = dst
        nc.vector.tensor_tensor(hc_a[:], hc_a[:], xot[:], AL.add)
        for i in range(HCH):
            pt = jps.tile([128, 64], FP, tag="jt")
            _tp(nc, pt[:], hc_a[:, i * 128:(i + 1) * 128], ident)
            (nc.vector.tensor_copy if i % 2 == 0 else nc.scalar.copy)(
                fin[:, i * 64:(i + 1) * 64], pt[:])
        _ln_rows(nc, small, fin, fin, lngB, lnbB, HCH, "lnf", epsc)
        nc.sync.dma_start(io["outH"].rearrange("(i p) e -> p i e", p=128),
                          fin[:].rearrange("p (i e) -> p i e", e=64))

def _build():
    if "nc" in _CACHE:
        return _CACHE["nc"]
    nc = bacc.Bacc("TRN2", target_bir_lowering=False, debug=False,
                   enable_asserts=True, num_devices=8)
    io = {}

    def din(name, shape):
        io[name] = nc.dram_tensor(name, shape, FP, kind="ExternalInput").ap()

    def dout(name, shape):
        io[name] = nc.dram_tensor(name, shape, FP, kind="ExternalOutput").ap()

    din("xK", [N, G])
    din("lastK", [N, G])
    din("origH", [HALF, E])
    din("fc1s", [64, 16])
    din("fc2s", [16, 2])
    din("fc3s", [2, 64])
    din("wz", [128, 64])
    din("wr", [128, 64])
    din("wh", [128, 64])
    din("qw", [64, 256])
    din("kw", [64, 256])
    din("corr3", [3, N])
    din("w1", [64, 64])
    din("w2", [64, 64])
    din("w3", [64, 64])
    din("b1", [64, 1])
    din("fc1b", [16, 1])
    din("fc2b", [2, 1])
    din("fc3b", [64, 1])
    din("b2", [64, 1])
    din("b3", [64, 1])
    for nm in ("bng", "bnb", "xng", "xnb", "lng", "lnb"):
        din(nm, [1, 64])
    dout("outH", [HALF, E])
    dout("lastH", [HALF, G])

    with tile.TileContext(nc) as tc:
        with ExitStack() as ctx:
            _emit(ctx, tc, io)
    nc.compile()
    nc.m = get_hw_module(nc.m)
    _CACHE["nc"] = nc
    return nc


def _host_prep(inputs):
    f32 = np.float32
    inp = {k: np.ascontiguousarray(np.asarray(v, f32)) for k, v in inputs.items()}
    ch = 1.0 + inp["mlp_w"].sum(axis=0)
    assert (ch > 0).all(), "head-mixing scale fold requires positive c_h"
    qw_eff = (inp["q_w"] * np.repeat(ch / np.sqrt(G), G)[None, :]).astype(f32)
    fc2s = inp["fc2_w"]
    fc3s = inp["fc3_w"]
    shared = {
        "fc1s": inp["fc1_w"], "fc2s": fc2s, "fc3s": fc3s,
        "fc1b": inp["fc1_b"][:, None],
        "fc2b": inp["fc2_b"][:, None], "fc3b": inp["fc3_b"][:, None],
        "wz": inp["w_z"], "wr": inp["w_r"], "wh": inp["w_h"],
        "qw": qw_eff, "kw": inp["k_w"],
        "w1": inp["gcn_w1"], "w2": inp["gcn_w2"], "w3": inp["gcn_w3"],
        "b1": inp["gcn_b1"][:, None], "b2": inp["gcn_b2"][:, None],
        "b3": inp["gcn_b3"][:, None],
        "bng": inp["bn_g"][None, :], "bnb": inp["bn_b"][None, :],
        "xng": inp["x_nom_g"][None, :], "xnb": inp["x_nom_b"][None, :],
        "lng": inp["last_nom_g"][None, :], "lnb": inp["last_nom_b"][None, :],
    }
    shared = {k: np.ascontiguousarray(v, f32) for k, v in shared.items()}
    in_maps = []
    for core in range(8):
        b, h = core // 2, core % 2
        off = h * HALF
        rot = lambda a: np.ascontiguousarray(np.roll(a, -off, axis=0), f32)
        corr3 = np.stack([
            np.roll(inp["attn_norm_g"], -off),
            np.roll(inp["skip_norm_g"], -off),
            np.roll(inp["attn_norm_b"] + inp["skip_norm_b"], -off),
        ]).astype(f32)
        m = dict(shared)
        m["xK"] = rot(inp["x"][b])
        m["lastK"] = rot(inp["last_G_emb"][b])
        m["origH"] = np.ascontiguousarray(inp["orig_x"][b, off:off + HALF], f32)
        m["corr3"] = np.ascontiguousarray(corr3)
        in_maps.append(m)
    return in_maps


def run(inputs, trace=False):
    nc = _build()
    in_maps = _host_prep(inputs)
    res = run_bass_kernel_spmd(nc, in_maps, core_ids=list(range(8)), trace=trace)
    out = np.zeros((B, N, E), np.float32)
    last = np.zeros((B, N, G), np.float32)
    for core in range(8):
        b, h = core // 2, core % 2
        off = h * HALF
        out[b, off:off + HALF] = res.results[core]["outH"]
        last[b, off:off + HALF] = res.results[core]["lastH"]
    return (out, last), res


def kernel(**inputs):
    return run(inputs)[0]



# revision 7
# speedup vs baseline: 1.2211x; 1.2211x over previous
"""Trainium2 Bass kernel for nn_DGCN (gnn_message_passing).

Sharding: 8 shards = (batch b in 0..3, row-half h in 0..1). Each core gets
the full 2048-node K-side tensors of its batch with the node axis ROTATED
by h*1024 so the adjacency diagonal lands at the same tile position on
every core (uniform SPMD program); the core computes rows 0..1023 of the
rotated order, which are rows [h*1024, (h+1)*1024) of the original order.

Restructuring vs the reference (v2 — latency-oriented rewrite):
 - All heavy tensors bf16; host pre-transposes x/last (no on-device input
   transposes) and pre-folds: head-mix scalars c_h and the Hg-LayerNorm
   gain into q_w/k_w; the LN shift becomes an extra contraction row of an
   augmented [66 x 256] weight (rhs rows = [Hg*a ; c ; 1]).
 - Per-node LN statistics come from matmuls (ones/selector weights), are
   moved into a [16,128]/[8,128] chunk layout by SBUF-SBUF DMA so the
   pointwise stats math runs on 16 partitions instead of one, and are
   scattered back as rows that feed K=1 broadcast matmuls or gpsimd
   partition_broadcast.
 - diag(L)_i = relu(bracket)_ii / rowsum_j relu(bracket)_ij is invariant
   to any positive per-row scale, so the 1/sdA row scaling of the fused
   pre-relu matrix cancels and is dropped; only the x3 lhsT rows carry
   the rsS/rsA ratio and the stat rows carry {-muA, -rho*muS, sdA}.
 - Row-sums of relu come from accum_out on the relu ops; partition-axis
   sums (att row sum-of-squares) run on gpsimd partition_all_reduce.
 - GCN biases ride the next layer's matmul as host-folded b@W rows paired
   with a constant ones row.
"""

import sys

if '/opt/trn_rl_repo' not in sys.path:
    sys.path.insert(0, '/opt/trn_rl_repo')

from contextlib import ExitStack

import numpy as np
import ml_dtypes

import concourse.bass as bass
import concourse.tile as tile
from concourse import bacc, bass_isa, masks, mybir
from concourse.bass_interp import get_hw_module
from concourse.bass_utils import run_bass_kernel_spmd

FP = mybir.dt.float32
BF = mybir.dt.bfloat16
AL = mybir.AluOpType
AF = mybir.ActivationFunctionType
AX = mybir.AxisListType
RED = bass_isa.ReduceOp

B, N, E, G, H = 4, 2048, 64, 64, 4
D = H * G          # 256
HALF = N // 2      # own rows per core
NCH = N // 128     # 16 chunks over all nodes
HCH = HALF // 128  # 8 own chunks
MB = 512
NMB = N // MB      # 4
EPS = 1e-5

_CACHE = {}


def _tp(nc, out_ap, in_ap, ident):
    k = in_ap.partition_size()
    nc.tensor.transpose(out_ap, in_ap, ident[0:k, 0:k])


def _leaky(nc, dst):
    nc.vector.scalar_tensor_tensor(dst, dst, 0.01, dst, AL.mult, AL.max)


def _ln_rows(nc, pool, t_in, t_out, g_b, b_b, ngr, tag, epsc):
    """LayerNorm over 64-wide groups: t_in [128, ngr*64] -> t_out."""
    a3 = t_in[:].rearrange("p (g e) -> p g e", e=64)
    o3 = t_out[:].rearrange("p (g e) -> p g e", e=64)
    sm = pool.tile([128, ngr], FP, tag=f"{tag}_sm")
    nc.vector.tensor_reduce(sm[:], a3, AX.X, AL.add)
    sq = pool.tile([128, ngr * 64], FP, tag=f"{tag}_sq")
    nc.scalar.square(sq[:], t_in[:])
    sqs = pool.tile([128, ngr], FP, tag=f"{tag}_sqs")
    nc.vector.tensor_reduce(sqs[:], sq[:].rearrange("p (g e) -> p g e", e=64),
                            AX.X, AL.add)
    mu = pool.tile([128, ngr], FP, tag=f"{tag}_mu")
    nc.vector.tensor_scalar(mu[:], sm[:], 1.0 / 64, None, AL.mult)
    mu2 = pool.tile([128, ngr], FP, tag=f"{tag}_mu2")
    nc.vector.tensor_tensor(mu2[:], mu[:], mu[:], AL.mult)
    var = pool.tile([128, ngr], FP, tag=f"{tag}_var")
    nc.vector.scalar_tensor_tensor(var[:], sqs[:], 1.0 / 64, mu2[:], AL.mult,
                                   AL.subtract)
    sd = pool.tile([128, ngr], FP, tag=f"{tag}_sd")
    nc.scalar.activation(sd[:], var[:], AF.Sqrt, bias=epsc[0:128, 0:1])
    rs = pool.tile([128, ngr], FP, tag=f"{tag}_rs")
    nc.vector.reciprocal(rs[:], sd[:])
    mu_b = mu[:].unsqueeze(2).broadcast_to([128, ngr, 64])
    rs_b = rs[:].unsqueeze(2).broadcast_to([128, ngr, 64])
    g3 = g_b[:].unsqueeze(1).broadcast_to([128, ngr, 64])
    b3 = b_b[:].unsqueeze(1).broadcast_to([128, ngr, 64])
    xc = pool.tile([128, ngr * 64], FP, tag=f"{tag}_xc")
    xc3 = xc[:].rearrange("p (g e) -> p g e", e=64)
    nc.vector.tensor_tensor(xc3, a3, mu_b, AL.subtract)
    nc.vector.tensor_tensor(xc3, xc3, rs_b, AL.mult)
    nc.vector.tensor_tensor(xc3, xc3, g3, AL.mult)
    nc.vector.tensor_tensor(o3, xc3, b3, AL.add)


def _emit(ctx: ExitStack, tc: tile.TileContext, io: dict):
    nc = tc.nc

    persist = ctx.enter_context(tc.tile_pool(name="persist", bufs=1))
    small = ctx.enter_context(tc.tile_pool(name="small", bufs=1))

    # ---------------- params ----------------
    def load(name, shape, dt=BF):
        t = persist.tile(shape, dt, tag=name)
        nc.gpsimd.dma_start(t[:], io[name][:])
        return t

    fc1s = load("fc1s", [64, 16])
    fc2s = load("fc2s", [16, 2])
    fc3s = load("fc3s", [2, 64])
    fc3r = load("fc3r", [1, 64])
    wz = load("wz", [128, 64])
    wr = load("wr", [128, 64])
    wh = load("wh", [128, 64])
    qA = load("qA", [66, 256])
    kA = load("kA", [66, 256])
    w1a = load("w1a", [65, 64])
    w2a = load("w2a", [65, 64])
    w3a = load("w3a", [65, 64])
    fc1b = load("fc1b", [16, 1], FP)
    fc2b = load("fc2b", [2, 1], FP)
    fc3b = load("fc3b", [64, 1], FP)
    b3c = load("b3c", [64, 1], FP)
    ga_r = load("ga_r", [1, N])
    gs_r = load("gs_r", [1, N])

    # LN parameter rows -> [128, 64] broadcast tiles via gpsimd
    brows = {}
    for nm in ("bng", "bnb", "xng", "xnb", "lng", "lnb"):
        r = persist.tile([1, 64], FP, tag=f"{nm}_r")
        nc.gpsimd.dma_start(r[:], io[nm][:])
        t = persist.tile([128, 64], FP, tag=f"{nm}_b")
        nc.gpsimd.partition_broadcast(t[:], r[:])
        brows[nm] = t

    ident = persist.tile([128, 128], FP, tag="ident")
    masks.make_identity(nc, ident[:])
    identb = persist.tile([128, 128], BF, tag="identb")
    masks.make_identity(nc, identb[:])
    epsc = persist.tile([128, 1], FP, tag="epsc")
    nc.gpsimd.memset(epsc[:], EPS)
    onesrow64 = persist.tile([1, 64], BF, tag="onesrow64")
    nc.gpsimd.memset(onesrow64[:], 1.0)
    sel2 = persist.tile([128, 2], BF, tag="sel2")
    nc.gpsimd.memset(sel2[:], 0.0)
    nc.gpsimd.memset(sel2[0:64, 0:1], 1.0)
    nc.gpsimd.memset(sel2[64:128, 1:2], 1.0)

    # ---------------- big persistent tiles ----------------
    xT = persist.tile([64, N], BF, tag="xT")
    lastT = persist.tile([64, N], BF, tag="lastT")
    c1 = persist.tile([128, N], BF, tag="c1")      # [x3 ; last]
    c2 = persist.tile([128, N], BF, tag="c2")      # [r*last ; x3]
    hgsq = persist.tile([128, N], BF, tag="hgsq")  # [Hg_raw ; Hg_raw^2]
    HgQ = persist.tile([66, N], BF, tag="HgQ")     # [Hg*a ; c ; 1]
    a_row = persist.tile([1, N], BF, tag="a_row")
    kT0 = persist.tile([128, N], BF, tag="kT0")
    kT1 = persist.tile([128, N], BF, tag="kT1")
    k2T0 = persist.tile([128, N], BF, tag="k2T0")
    k2T1 = persist.tile([128, N], BF, tag="k2T1")
    qT0 = persist.tile([128, HALF], BF, tag="qT0")
    qT1 = persist.tile([128, HALF], BF, tag="qT1")
    x3gs = persist.tile([67, N], BF, tag="x3gs")   # [x3*gs ; ga ; gs ; cb]
    x3rA = persist.tile([67, HALF], BF, tag="x3rA")  # [x3*rho ; -muA ; -rho*muS ; sdA]
    ga_b = persist.tile([128, N], BF, tag="ga_b")
    gs_b = persist.tile([64, N], BF, tag="gs_b")
    gt_sb = persist.tile([128, 256], BF, tag="gt_sb")
    gb_sb = persist.tile([128, 256], BF, tag="gb_sb")
    gs_f = persist.tile([64, 64], BF, tag="gs_f")
    ks0 = persist.tile([128, 1], BF, tag="ks0")
    ks1 = persist.tile([128, 1], BF, tag="ks1")
    xsb = persist.tile([64, 1], BF, tag="xsb")
    rc32 = persist.tile([128, 4 * HCH], FP, tag="rc32")
    dg8 = persist.tile([128, HCH], FP, tag="dg8")
    dl = persist.tile([128, HCH], FP, tag="dl")
    cT_sb = persist.tile([128, NCH], FP, tag="cT_sb")
    x1T = persist.tile([16, N], BF, tag="x1T")
    x2a = persist.tile([3, N], BF, tag="x2a")      # [x2 ; 1]
    fc3a = persist.tile([3, 64], BF, tag="fc3a")   # [fc3 ; fc3_b]
    e01 = persist.tile([128, HALF], FP, tag="e01")
    e01r = persist.tile([128, HALF], FP, tag="e01r")
    esr = persist.tile([64, HALF], FP, tag="esr")
    lastR = persist.tile([128, HCH * 64], FP, tag="lastR")
    xo = persist.tile([128, HCH * 64], FP, tag="xo")
    x1aug = persist.tile([65, HALF], BF, tag="x1aug")  # [xo^T + b3 ; 1]
    hca = persist.tile([65, HALF], BF, tag="hca")
    hcb = persist.tile([65, HALF], BF, tag="hcb")
    fin = persist.tile([128, HCH * 64], FP, tag="fin")

    onesN = persist.tile([1, N], BF, tag="onesN")
    nc.gpsimd.memset(onesN[:], 1.0)
    nc.sync.dma_start(HgQ[65:66, :], onesN[:])
    nc.sync.dma_start(x2a[2:3, :], onesN[:])
    nc.gpsimd.memset(x1aug[64:65, :], 1.0)
    nc.gpsimd.memset(hca[64:65, :], 1.0)
    nc.gpsimd.memset(hcb[64:65, :], 1.0)

    # input loads
    nc.sync.dma_start(xT[:], io["xT"][:])
    nc.sync.dma_start(lastT[:], io["lastT"][:])
    nc.sync.dma_start(c1[64:128, :], io["lastT"][:])
    nc.sync.dma_start(x3gs[64:67, :], io["corr3"][:])
    nc.sync.dma_start(xo[:].rearrange("p (i e) -> p i e", e=64),
                      io["origH"].rearrange("(i p) e -> p i e", p=128))
    nc.gpsimd.partition_broadcast(ga_b[:], ga_r[:])
    nc.gpsimd.partition_broadcast(gs_b[:], gs_r[:])
    nc.gpsimd.dma_start(fc3a[0:2, :], io["fc3s"][:])
    nc.gpsimd.dma_start(fc3a[2:3, :], io["fc3r"][:])

    frontA = ExitStack()
    fps = frontA.enter_context(tc.tile_pool(name="fps", bufs=4, space="PSUM"))
    gw = frontA.enter_context(tc.tile_pool(name="gw", bufs=3))

    # ============ hyper fc stack (T layout, per 512 block) ============
    xacc = small.tile([64, NMB], FP, tag="xacc")
    for j in range(NMB):
        sl = slice(j * MB, (j + 1) * MB)
        p1 = fps.tile([16, MB], FP, tag="fp")
        nc.tensor.matmul(p1[:], fc1s[:], xT[:, sl], start=True, stop=True)
        nc.scalar.activation(x1T[:, sl], p1[:], AF.Sigmoid, bias=fc1b[:, 0:1])
        p2 = fps.tile([2, MB], FP, tag="fp")
        nc.tensor.matmul(p2[:], fc2s[:], x1T[:, sl], start=True, stop=True)
        nc.scalar.activation(x2a[0:2, sl], p2[:], AF.Sigmoid, bias=fc2b[:, 0:1])
        p3 = fps.tile([64, MB], FP, tag="fp")
        nc.tensor.matmul(p3[:], fc3s[:], x2a[0:2, sl], start=True, stop=True)
        nc.scalar.activation(c1[0:64, sl], p3[:], AF.Identity, bias=fc3b[:, 0:1],
                             accum_out=xacc[:, j:j + 1])
        nc.vector.tensor_copy(c2[64:128, sl], c1[0:64, sl])

    xs_f = small.tile([64, 1], FP, tag="xs_f")
    nc.vector.tensor_reduce(xs_f[:], xacc[:], AX.X, AL.add)
    nc.vector.tensor_copy(xsb[:], xs_f[:])

    # ================= GRU gates =================
    for j in range(NMB):
        sl = slice(j * MB, (j + 1) * MB)
        zp = fps.tile([64, MB], FP, tag="fp")
        nc.tensor.matmul(zp[:], wz[:], c1[:, sl], start=True, stop=True)
        zt = gw.tile([64, MB], BF, tag="zt")
        nc.scalar.activation(zt[:], zp[:], AF.Sigmoid)
        rp = fps.tile([64, MB], FP, tag="fp")
        nc.tensor.matmul(rp[:], wr[:], c1[:, sl], start=True, stop=True)
        rt = gw.tile([64, MB], BF, tag="rt")
        nc.scalar.activation(rt[:], rp[:], AF.Sigmoid)
        nc.gpsimd.tensor_tensor(c2[0:64, sl], rt[:], lastT[:, sl], AL.mult)
        hp = fps.tile([64, MB], FP, tag="fp")
        nc.tensor.matmul(hp[:], wh[:], c2[:, sl], start=True, stop=True)
        ht = gw.tile([64, MB], BF, tag="ht")
        nc.scalar.activation(ht[:], hp[:], AF.Tanh)
        dt_ = gw.tile([64, MB], BF, tag="dt_")
        nc.vector.tensor_tensor(dt_[:], ht[:], lastT[:, sl], AL.subtract)
        nc.vector.tensor_tensor(dt_[:], dt_[:], zt[:], AL.mult)
        nc.vector.tensor_tensor(hgsq[0:64, sl], dt_[:], lastT[:, sl], AL.add)

    # Hg^2 into partitions 64:128 (scalar engine)
    nc.scalar.square(hgsq[64:128, 0:HALF], hgsq[0:64, 0:HALF])
    nc.scalar.square(hgsq[64:128, HALF:N], hgsq[0:64, HALF:N])

    # ============ Hg LN stats via matmul + [16,128] stats-land ============
    sum_sb = small.tile([2, N], FP, tag="sum_sb")
    for j in range(NMB):
        sl = slice(j * MB, (j + 1) * MB)
        sp = fps.tile([2, MB], FP, tag="fp")
        nc.tensor.matmul(sp[:], sel2[:], hgsq[:, sl], start=True, stop=True)
        (nc.vector.tensor_copy if j % 2 == 0 else nc.scalar.copy)(sum_sb[:, sl], sp[:])
    hst0 = small.tile([16, 128], FP, tag="hst0")
    nc.sync.dma_start(hst0[:], sum_sb[0:1, :].rearrange("o (i p) -> o i p", p=128))
    hst1 = small.tile([16, 128], FP, tag="hst1")
    nc.sync.dma_start(hst1[:], sum_sb[1:2, :].rearrange("o (i p) -> o i p", p=128))
    s0, s1 = hst0[:], hst1[:]
    hmu = small.tile([16, 128], FP, tag="hmu")
    nc.vector.tensor_scalar(hmu[:], s0, 1.0 / 64, None, AL.mult)
    hmu2 = small.tile([16, 128], FP, tag="hmu2")
    nc.vector.tensor_tensor(hmu2[:], hmu[:], hmu[:], AL.mult)
    hvar = small.tile([16, 128], FP, tag="hvar")
    nc.vector.scalar_tensor_tensor(hvar[:], s1, 1.0 / 64, hmu2[:], AL.mult, AL.subtract)
    hsd = small.tile([16, 128], FP, tag="hsd")
    nc.scalar.activation(hsd[:], hvar[:], AF.Sqrt, bias=epsc[0:16, 0:1])
    ha = small.tile([16, 128], FP, tag="ha")
    nc.vector.reciprocal(ha[:], hsd[:])
    hc = small.tile([16, 128], FP, tag="hc")
    nc.vector.scalar_tensor_tensor(hc[:], hmu[:], -1.0, ha[:], AL.mult, AL.mult)
    ha16 = small.tile([16, 128], BF, tag="ha16")
    nc.vector.tensor_copy(ha16[:], ha[:])
    hc16 = small.tile([16, 128], BF, tag="hc16")
    nc.scalar.copy(hc16[:], hc[:])
    nc.sync.dma_start(a_row[:].rearrange("o (i p) -> o i p", p=128), ha16[:])
    nc.sync.dma_start(HgQ[64:65, :].rearrange("o (i p) -> o i p", p=128), hc16[:])
    # c in chunk-column layout for the lastH bias path
    pc = fps.tile([128, 16], FP, tag="fp")
    _tp(nc, pc[:], hc[:], ident)
    nc.scalar.copy(cT_sb[:], pc[:])

    # HgA = Hg_raw * a  (a broadcast by K=1 matmul, fused multiply from PSUM)
    for j in range(NMB):
        sl = slice(j * MB, (j + 1) * MB)
        ab = fps.tile([64, MB], FP, tag="fp")
        nc.tensor.matmul(ab[:], onesrow64[:], a_row[:, sl], start=True, stop=True)
        nc.vector.tensor_tensor(HgQ[0:64, sl], hgsq[0:64, sl], ab[:], AL.mult)

    # ===================== q / k projections =====================
    kacc = small.tile([128, 8], FP, tag="kacc")
    for half, (dst, wsl) in enumerate(((kT0, slice(0, 128)), (kT1, slice(128, 256)))):
        for j in range(NMB):
            sl = slice(j * MB, (j + 1) * MB)
            kp = fps.tile([128, MB], FP, tag="fp")
            nc.tensor.matmul(kp[:], kA[:, wsl], HgQ[:, sl], start=True, stop=True)
            nc.scalar.copy(dst[:, sl], kp[:])
            nc.vector.scalar_tensor_tensor(
                dst[:, sl], dst[:, sl], 0.01, dst[:, sl], AL.mult, AL.max,
                accum_out=kacc[:, 4 * half + j:4 * half + j + 1])
    for dst, wsl in ((qT0, slice(0, 128)), (qT1, slice(128, 256))):
        for j in range(2):
            sl = slice(j * MB, (j + 1) * MB)
            qp = fps.tile([128, MB], FP, tag="fp")
            nc.tensor.matmul(qp[:], qA[:, wsl], HgQ[:, sl], start=True, stop=True)
            nc.scalar.copy(dst[:, sl], qp[:])
            _leaky(nc, dst[:, sl])
    ks_f = small.tile([128, 2], FP, tag="ks_f")
    nc.vector.tensor_reduce(ks_f[:], kacc[:].rearrange("p (h j) -> p h j", j=4),
                            AX.X, AL.add)
    nc.vector.tensor_copy(ks0[:], ks_f[:, 0:1])
    nc.vector.tensor_copy(ks1[:], ks_f[:, 1:2])
    # k2 = k * ga ; x3gs rows 0:64 = x3 * gs
    nc.vector.tensor_tensor(k2T0[:], kT0[:], ga_b[:], AL.mult)
    nc.vector.tensor_tensor(k2T1[:], kT1[:], ga_b[:], AL.mult)
    nc.vector.tensor_tensor(x3gs[0:64, :], c1[0:64, :], gs_b[:], AL.mult)

    frontA.close()

    # ===================== Gram matrices =====================
    with tc.tile_pool(name="gps", bufs=3, space="PSUM") as gps, \
         tc.tile_pool(name="krpp", bufs=3, space="PSUM") as krpp, \
         tc.tile_pool(name="krp", bufs=3) as krp:
        gt_ps = gps.tile([128, 256], FP, tag="g", padded_shape=[128, 512])
        gb_ps = gps.tile([128, 256], FP, tag="g", padded_shape=[128, 512])
        for mi in range(NCH):
            msl = slice(mi * 128, (mi + 1) * 128)
            krq = krpp.tile([128, 256], FP, tag="kr", padded_shape=[128, 512])
            nc.tensor.matmul(krq[:], HgQ[:, msl], kA[:], start=True, stop=True)
            kr = krp.tile([128, 256], BF, tag="kr")
            nc.scalar.copy(kr[:], krq[:])
            _leaky(nc, kr[:])
            nc.tensor.matmul(gt_ps[:], kr[:, 0:128], kr[:],
                             start=(mi == 0), stop=(mi == NCH - 1))
            nc.tensor.matmul(gb_ps[:], kr[:, 128:256], kr[:],
                             start=(mi == 0), stop=(mi == NCH - 1))
        nc.vector.tensor_copy(gt_sb[:], gt_ps[:])
        nc.scalar.copy(gb_sb[:], gb_ps[:])
        # soc gram from x3 rows (rows via [x2 ; 1] @ [fc3 ; b] augmentation)
        gs_ps = gps.tile([64, 64], FP, tag="g", padded_shape=[64, 512])
        for mi in range(NCH):
            msl = slice(mi * 128, (mi + 1) * 128)
            xrq = krpp.tile([128, 64], FP, tag="kr", padded_shape=[128, 512])
            nc.tensor.matmul(xrq[:], x2a[:, msl], fc3a[:], start=True, stop=True)
            xr = krp.tile([128, 64], BF, tag="xr")
            (nc.vector.tensor_copy if mi % 2 == 0 else nc.scalar.copy)(xr[:], xrq[:])
            nc.tensor.matmul(gs_ps[:], xr[:], xr[:],
                             start=(mi == 0), stop=(mi == NCH - 1))
        nc.vector.tensor_copy(gs_f[:], gs_ps[:])

    # ============== own-row stats: S1, T1, S2, T2 ==============
    s1t = small.tile([8, 128], FP, tag="s1t")
    t1t = small.tile([8, 128], FP, tag="t1t")
    s2t = small.tile([8, 128], FP, tag="s2t")
    t2t = small.tile([8, 128], FP, tag="t2t")
    with tc.tile_pool(name="ups", bufs=2, space="PSUM") as ups, \
         tc.tile_pool(name="rps", bufs=2, space="PSUM") as rps:
        s1p = rps.tile([1, HALF], FP, tag="r1")
        t1p = rps.tile([1, HALF], FP, tag="r1")
        for jb in range(2):
            sl = slice(jb * MB, (jb + 1) * MB)
            nc.tensor.matmul(s1p[:, sl], ks0[:], qT0[:, sl], start=True, stop=False)
            nc.tensor.matmul(s1p[:, sl], ks1[:], qT1[:, sl], start=False, stop=True)
            nc.tensor.matmul(t1p[:, sl], xsb[:], c1[0:64, sl], start=True, stop=True)
        s1sb = small.tile([1, HALF], FP, tag="s1sb")
        nc.scalar.copy(s1sb[:], s1p[:])
        t1sb = small.tile([1, HALF], FP, tag="t1sb")
        nc.scalar.copy(t1sb[:], t1p[:])
        # S2 = q^T Gram q via u = G q ; e = u*q ; partition-sum on gpsimd
        ut0 = ups.tile([128, HALF], FP, tag="ut")
        ut1 = ups.tile([128, HALF], FP, tag="ut")
        for jb in range(2):
            sl = slice(jb * MB, (jb + 1) * MB)
            nc.tensor.matmul(ut0[:, sl], gt_sb[:, 0:128], qT0[:, sl], start=True, stop=False)
            nc.tensor.matmul(ut0[:, sl], gb_sb[:, 0:128], qT1[:, sl], start=False, stop=True)
            nc.tensor.matmul(ut1[:, sl], gt_sb[:, 128:256], qT0[:, sl], start=True, stop=False)
            nc.tensor.matmul(ut1[:, sl], gb_sb[:, 128:256], qT1[:, sl], start=False, stop=True)
        e1t = small.tile([128, HALF], FP, tag="e1t")
        nc.vector.tensor_tensor(e01[:], ut0[:], qT0[:], AL.mult)
        nc.vector.tensor_tensor(e1t[:], ut1[:], qT1[:], AL.mult)
        nc.vector.tensor_tensor(e01[:], e01[:], e1t[:], AL.add)
        nc.gpsimd.partition_all_reduce(e01r[:], e01[:], 128, RED.add)
        # T2 via us = Gs x3 ; es = us * x3
        es = small.tile([64, HALF], FP, tag="es")
        us = ups.tile([64, HALF], FP, tag="ut")
        for jb in range(2):
            sl = slice(jb * MB, (jb + 1) * MB)
            nc.tensor.matmul(us[:, sl], gs_f[:], c1[0:64, sl], start=True, stop=True)
        nc.vector.tensor_tensor(es[:], us[:], c1[0:64, 0:HALF], AL.mult)
        nc.gpsimd.partition_all_reduce(esr[:], es[:], 64, RED.add)
    nc.sync.dma_start(s1t[:], s1sb[:].rearrange("o (i p) -> o i p", p=128))
    nc.sync.dma_start(t1t[:], t1sb[:].rearrange("o (i p) -> o i p", p=128))
    nc.sync.dma_start(s2t[:], e01r[0:1, :].rearrange("o (i p) -> o i p", p=128))
    nc.sync.dma_start(t2t[:], esr[0:1, :].rearrange("o (i p) -> o i p", p=128))

    # own stats-land [8, 128]
    muA = small.tile([8, 128], FP, tag="muA")
    nc.vector.tensor_scalar(muA[:], s1t[:], 1.0 / N, None, AL.mult)
    muS = small.tile([8, 128], FP, tag="muS")
    nc.vector.tensor_scalar(muS[:], t1t[:], 1.0 / N, None, AL.mult)
    m2 = small.tile([8, 128], FP, tag="m2")
    nc.vector.tensor_tensor(m2[:], muA[:], muA[:], AL.mult)
    varA = small.tile([8, 128], FP, tag="varA")
    nc.vector.scalar_tensor_tensor(varA[:], s2t[:], 1.0 / N, m2[:],
                                   AL.mult, AL.subtract)
    sdA = small.tile([8, 128], FP, tag="sdA")
    nc.scalar.activation(sdA[:], varA[:], AF.Sqrt, bias=epsc[0:8, 0:1])
    m2s = small.tile([8, 128], FP, tag="m2s")
    nc.vector.tensor_tensor(m2s[:], muS[:], muS[:], AL.mult)
    varS = small.tile([8, 128], FP, tag="varS")
    nc.vector.scalar_tensor_tensor(varS[:], t2t[:], 1.0 / N, m2s[:],
                                   AL.mult, AL.subtract)
    sdS = small.tile([8, 128], FP, tag="sdS")
    nc.scalar.activation(sdS[:], varS[:], AF.Sqrt, bias=epsc[0:8, 0:1])
    rsS = small.tile([8, 128], FP, tag="rsS")
    nc.vector.reciprocal(rsS[:], sdS[:])
    rho = small.tile([8, 128], FP, tag="rho")
    nc.vector.tensor_tensor(rho[:], rsS[:], sdA[:], AL.mult)
    r64t = small.tile([8, 128], BF, tag="r64t")
    nc.vector.tensor_scalar(r64t[:], muA[:], -1.0, None, AL.mult)
    r65t = small.tile([8, 128], BF, tag="r65t")
    nc.vector.scalar_tensor_tensor(r65t[:], muS[:], -1.0, rho[:], AL.mult, AL.mult)
    r66t = small.tile([8, 128], BF, tag="r66t")
    nc.scalar.copy(r66t[:], sdA[:])
    rho16 = small.tile([8, 128], BF, tag="rho16")
    nc.scalar.copy(rho16[:], rho[:])
    nc.sync.dma_start(x3rA[64:65, :].rearrange("o (i p) -> o i p", p=128), r64t[:])
    nc.sync.dma_start(x3rA[65:66, :].rearrange("o (i p) -> o i p", p=128), r65t[:])
    nc.sync.dma_start(x3rA[66:67, :].rearrange("o (i p) -> o i p", p=128), r66t[:])
    rho_row = small.tile([1, HALF], BF, tag="rho_row")
    nc.sync.dma_start(rho_row[:].rearrange("o (i p) -> o i p", p=128), rho16[:])
    rho_b = small.tile([64, HALF], BF, tag="rho_b")
    nc.gpsimd.partition_broadcast(rho_b[:], rho_row[:])
    nc.vector.tensor_tensor(x3rA[0:64, :], c1[0:64, 0:HALF], rho_b[:], AL.mult)

    # ============ lastH output (Hg LN rows, own half) ============
    with tc.tile_pool(name="lps", bufs=2, space="PSUM") as lps:
        for i in range(HCH):
            pt = lps.tile([128, 64], BF, tag="lpt", padded_shape=[128, 1024])
            _tp(nc, pt[:], HgQ[0:64, i * 128:(i + 1) * 128], identb)
            nc.scalar.activation(lastR[:, i * 64:(i + 1) * 64], pt[:], AF.Identity,
                                 bias=cT_sb[:, i:i + 1])
    l3 = lastR[:].rearrange("p (g e) -> p g e", e=64)
    lg3 = brows["bng"][:].unsqueeze(1).broadcast_to([128, HCH, 64])
    lb3 = brows["bnb"][:].unsqueeze(1).broadcast_to([128, HCH, 64])
    nc.vector.tensor_tensor(l3, l3, lg3, AL.mult)
    nc.vector.tensor_tensor(l3, l3, lb3, AL.add)
    nc.sync.dma_start(io["lastH"].rearrange("(i p) e -> p i e", p=128),
                      lastR[:].rearrange("p (i e) -> p i e", e=64))

    # ============ xo LN + transpose (independent of phase I) ============
    _ln_rows(nc, small, xo, xo, brows["xng"], brows["xnb"], HCH, "lnx", epsc)
    with tc.tile_pool(name="xps", bufs=2, space="PSUM") as xps:
        for i in range(HCH):
            pt = xps.tile([64, 128], FP, tag="xpt", padded_shape=[64, 512])
            _tp(nc, pt[:], xo[:, i * 64:(i + 1) * 64], ident)
            nc.scalar.activation(x1aug[0:64, i * 128:(i + 1) * 128], pt[:],
                                 AF.Identity, bias=b3c[:, 0:1])

    # =================== phase I: fused adjacency ===================
    with tc.tile_pool(name="zps", bufs=8, space="PSUM") as zps, \
         tc.tile_pool(name="scrv", bufs=2) as scrv, \
         tc.tile_pool(name="scra", bufs=2) as scra:
        for i in range(HCH):
            csl = slice(i * 128, (i + 1) * 128)
            zp = [zps.tile([128, MB], FP, tag="zpt", name=f"zp_{i}_{m}")
                  for m in range(NMB)]
            for mb in range(NMB):
                nc.tensor.matmul(zp[mb][:], qT0[:, csl],
                                 k2T0[:, mb * MB:(mb + 1) * MB],
                                 start=True, stop=False)
            for mb in range(NMB):
                nc.tensor.matmul(zp[mb][:], qT1[:, csl],
                                 k2T1[:, mb * MB:(mb + 1) * MB],
                                 start=False, stop=False)
            for mb in range(NMB):
                nc.tensor.matmul(zp[mb][:], x3rA[:, csl],
                                 x3gs[:, mb * MB:(mb + 1) * MB],
                                 start=False, stop=True)
            for mb in range(NMB):
                acc = rc32[:, 4 * i + mb:4 * i + mb + 1]
                if mb % 2 == 0:
                    scr = scrv.tile([128, MB], FP, tag="scr")
                    nc.vector.tensor_scalar(scr[:], zp[mb][:], 0.0, None, AL.max,
                                            AL.add, accum_out=acc)
                else:
                    scr = scra.tile([128, MB], FP, tag="scr2")
                    nc.scalar.activation(scr[:], zp[mb][:], AF.Relu, accum_out=acc)
                if mb == i // 4:
                    off = (i * 128) % MB
                    dsel = scrv.tile([128, 128], FP, tag="dsel")
                    nc.gpsimd.affine_select(
                        out=dsel[:], in_=scr[:, off:off + 128], compare_op=AL.is_equal,
                        fill=0.0, base=0, pattern=[[-1, 128]], channel_multiplier=1)
                    nc.vector.tensor_reduce(dg8[:, i:i + 1], dsel[:], AX.X, AL.add)
        rs8 = small.tile([128, HCH], FP, tag="rs8")
        nc.vector.tensor_reduce(rs8[:], rc32[:].rearrange("p (i m) -> p i m", m=4),
                                AX.X, AL.add)
        nc.vector.reciprocal(rs8[:], rs8[:])
        nc.vector.tensor_tensor(dl[:], dg8[:], rs8[:], AL.mult)

    # ======================= GCN tail =======================
    with tc.tile_pool(name="jps", bufs=4, space="PSUM") as jps, \
         tc.tile_pool(name="jw", bufs=2) as jw:
        ptd = jps.tile([HCH, 128], FP, tag="jt")
        _tp(nc, ptd[:], dl[:], ident)
        s8d = small.tile([HCH, 128], BF, tag="s8d")
        nc.vector.tensor_copy(s8d[:], ptd[:])
        dl_row = small.tile([1, HALF], BF, tag="dl_row")
        nc.sync.dma_start(dl_row[:].rearrange("o (i p) -> o i p", p=128), s8d[:])
        dls = small.tile([64, HALF], BF, tag="dls")
        nc.gpsimd.partition_broadcast(dls[:], dl_row[:])

        for (wt, rhs, dst) in ((w1a, x1aug, hca), (w2a, hca, hcb), (w3a, hcb, None)):
            for jb in range(2):
                sl = slice(jb * MB, (jb + 1) * MB)
                ph = jps.tile([64, MB], FP, tag="jt")
                nc.tensor.matmul(ph[:], wt[:], rhs[:, sl], start=True, stop=True)
                if dst is not None:
                    nc.vector.tensor_tensor(dst[0:64, sl], ph[:], dls[:, sl], AL.mult)
                else:
                    hn = jw.tile([64, MB], FP, tag="hn")
                    nc.vector.tensor_tensor(hn[:], ph[:], dls[:, sl], AL.mult)
                    nc.vector.tensor_tensor(hn[:], hn[:], x1aug[0:64, sl], AL.add)
                    for ii in range(4):
                        i = jb * 4 + ii
                        pt = jps.tile([128, 64], FP, tag="jt")
                        _tp(nc, pt[:], hn[:, ii * 128:(ii + 1) * 128], ident)
                        (nc.vector.tensor_copy if ii % 2 == 0 else nc.scalar.copy)(
                            fin[:, i * 64:(i + 1) * 64], pt[:])
        _ln_rows(nc, small, fin, fin, brows["lng"], brows["lnb"], HCH, "lnf", epsc)
        nc.sync.dma_start(io["outH"].rearrange("(i p) e -> p i e", p=128),
                          fin[:].rearrange("p (i e) -> p i e", e=64))


def _build():
    if "nc" in _CACHE:
        return _CACHE["nc"]
    nc = bacc.Bacc("TRN2", target_bir_lowering=False, debug=False,
                   enable_asserts=True, num_devices=8)
    io = {}

    def din(name, shape, dt=FP):
        io[name] = nc.dram_tensor(name, shape, dt, kind="ExternalInput").ap()

    def dout(name, shape):
        io[name] = nc.dram_tensor(name, shape, FP, kind="ExternalOutput").ap()

    din("xT", [G, N], BF)
    din("lastT", [G, N], BF)
    din("origH", [HALF, E])
    din("corr3", [3, N], BF)
    din("ga_r", [1, N], BF)
    din("gs_r", [1, N], BF)
    din("fc1s", [64, 16], BF)
    din("fc2s", [16, 2], BF)
    din("fc3s", [2, 64], BF)
    din("fc3r", [1, 64], BF)
    din("wz", [128, 64], BF)
    din("wr", [128, 64], BF)
    din("wh", [128, 64], BF)
    din("qA", [66, 256], BF)
    din("kA", [66, 256], BF)
    din("w1a", [65, 64], BF)
    din("w2a", [65, 64], BF)
    din("w3a", [65, 64], BF)
    din("fc1b", [16, 1])
    din("fc2b", [2, 1])
    din("fc3b", [64, 1])
    din("b3c", [64, 1])
    for nm in ("bng", "bnb", "xng", "xnb", "lng", "lnb"):
        din(nm, [1, 64])
    dout("outH", [HALF, E])
    dout("lastH", [HALF, G])

    with tile.TileContext(nc) as tc:
        with ExitStack() as ctx:
            _emit(ctx, tc, io)
    nc.compile()
    nc.m = get_hw_module(nc.m)
    _CACHE["nc"] = nc
    return nc


def _host_prep(inputs):
    f32 = np.float32
    bf = ml_dtypes.bfloat16
    inp = {k: np.asarray(v, f32) for k, v in inputs.items()}
    ch = 1.0 + inp["mlp_w"].sum(axis=0)
    assert (ch > 0).all(), "head-mixing scale fold requires positive c_h"
    g, b = inp["bn_g"], inp["bn_b"]
    qw_c = inp["q_w"] * np.repeat(ch / np.sqrt(G), G)[None, :]
    Wq = g[:, None] * qw_c
    qA = np.concatenate([Wq, Wq.sum(axis=0)[None], (b @ qw_c)[None]], axis=0)
    Wk = g[:, None] * inp["k_w"]
    kA = np.concatenate([Wk, Wk.sum(axis=0)[None], (b @ inp["k_w"])[None]], axis=0)
    w1 = inp["gcn_w1"]
    # layer-1 rhs carries xo + b3 (pre-folded for the final residual add);
    # the ones-row weight removes the spurious b3 @ w1 term.
    w1a = np.concatenate([w1, -(inp["gcn_b3"] @ w1)[None]], axis=0)
    w2a = np.concatenate([inp["gcn_w2"], (inp["gcn_b1"] @ inp["gcn_w2"])[None]], axis=0)
    w3a = np.concatenate([inp["gcn_w3"], (inp["gcn_b2"] @ inp["gcn_w3"])[None]], axis=0)

    def c(a, dt=bf):
        return np.ascontiguousarray(np.asarray(a, dt))

    shared = {
        "fc1s": c(inp["fc1_w"]), "fc2s": c(inp["fc2_w"]), "fc3s": c(inp["fc3_w"]),
        "fc3r": c(inp["fc3_b"][None, :]),
        "wz": c(inp["w_z"]), "wr": c(inp["w_r"]), "wh": c(inp["w_h"]),
        "qA": c(qA), "kA": c(kA),
        "w1a": c(w1a), "w2a": c(w2a), "w3a": c(w3a),
        "fc1b": c(inp["fc1_b"][:, None], f32),
        "fc2b": c(inp["fc2_b"][:, None], f32),
        "fc3b": c(inp["fc3_b"][:, None], f32),
        "b3c": c(inp["gcn_b3"][:, None], f32),
        "bng": c(g[None, :], f32), "bnb": c(b[None, :], f32),
        "xng": c(inp["x_nom_g"][None, :], f32), "xnb": c(inp["x_nom_b"][None, :], f32),
        "lng": c(inp["last_nom_g"][None, :], f32), "lnb": c(inp["last_nom_b"][None, :], f32),
    }
    in_maps = []
    for core in range(8):
        bi, h = core // 2, core % 2
        off = h * HALF
        ga = np.roll(inp["attn_norm_g"], -off)
        gs = np.roll(inp["skip_norm_g"], -off)
        cb = np.roll(inp["attn_norm_b"] + inp["skip_norm_b"], -off)
        m = dict(shared)
        m["xT"] = c(np.roll(inp["x"][bi], -off, axis=0).T)
        m["lastT"] = c(np.roll(inp["last_G_emb"][bi], -off, axis=0).T)
        m["origH"] = c(inp["orig_x"][bi, off:off + HALF], f32)
        m["corr3"] = c(np.stack([ga, gs, cb]))
        m["ga_r"] = c(ga[None, :])
        m["gs_r"] = c(gs[None, :])
        in_maps.append(m)
    return in_maps


def run(inputs, trace=False):
    nc = _build()
    in_maps = _host_prep(inputs)
    res = run_bass_kernel_spmd(nc, in_maps, core_ids=list(range(8)), trace=trace)
    out = np.zeros((B, N, E), np.float32)
    last = np.zeros((B, N, G), np.float32)
    for core in range(8):
        bi, h = core // 2, core % 2
        off = h * HALF
        out[bi, off:off + HALF] = res.results[core]["outH"]
        last[bi, off:off + HALF] = res.results[core]["lastH"]
    return (out, last), res


def kernel(**inputs):
    return run(inputs)[0]


# revision 9
# speedup vs baseline: 1.3618x; 1.1152x over previous
"""Trainium2 Bass kernel for nn_DGCN (gnn_message_passing).

Sharding: 8 shards = (batch b in 0..3, row-half h in 0..1). Each core gets
the full 2048-node K-side tensors of its batch with the node axis ROTATED
by h*1024 so the adjacency diagonal lands at the same tile position on
every core (uniform SPMD program); the core computes rows 0..1023 of the
rotated order, which are rows [h*1024, (h+1)*1024) of the original order.

v3 — latency-oriented rewrite:
 - All heavy tensors bf16; host pre-transposes x/last; all small params
   arrive in two packed mega-tiles (one bf16, one fp32) = 2 DMAs.
 - Head-mix scalars c_h and the Hg-LayerNorm gain fold into q_w/k_w; the
   LN shift becomes an extra contraction row of an augmented [66 x 256]
   weight (rhs rows = [Hg*a ; c ; 1]).
 - Every per-node LN statistic is computed by K-dim matmuls into a
   [128 nodes, n] column layout (N=1/2 matmuls), processed by tiny DVE
   ops there, and scattered to broadcast rows via one PE transpose + one
   SBUF-SBUF DMA. No gpsimd partition reduces/broadcasts on hot paths.
 - diag(L)_i = relu(bracket)_ii / rowsum_j relu(bracket)_ij is invariant
   to positive per-row scales, so the 1/sdA row scale of the fused
   pre-relu matrix cancels; only the x3 lhsT rows carry rsS/rsA and the
   stat rows carry {-muA, -rho*muS, sdA}.
 - relu row-sums via accum_out; GCN biases ride the next layer's matmul
   as host-folded b@W rows against a constant ones row.
"""

import sys

if '/opt/trn_rl_repo' not in sys.path:
    sys.path.insert(0, '/opt/trn_rl_repo')

from contextlib import ExitStack

import numpy as np
import ml_dtypes

import concourse.bass as bass
import concourse.tile as tile
from concourse import bacc, mybir
from concourse.bass_interp import get_hw_module
from concourse.bass_utils import run_bass_kernel_spmd

FP = mybir.dt.float32
BF = mybir.dt.bfloat16
AL = mybir.AluOpType
AF = mybir.ActivationFunctionType
AX = mybir.AxisListType

B, N, E, G, H = 4, 2048, 64, 64, 4
D = H * G          # 256
HALF = N // 2      # own rows per core
NCH = N // 128     # 16 chunks over all nodes
HCH = HALF // 128  # 8 own chunks
MB = 512
NMB = N // MB      # 4
EPS = 1e-5

# wpack (bf16 [128, WPACK_W]) column layout
W_IDB, W_WZ, W_WR, W_WH = 0, 128, 192, 256
W_QA, W_KA = 320, 576
W_FC1, W_FC2, W_FC3A = 832, 848, 850
W_W1A, W_W2A, W_W3A = 914, 978, 1042
W_SEL, W_ONE = 1106, 1108
WPACK_W = 1280
# fpack (fp32 [128, FPACK_W]) column layout
F_IDF, F_B, F_EPS, F_BN = 0, 128, 132, 136
FPACK_W = 528

_CACHE = {}


def _tp(nc, out_ap, in_ap, ident):
    k = in_ap.partition_size()
    nc.tensor.transpose(out_ap, in_ap, ident[0:k, 0:k])


def _leaky(nc, dst):
    nc.vector.scalar_tensor_tensor(dst, dst, 0.01, dst, AL.mult, AL.max)


def _ln_rows(nc, pool, t_in, t_out, g_b, b_b, ngr, tag, epsc):
    """LayerNorm over 64-wide groups: t_in [128, ngr*64] -> t_out."""
    a3 = t_in[:].rearrange("p (g e) -> p g e", e=64)
    o3 = t_out[:].rearrange("p (g e) -> p g e", e=64)
    sm = pool.tile([128, ngr], FP, tag=f"{tag}_sm")
    nc.vector.tensor_reduce(sm[:], a3, AX.X, AL.add)
    sq = pool.tile([128, ngr * 64], FP, tag=f"{tag}_sq")
    nc.scalar.square(sq[:], t_in[:])
    sqs = pool.tile([128, ngr], FP, tag=f"{tag}_sqs")
    nc.vector.tensor_reduce(sqs[:], sq[:].rearrange("p (g e) -> p g e", e=64),
                            AX.X, AL.add)
    mu = pool.tile([128, ngr], FP, tag=f"{tag}_mu")
    nc.vector.tensor_scalar(mu[:], sm[:], 1.0 / 64, None, AL.mult)
    mu2 = pool.tile([128, ngr], FP, tag=f"{tag}_mu2")
    nc.vector.tensor_tensor(mu2[:], mu[:], mu[:], AL.mult)
    var = pool.tile([128, ngr], FP, tag=f"{tag}_var")
    nc.vector.scalar_tensor_tensor(var[:], sqs[:], 1.0 / 64, mu2[:], AL.mult,
                                   AL.subtract)
    sd = pool.tile([128, ngr], FP, tag=f"{tag}_sd")
    nc.scalar.activation(sd[:], var[:], AF.Sqrt, bias=epsc)
    rs = pool.tile([128, ngr], FP, tag=f"{tag}_rs")
    nc.vector.reciprocal(rs[:], sd[:])
    mu_b = mu[:].unsqueeze(2).broadcast_to([128, ngr, 64])
    rs_b = rs[:].unsqueeze(2).broadcast_to([128, ngr, 64])
    g3 = g_b.unsqueeze(1).broadcast_to([128, ngr, 64])
    b3 = b_b.unsqueeze(1).broadcast_to([128, ngr, 64])
    xc = pool.tile([128, ngr * 64], FP, tag=f"{tag}_xc")
    xc3 = xc[:].rearrange("p (g e) -> p g e", e=64)
    nc.vector.tensor_tensor(xc3, a3, mu_b, AL.subtract)
    nc.vector.tensor_tensor(xc3, xc3, rs_b, AL.mult)
    nc.vector.tensor_tensor(xc3, xc3, g3, AL.mult)
    nc.vector.tensor_tensor(o3, xc3, b3, AL.add)


def _emit(ctx: ExitStack, tc: tile.TileContext, io: dict):
    nc = tc.nc

    persist = ctx.enter_context(tc.tile_pool(name="persist", bufs=1))
    small = ctx.enter_context(tc.tile_pool(name="small", bufs=1))

    # ---------------- packed params (2 DMAs) ----------------
    wp = persist.tile([128, WPACK_W], BF, tag="wp")
    nc.sync.dma_start(wp[:], io["wpack"][:])
    fp_ = persist.tile([128, FPACK_W], FP, tag="fp_")
    nc.sync.dma_start(fp_[:], io["fpack"][:])

    identb = wp[:, W_IDB:W_IDB + 128]
    wz = wp[:, W_WZ:W_WZ + 64]
    wr = wp[:, W_WR:W_WR + 64]
    wh = wp[:, W_WH:W_WH + 64]
    qA = wp[0:66, W_QA:W_QA + 256]
    kA = wp[0:66, W_KA:W_KA + 256]
    fc1s = wp[0:64, W_FC1:W_FC1 + 16]
    fc2s = wp[0:16, W_FC2:W_FC2 + 2]
    fc3s = wp[0:2, W_FC3A:W_FC3A + 64]
    fc3a = wp[0:3, W_FC3A:W_FC3A + 64]
    w1a = wp[0:65, W_W1A:W_W1A + 64]
    w2a = wp[0:65, W_W2A:W_W2A + 64]
    w3a = wp[0:65, W_W3A:W_W3A + 64]
    sel2 = wp[:, W_SEL:W_SEL + 2]
    ones128c = wp[:, W_ONE:W_ONE + 1]
    ones64c = wp[0:64, W_ONE:W_ONE + 1]
    onesr128 = wp[0:1, W_ONE:W_ONE + 128]
    onesr64 = wp[0:1, W_ONE:W_ONE + 64]

    ident = fp_[:, F_IDF:F_IDF + 128]
    fc1b = fp_[0:16, F_B + 0:F_B + 1]
    fc2b = fp_[0:2, F_B + 1:F_B + 2]
    fc3b = fp_[0:64, F_B + 2:F_B + 3]
    b3c = fp_[0:64, F_B + 3:F_B + 4]
    epsc = fp_[0:128, F_EPS:F_EPS + 1]

    # LN parameter rows -> [128, 64] broadcast tiles via gpsimd (small)
    brows = {}
    for k, nm in enumerate(("bng", "bnb", "xng", "xnb", "lng", "lnb")):
        t = persist.tile([128, 64], FP, tag=f"{nm}_b", name=f"{nm}_b")
        nc.gpsimd.partition_broadcast(t[:], fp_[0:1, F_BN + 64 * k:F_BN + 64 * (k + 1)])
        brows[nm] = t

    # ---------------- big persistent tiles ----------------
    xT = persist.tile([64, N], BF, tag="xT")
    lastT = persist.tile([64, N], BF, tag="lastT")
    c1 = persist.tile([128, N], BF, tag="c1")      # [x3 ; last]
    c2 = persist.tile([128, N], BF, tag="c2")      # [r*last ; x3]
    hgsq = persist.tile([128, N], BF, tag="hgsq")  # [Hg_raw ; Hg_raw^2]
    HgQ = persist.tile([66, N], BF, tag="HgQ")     # [Hg*a ; c ; 1]
    a_row = persist.tile([1, N], BF, tag="a_row")
    kT0 = persist.tile([128, N], BF, tag="kT0")
    kT1 = persist.tile([128, N], BF, tag="kT1")
    k2T0 = persist.tile([128, N], BF, tag="k2T0")
    k2T1 = persist.tile([128, N], BF, tag="k2T1")
    qT0 = persist.tile([128, HALF], BF, tag="qT0")
    qT1 = persist.tile([128, HALF], BF, tag="qT1")
    x3gs = persist.tile([67, N], BF, tag="x3gs")   # [x3*gs ; ga ; gs ; cb]
    x3rA = persist.tile([67, HALF], BF, tag="x3rA")  # [x3*rho ; -muA ; -rho*muS ; sdA]
    ga_b = persist.tile([128, N], BF, tag="ga_b")
    gs_b = persist.tile([64, N], BF, tag="gs_b")
    ga_r = persist.tile([1, N], BF, tag="ga_r")
    gs_r = persist.tile([1, N], BF, tag="gs_r")
    gt_sb = persist.tile([128, 256], BF, tag="gt_sb")
    gb_sb = persist.tile([128, 256], BF, tag="gb_sb")
    gs_f = persist.tile([64, 64], BF, tag="gs_f")
    ks0 = persist.tile([128, 1], BF, tag="ks0")
    ks1 = persist.tile([128, 1], BF, tag="ks1")
    xsb = persist.tile([64, 1], BF, tag="xsb")
    rc32 = persist.tile([128, 4 * HCH], FP, tag="rc32")
    dg8 = persist.tile([128, HCH], FP, tag="dg8")
    dl = persist.tile([128, HCH], FP, tag="dl")
    x1T = persist.tile([16, N], BF, tag="x1T")
    x2a = persist.tile([3, N], BF, tag="x2a")      # [x2 ; 1]
    e01 = persist.tile([128, HALF], BF, tag="e01")
    essb = persist.tile([64, HALF], BF, tag="essb")
    htri = persist.tile([128, 32], FP, tag="htri")   # cols 0:16 a, 16:32 c
    lastR = persist.tile([128, HCH * 64], FP, tag="lastR")
    xo = persist.tile([128, HCH * 64], FP, tag="xo")
    x1aug = persist.tile([65, HALF], BF, tag="x1aug")  # [xo^T + b3 ; 1]
    hca = persist.tile([65, HALF], BF, tag="hca")
    hcb = persist.tile([65, HALF], BF, tag="hcb")
    fin = persist.tile([128, HCH * 64], FP, tag="fin")

    # input loads
    nc.sync.dma_start(xT[:], io["xT"][:])
    nc.sync.dma_start(lastT[:], io["lastT"][:])
    nc.sync.dma_start(c1[64:128, :], io["lastT"][:])
    nc.sync.dma_start(x3gs[64:67, :], io["corr4"][0:3, :])
    nc.sync.dma_start(ga_r[:], io["corr4"][0:1, :])
    nc.sync.dma_start(gs_r[:], io["corr4"][1:2, :])
    nc.sync.dma_start(xo[:].rearrange("p (i e) -> p i e", e=64),
                      io["origH"].rearrange("(i p) e -> p i e", p=128))
    # constant-ones rows
    nc.gpsimd.dma_start(HgQ[65:66, :], io["corr4"][3:4, :])
    nc.gpsimd.dma_start(x2a[2:3, :], io["corr4"][3:4, :])
    nc.gpsimd.dma_start(x1aug[64:65, :], io["corr4"][3:4, 0:HALF])
    nc.gpsimd.dma_start(hca[64:65, :], io["corr4"][3:4, 0:HALF])
    nc.gpsimd.dma_start(hcb[64:65, :], io["corr4"][3:4, 0:HALF])

    frontA = ExitStack()
    fps = frontA.enter_context(tc.tile_pool(name="fps", bufs=4, space="PSUM"))
    sps = frontA.enter_context(tc.tile_pool(name="sps", bufs=2, space="PSUM"))
    gw = frontA.enter_context(tc.tile_pool(name="gw", bufs=3))

    # ---- ga / gs broadcast tiles via K=1 matmuls ----
    for j in range(NMB):
        sl = slice(j * MB, (j + 1) * MB)
        gp1 = fps.tile([128, MB], FP, tag="fp")
        nc.tensor.matmul(gp1[:], onesr128, ga_r[:, sl], start=True, stop=True)
        (nc.vector.tensor_copy if j % 2 == 0 else nc.scalar.copy)(ga_b[:, sl], gp1[:])
        gp2 = fps.tile([64, MB], FP, tag="fp")
        nc.tensor.matmul(gp2[:], onesr64, gs_r[:, sl], start=True, stop=True)
        (nc.scalar.copy if j % 2 == 0 else nc.vector.tensor_copy)(gs_b[:, sl], gp2[:])

    # ============ hyper fc stack (T layout, per 512 block) ============
    xacc = small.tile([64, NMB], FP, tag="xacc")
    for j in range(NMB):
        sl = slice(j * MB, (j + 1) * MB)
        p1 = fps.tile([16, MB], FP, tag="fp")
        nc.tensor.matmul(p1[:], fc1s, xT[:, sl], start=True, stop=True)
        nc.scalar.activation(x1T[:, sl], p1[:], AF.Sigmoid, bias=fc1b)
        p2 = fps.tile([2, MB], FP, tag="fp")
        nc.tensor.matmul(p2[:], fc2s, x1T[:, sl], start=True, stop=True)
        nc.scalar.activation(x2a[0:2, sl], p2[:], AF.Sigmoid, bias=fc2b)
        p3 = fps.tile([64, MB], FP, tag="fp")
        nc.tensor.matmul(p3[:], fc3s, x2a[0:2, sl], start=True, stop=True)
        nc.scalar.activation(c1[0:64, sl], p3[:], AF.Identity, bias=fc3b,
                             accum_out=xacc[:, j:j + 1])
        nc.vector.tensor_copy(c2[64:128, sl], c1[0:64, sl])

    xs_f = small.tile([64, 1], FP, tag="xs_f")
    nc.vector.tensor_reduce(xs_f[:], xacc[:], AX.X, AL.add)
    nc.vector.tensor_copy(xsb[:], xs_f[:])

    # ================= GRU gates =================
    for j in range(NMB):
        sl = slice(j * MB, (j + 1) * MB)
        zp = fps.tile([64, MB], FP, tag="fp")
        nc.tensor.matmul(zp[:], wz, c1[:, sl], start=True, stop=True)
        zt = gw.tile([64, MB], BF, tag="zt")
        nc.scalar.activation(zt[:], zp[:], AF.Sigmoid)
        rp = fps.tile([64, MB], FP, tag="fp")
        nc.tensor.matmul(rp[:], wr, c1[:, sl], start=True, stop=True)
        rt = gw.tile([64, MB], BF, tag="rt")
        nc.scalar.activation(rt[:], rp[:], AF.Sigmoid)
        nc.gpsimd.tensor_tensor(c2[0:64, sl], rt[:], lastT[:, sl], AL.mult)
        hp = fps.tile([64, MB], FP, tag="fp")
        nc.tensor.matmul(hp[:], wh, c2[:, sl], start=True, stop=True)
        ht = gw.tile([64, MB], BF, tag="ht")
        nc.scalar.activation(ht[:], hp[:], AF.Tanh)
        dt_ = gw.tile([64, MB], BF, tag="dt_")
        nc.vector.tensor_tensor(dt_[:], ht[:], lastT[:, sl], AL.subtract)
        nc.vector.tensor_tensor(dt_[:], dt_[:], zt[:], AL.mult)
        nc.vector.tensor_tensor(hgsq[0:64, sl], dt_[:], lastT[:, sl], AL.add)

    # Hg^2 into partitions 64:128
    nc.scalar.square(hgsq[64:128, 0:HALF], hgsq[0:64, 0:HALF])
    nc.scalar.square(hgsq[64:128, HALF:N], hgsq[0:64, HALF:N])

    # ====== Hg LN stats: per-chunk K-matmuls into [128, 2] cols ======
    hs2 = sps.tile([128, 2 * NCH], FP, tag="hs", bufs=1, padded_shape=[128, 512])
    for mi in range(NCH):
        nc.tensor.matmul(hs2[:, 2 * mi:2 * mi + 2], hgsq[:, mi * 128:(mi + 1) * 128],
                         sel2, start=True, stop=True)
    hv = hs2[:].rearrange("p (i s) -> p s i", s=2)
    hmu = small.tile([128, NCH], FP, tag="hmu")
    nc.vector.tensor_scalar(hmu[:].unsqueeze(1), hv[:, 0:1, :], 1.0 / 64, None, AL.mult)
    hmu2 = small.tile([128, NCH], FP, tag="hmu2")
    nc.vector.tensor_tensor(hmu2[:], hmu[:], hmu[:], AL.mult)
    hvar = small.tile([128, NCH], FP, tag="hvar")
    nc.vector.scalar_tensor_tensor(hvar[:].unsqueeze(1), hv[:, 1:2, :], 1.0 / 64,
                                   hmu2[:].unsqueeze(1), AL.mult, AL.subtract)
    hsd = small.tile([128, NCH], FP, tag="hsd")
    nc.scalar.activation(hsd[:], hvar[:], AF.Sqrt, bias=epsc)
    nc.vector.reciprocal(htri[:, 0:16], hsd[:])
    nc.vector.scalar_tensor_tensor(htri[:, 16:32], hmu[:], -1.0, htri[:, 0:16],
                                   AL.mult, AL.mult)
    hrp = sps.tile([32, 128], FP, tag="hrp", bufs=1, padded_shape=[32, 512])
    _tp(nc, hrp[:], htri[:], ident)
    hrow = small.tile([32, 128], BF, tag="hrow")
    nc.vector.tensor_copy(hrow[:], hrp[:])
    nc.sync.dma_start(a_row[:].rearrange("o (i p) -> o i p", p=128), hrow[0:16, :])
    nc.sync.dma_start(HgQ[64:65, :].rearrange("o (i p) -> o i p", p=128), hrow[16:32, :])

    # HgA = Hg_raw * a  (a broadcast by K=1 matmul, fused multiply from PSUM)
    for j in range(NMB):
        sl = slice(j * MB, (j + 1) * MB)
        ab = fps.tile([64, MB], FP, tag="fp")
        nc.tensor.matmul(ab[:], onesr64, a_row[:, sl], start=True, stop=True)
        nc.vector.tensor_tensor(HgQ[0:64, sl], hgsq[0:64, sl], ab[:], AL.mult)

    # ===================== q / k projections =====================
    kacc = small.tile([128, 8], FP, tag="kacc")
    for half, (dst, wsl) in enumerate(((kT0, slice(W_KA, W_KA + 128)),
                                       (kT1, slice(W_KA + 128, W_KA + 256)))):
        for j in range(NMB):
            sl = slice(j * MB, (j + 1) * MB)
            kp = fps.tile([128, MB], FP, tag="fp")
            nc.tensor.matmul(kp[:], wp[0:66, wsl], HgQ[:, sl], start=True, stop=True)
            nc.scalar.copy(dst[:, sl], kp[:])
            nc.vector.scalar_tensor_tensor(
                dst[:, sl], dst[:, sl], 0.01, dst[:, sl], AL.mult, AL.max,
                accum_out=kacc[:, 4 * half + j:4 * half + j + 1])
    for dst, wsl in ((qT0, slice(W_QA, W_QA + 128)), (qT1, slice(W_QA + 128, W_QA + 256))):
        for j in range(2):
            sl = slice(j * MB, (j + 1) * MB)
            qp = fps.tile([128, MB], FP, tag="fp")
            nc.tensor.matmul(qp[:], wp[0:66, wsl], HgQ[:, sl], start=True, stop=True)
            nc.scalar.copy(dst[:, sl], qp[:])
            _leaky(nc, dst[:, sl])
    ks_f = small.tile([128, 2], FP, tag="ks_f")
    nc.vector.tensor_reduce(ks_f[:], kacc[:].rearrange("p (h j) -> p h j", j=4),
                            AX.X, AL.add)
    nc.vector.tensor_copy(ks0[:], ks_f[:, 0:1])
    nc.vector.tensor_copy(ks1[:], ks_f[:, 1:2])
    # k2 = k * ga ; x3gs rows 0:64 = x3 * gs
    nc.vector.tensor_tensor(k2T0[:], kT0[:], ga_b[:], AL.mult)
    nc.vector.tensor_tensor(k2T1[:], kT1[:], ga_b[:], AL.mult)
    nc.vector.tensor_tensor(x3gs[0:64, :], c1[0:64, :], gs_b[:], AL.mult)

    frontA.close()

    # ===================== Gram matrices =====================
    with tc.tile_pool(name="gps", bufs=3, space="PSUM") as gps, \
         tc.tile_pool(name="krpp", bufs=3, space="PSUM") as krpp, \
         tc.tile_pool(name="krp", bufs=3) as krp:
        gt_ps = gps.tile([128, 256], FP, tag="g", padded_shape=[128, 512])
        gb_ps = gps.tile([128, 256], FP, tag="g", padded_shape=[128, 512])
        for mi in range(NCH):
            msl = slice(mi * 128, (mi + 1) * 128)
            krq = krpp.tile([128, 256], FP, tag="kr", padded_shape=[128, 512])
            nc.tensor.matmul(krq[:], HgQ[:, msl], kA, start=True, stop=True)
            kr = krp.tile([128, 256], BF, tag="kr")
            nc.scalar.copy(kr[:], krq[:])
            _leaky(nc, kr[:])
            nc.tensor.matmul(gt_ps[:], kr[:, 0:128], kr[:],
                             start=(mi == 0), stop=(mi == NCH - 1))
            nc.tensor.matmul(gb_ps[:], kr[:, 128:256], kr[:],
                             start=(mi == 0), stop=(mi == NCH - 1))
        nc.vector.tensor_copy(gt_sb[:], gt_ps[:])
        nc.scalar.copy(gb_sb[:], gb_ps[:])
        # soc gram from x3 rows (rows via [x2 ; 1] @ [fc3 ; b] augmentation)
        gs_ps = gps.tile([64, 64], FP, tag="g", padded_shape=[64, 512])
        for mi in range(NCH):
            msl = slice(mi * 128, (mi + 1) * 128)
            xrq = krpp.tile([128, 64], FP, tag="kr", padded_shape=[128, 512])
            nc.tensor.matmul(xrq[:], x2a[:, msl], fc3a, start=True, stop=True)
            xr = krp.tile([128, 64], BF, tag="xr")
            (nc.vector.tensor_copy if mi % 2 == 0 else nc.scalar.copy)(xr[:], xrq[:])
            nc.tensor.matmul(gs_ps[:], xr[:], xr[:],
                             start=(mi == 0), stop=(mi == NCH - 1))
        nc.vector.tensor_copy(gs_f[:], gs_ps[:])

    # ============ lastH output (Hg LN rows, own half) ============
    with tc.tile_pool(name="lps", bufs=2, space="PSUM") as lps:
        for i in range(HCH):
            pt = lps.tile([128, 64], BF, tag="lpt", padded_shape=[128, 1024])
            _tp(nc, pt[:], HgQ[0:64, i * 128:(i + 1) * 128], identb)
            nc.scalar.activation(lastR[:, i * 64:(i + 1) * 64], pt[:], AF.Identity,
                                 bias=htri[:, 16 + i:17 + i])
    l3 = lastR[:].rearrange("p (g e) -> p g e", e=64)
    lg3 = brows["bng"][:].unsqueeze(1).broadcast_to([128, HCH, 64])
    lb3 = brows["bnb"][:].unsqueeze(1).broadcast_to([128, HCH, 64])
    nc.vector.tensor_tensor(l3, l3, lg3, AL.mult)
    nc.vector.tensor_tensor(l3, l3, lb3, AL.add)
    nc.sync.dma_start(io["lastH"].rearrange("(i p) e -> p i e", p=128),
                      lastR[:].rearrange("p (i e) -> p i e", e=64))

    # ============ xo LN + transpose (independent of phase I) ============
    _ln_rows(nc, small, xo, xo, brows["xng"][:], brows["xnb"][:], HCH, "lnx", epsc)
    with tc.tile_pool(name="xps", bufs=2, space="PSUM") as xps:
        for i in range(HCH):
            pt = xps.tile([64, 128], FP, tag="xpt", padded_shape=[64, 512])
            _tp(nc, pt[:], xo[:, i * 64:(i + 1) * 64], ident)
            nc.scalar.activation(x1aug[0:64, i * 128:(i + 1) * 128], pt[:],
                                 AF.Identity, bias=b3c)

    # ============== own-row stats: S1, T1, S2, T2 ==============
    with tc.tile_pool(name="ups", bufs=2, space="PSUM") as ups, \
         tc.tile_pool(name="pps", bufs=2, space="PSUM") as pps:
        # u = G q (contraction over q features); e01 = (u0*q0 + u1*q1)
        ut0 = ups.tile([128, HALF], FP, tag="ut")
        ut1 = ups.tile([128, HALF], FP, tag="ut")
        for jb in range(2):
            sl = slice(jb * MB, (jb + 1) * MB)
            nc.tensor.matmul(ut0[:, sl], gt_sb[:, 0:128], qT0[:, sl], start=True, stop=False)
            nc.tensor.matmul(ut0[:, sl], gb_sb[:, 0:128], qT1[:, sl], start=False, stop=True)
            nc.tensor.matmul(ut1[:, sl], gt_sb[:, 128:256], qT0[:, sl], start=True, stop=False)
            nc.tensor.matmul(ut1[:, sl], gb_sb[:, 128:256], qT1[:, sl], start=False, stop=True)
        e1t = small.tile([128, HALF], BF, tag="e1t")
        nc.vector.tensor_tensor(e01[:], ut0[:], qT0[:], AL.mult)
        nc.vector.tensor_tensor(e1t[:], ut1[:], qT1[:], AL.mult)
        nc.vector.tensor_tensor(e01[:], e01[:], e1t[:], AL.add)
        us = ups.tile([64, HALF], FP, tag="ut")
        for jb in range(2):
            sl = slice(jb * MB, (jb + 1) * MB)
            nc.tensor.matmul(us[:, sl], gs_f[:], c1[0:64, sl], start=True, stop=True)
        nc.vector.tensor_tensor(essb[:], us[:], c1[0:64, 0:HALF], AL.mult)

        p32 = pps.tile([128, 32], FP, tag="p32", padded_shape=[128, 512])
        for i in range(HCH):
            csl = slice(i * 128, (i + 1) * 128)
            nc.tensor.matmul(p32[:, i:i + 1], qT0[:, csl], ks0[:], start=True, stop=False)
            nc.tensor.matmul(p32[:, i:i + 1], qT1[:, csl], ks1[:], start=False, stop=True)
            nc.tensor.matmul(p32[:, 8 + i:9 + i], c1[0:64, csl], xsb[:],
                             start=True, stop=True)
            nc.tensor.matmul(p32[:, 16 + i:17 + i], e01[:, csl], ones128c,
                             start=True, stop=True)
            nc.tensor.matmul(p32[:, 24 + i:25 + i], essb[:, csl], ones64c,
                             start=True, stop=True)

        # stats math in [128, 8]
        muA = small.tile([128, 8], FP, tag="muA")
        nc.vector.tensor_scalar(muA[:], p32[:, 0:8], 1.0 / N, None, AL.mult)
        muS = small.tile([128, 8], FP, tag="muS")
        nc.vector.tensor_scalar(muS[:], p32[:, 8:16], 1.0 / N, None, AL.mult)
        m2 = small.tile([128, 8], FP, tag="m2")
        nc.vector.tensor_tensor(m2[:], muA[:], muA[:], AL.mult)
        varA = small.tile([128, 8], FP, tag="varA")
        nc.vector.scalar_tensor_tensor(varA[:], p32[:, 16:24], 1.0 / N, m2[:],
                                       AL.mult, AL.subtract)
        sdA = small.tile([128, 8], FP, tag="sdA")
        nc.scalar.activation(sdA[:], varA[:], AF.Sqrt, bias=epsc)
        m2s = small.tile([128, 8], FP, tag="m2s")
        nc.vector.tensor_tensor(m2s[:], muS[:], muS[:], AL.mult)
        varS = small.tile([128, 8], FP, tag="varS")
        nc.vector.scalar_tensor_tensor(varS[:], p32[:, 24:32], 1.0 / N, m2s[:],
                                       AL.mult, AL.subtract)
        sdS = small.tile([128, 8], FP, tag="sdS")
        nc.scalar.activation(sdS[:], varS[:], AF.Sqrt, bias=epsc)
        rsS = small.tile([128, 8], FP, tag="rsS")
        nc.vector.reciprocal(rsS[:], sdS[:])
        tri32 = small.tile([128, 32], FP, tag="tri32")
        nc.vector.tensor_tensor(tri32[:, 24:32], rsS[:], sdA[:], AL.mult)  # rho
        nc.vector.tensor_scalar(tri32[:, 0:8], muA[:], -1.0, None, AL.mult)
        nc.vector.scalar_tensor_tensor(tri32[:, 8:16], muS[:], -1.0, tri32[:, 24:32],
                                       AL.mult, AL.mult)
        nc.scalar.copy(tri32[:, 16:24], sdA[:])
        trp = pps.tile([32, 128], FP, tag="p32", padded_shape=[32, 512])
        _tp(nc, trp[:], tri32[:], ident)
        trow = small.tile([32, 128], BF, tag="trow")
        nc.vector.tensor_copy(trow[:], trp[:])
        nc.sync.dma_start(x3rA[64:67, :].rearrange("r (i p) -> r i p", p=128),
                          trow[0:24, :])
        rho_row = small.tile([1, HALF], BF, tag="rho_row")
        nc.sync.dma_start(rho_row[:].rearrange("o (i p) -> o i p", p=128),
                          trow[24:32, :])
        for jb in range(2):
            sl = slice(jb * MB, (jb + 1) * MB)
            rp_ = pps.tile([64, MB], FP, tag="p32", name=f"rhob_{jb}")
            nc.tensor.matmul(rp_[:], onesr64, rho_row[:, sl], start=True, stop=True)
            nc.vector.tensor_tensor(x3rA[0:64, sl], c1[0:64, sl], rp_[:], AL.mult)

    # =================== phase I: fused adjacency ===================
    with tc.tile_pool(name="zps", bufs=8, space="PSUM") as zps, \
         tc.tile_pool(name="scrv", bufs=2) as scrv, \
         tc.tile_pool(name="scra", bufs=2) as scra:
        for i in range(HCH):
            csl = slice(i * 128, (i + 1) * 128)
            zp = [zps.tile([128, MB], FP, tag="zpt", name=f"zp_{i}_{m}")
                  for m in range(NMB)]
            for mb in range(NMB):
                nc.tensor.matmul(zp[mb][:], qT0[:, csl],
                                 k2T0[:, mb * MB:(mb + 1) * MB],
                                 start=True, stop=False)
            for mb in range(NMB):
                nc.tensor.matmul(zp[mb][:], qT1[:, csl],
                                 k2T1[:, mb * MB:(mb + 1) * MB],
                                 start=False, stop=False)
            for mb in range(NMB):
                nc.tensor.matmul(zp[mb][:], x3rA[:, csl],
                                 x3gs[:, mb * MB:(mb + 1) * MB],
                                 start=False, stop=True)
            for mb in range(NMB):
                acc = rc32[:, 4 * i + mb:4 * i + mb + 1]
                if mb % 2 == 0:
                    scr = scrv.tile([128, MB], FP, tag="scr")
                    nc.vector.tensor_scalar(scr[:], zp[mb][:], 0.0, None, AL.max,
                                            AL.add, accum_out=acc)
                else:
                    scr = scra.tile([128, MB], FP, tag="scr2")
                    nc.scalar.activation(scr[:], zp[mb][:], AF.Relu, accum_out=acc)
                if mb == i // 4:
                    off = (i * 128) % MB
                    dsel = scrv.tile([128, 128], FP, tag="dsel")
                    nc.gpsimd.affine_select(
                        out=dsel[:], in_=scr[:, off:off + 128], compare_op=AL.is_equal,
                        fill=0.0, base=0, pattern=[[-1, 128]], channel_multiplier=1)
                    nc.vector.tensor_reduce(dg8[:, i:i + 1], dsel[:], AX.X, AL.add)
        rs8 = small.tile([128, HCH], FP, tag="rs8")
        nc.vector.tensor_reduce(rs8[:], rc32[:].rearrange("p (i m) -> p i m", m=4),
                                AX.X, AL.add)
        nc.vector.reciprocal(rs8[:], rs8[:])
        nc.vector.tensor_tensor(dl[:], dg8[:], rs8[:], AL.mult)

    # ======================= GCN tail =======================
    with tc.tile_pool(name="jps", bufs=4, space="PSUM") as jps, \
         tc.tile_pool(name="jw", bufs=2) as jw:
        ptd = jps.tile([HCH, 128], FP, tag="jt", padded_shape=[HCH, 512])
        _tp(nc, ptd[:], dl[:], ident)
        s8d = small.tile([HCH, 128], BF, tag="s8d")
        nc.vector.tensor_copy(s8d[:], ptd[:])
        dl_row = small.tile([1, HALF], BF, tag="dl_row")
        nc.sync.dma_start(dl_row[:].rearrange("o (i p) -> o i p", p=128), s8d[:])
        dls = small.tile([64, HALF], BF, tag="dls")
        for jb in range(2):
            sl = slice(jb * MB, (jb + 1) * MB)
            dp = jps.tile([64, MB], FP, tag="jt", name=f"dlsb_{jb}")
            nc.tensor.matmul(dp[:], onesr64, dl_row[:, sl], start=True, stop=True)
            nc.scalar.copy(dls[:, sl], dp[:])

        for (wt, rhs, dst) in ((w1a, x1aug, hca), (w2a, hca, hcb), (w3a, hcb, None)):
            for jb in range(2):
                sl = slice(jb * MB, (jb + 1) * MB)
                ph = jps.tile([64, MB], FP, tag="jt", name=f"ph_{jb}")
                nc.tensor.matmul(ph[:], wt, rhs[:, sl], start=True, stop=True)
                if dst is not None:
                    nc.vector.tensor_tensor(dst[0:64, sl], ph[:], dls[:, sl], AL.mult)
                else:
                    hn = jw.tile([64, MB], FP, tag="hn")
                    nc.vector.tensor_tensor(hn[:], ph[:], dls[:, sl], AL.mult)
                    nc.vector.tensor_tensor(hn[:], hn[:], x1aug[0:64, sl], AL.add)
                    for ii in range(4):
                        i = jb * 4 + ii
                        pt = jps.tile([128, 64], FP, tag="jt", name=f"fin_{i}")
                        _tp(nc, pt[:], hn[:, ii * 128:(ii + 1) * 128], ident)
                        (nc.vector.tensor_copy if ii % 2 == 0 else nc.scalar.copy)(
                            fin[:, i * 64:(i + 1) * 64], pt[:])
        _ln_rows(nc, small, fin, fin, brows["lng"][:], brows["lnb"][:], HCH, "lnf", epsc)
        nc.sync.dma_start(io["outH"].rearrange("(i p) e -> p i e", p=128),
                          fin[:].rearrange("p (i e) -> p i e", e=64))


def _build():
    if "nc" in _CACHE:
        return _CACHE["nc"]
    nc = bacc.Bacc("TRN2", target_bir_lowering=False, debug=False,
                   enable_asserts=True, num_devices=8)
    io = {}

    io["xT"] = nc.dram_tensor("xT", [G, N], BF, kind="ExternalInput").ap()
    io["lastT"] = nc.dram_tensor("lastT", [G, N], BF, kind="ExternalInput").ap()
    io["origH"] = nc.dram_tensor("origH", [HALF, E], FP, kind="ExternalInput").ap()
    io["corr4"] = nc.dram_tensor("corr4", [4, N], BF, kind="ExternalInput").ap()
    io["wpack"] = nc.dram_tensor("wpack", [128, WPACK_W], BF, kind="ExternalInput").ap()
    io["fpack"] = nc.dram_tensor("fpack", [128, FPACK_W], FP, kind="ExternalInput").ap()
    io["outH"] = nc.dram_tensor("outH", [HALF, E], FP, kind="ExternalOutput").ap()
    io["lastH"] = nc.dram_tensor("lastH", [HALF, G], FP, kind="ExternalOutput").ap()

    with tile.TileContext(nc) as tc:
        with ExitStack() as ctx:
            _emit(ctx, tc, io)
    nc.compile()
    nc.m = get_hw_module(nc.m)
    _CACHE["nc"] = nc
    return nc


def _host_prep(inputs):
    f32 = np.float32
    bf = ml_dtypes.bfloat16
    inp = {k: np.asarray(v, f32) for k, v in inputs.items()}
    ch = 1.0 + inp["mlp_w"].sum(axis=0)
    assert (ch > 0).all(), "head-mixing scale fold requires positive c_h"
    g, b = inp["bn_g"], inp["bn_b"]
    qw_c = inp["q_w"] * np.repeat(ch / np.sqrt(G), G)[None, :]
    Wq = g[:, None] * qw_c
    qA = np.concatenate([Wq, Wq.sum(axis=0)[None], (b @ qw_c)[None]], axis=0)
    Wk = g[:, None] * inp["k_w"]
    kA = np.concatenate([Wk, Wk.sum(axis=0)[None], (b @ inp["k_w"])[None]], axis=0)
    w1 = inp["gcn_w1"]
    w1a = np.concatenate([w1, -(inp["gcn_b3"] @ w1)[None]], axis=0)
    w2a = np.concatenate([inp["gcn_w2"], (inp["gcn_b1"] @ inp["gcn_w2"])[None]], axis=0)
    w3a = np.concatenate([inp["gcn_w3"], (inp["gcn_b2"] @ inp["gcn_w3"])[None]], axis=0)
    fc3a = np.concatenate([inp["fc3_w"], inp["fc3_b"][None, :]], axis=0)

    wpack = np.zeros((128, WPACK_W), f32)
    wpack[0:128, W_IDB:W_IDB + 128] = np.eye(128)
    wpack[0:128, W_WZ:W_WZ + 64] = inp["w_z"]
    wpack[0:128, W_WR:W_WR + 64] = inp["w_r"]
    wpack[0:128, W_WH:W_WH + 64] = inp["w_h"]
    wpack[0:66, W_QA:W_QA + 256] = qA
    wpack[0:66, W_KA:W_KA + 256] = kA
    wpack[0:64, W_FC1:W_FC1 + 16] = inp["fc1_w"]
    wpack[0:16, W_FC2:W_FC2 + 2] = inp["fc2_w"]
    wpack[0:3, W_FC3A:W_FC3A + 64] = fc3a
    wpack[0:65, W_W1A:W_W1A + 64] = w1a
    wpack[0:65, W_W2A:W_W2A + 64] = w2a
    wpack[0:65, W_W3A:W_W3A + 64] = w3a
    wpack[0:64, W_SEL:W_SEL + 1] = 1.0
    wpack[64:128, W_SEL + 1:W_SEL + 2] = 1.0
    wpack[:, W_ONE:W_ONE + 128] = 1.0

    fpack = np.zeros((128, FPACK_W), f32)
    fpack[0:128, F_IDF:F_IDF + 128] = np.eye(128)
    fpack[0:16, F_B + 0] = inp["fc1_b"]
    fpack[0:2, F_B + 1] = inp["fc2_b"]
    fpack[0:64, F_B + 2] = inp["fc3_b"]
    fpack[0:64, F_B + 3] = inp["gcn_b3"]
    fpack[0:128, F_EPS] = EPS
    for k, nm in enumerate(("bn_g", "bn_b", "x_nom_g", "x_nom_b",
                            "last_nom_g", "last_nom_b")):
        fpack[0, F_BN + 64 * k:F_BN + 64 * (k + 1)] = inp[nm]

    def c(a, dt=bf):
        return np.ascontiguousarray(np.asarray(a, dt))

    shared = {"wpack": c(wpack), "fpack": c(fpack, f32)}
    in_maps = []
    for core in range(8):
        bi, h = core // 2, core % 2
        off = h * HALF
        corr4 = np.stack([
            np.roll(inp["attn_norm_g"], -off),
            np.roll(inp["skip_norm_g"], -off),
            np.roll(inp["attn_norm_b"] + inp["skip_norm_b"], -off),
            np.ones(N, f32),
        ])
        m = dict(shared)
        m["xT"] = c(np.roll(inp["x"][bi], -off, axis=0).T)
        m["lastT"] = c(np.roll(inp["last_G_emb"][bi], -off, axis=0).T)
        m["origH"] = c(inp["orig_x"][bi, off:off + HALF], f32)
        m["corr4"] = c(corr4)
        in_maps.append(m)
    return in_maps


def run(inputs, trace=False):
    nc = _build()
    in_maps = _host_prep(inputs)
    res = run_bass_kernel_spmd(nc, in_maps, core_ids=list(range(8)), trace=trace)
    out = np.zeros((B, N, E), np.float32)
    last = np.zeros((B, N, G), np.float32)
    for core in range(8):
        bi, h = core // 2, core % 2
        off = h * HALF
        out[bi, off:off + HALF] = res.results[core]["outH"]
        last[bi, off:off + HALF] = res.results[core]["lastH"]
    return (out, last), res


def kernel(**inputs):
    return run(inputs)[0]


# revision 11
# speedup vs baseline: 1.4154x; 1.0394x over previous
"""Trainium2 Bass kernel for nn_DGCN (gnn_message_passing).

Sharding: 8 shards = (batch b in 0..3, row-half h in 0..1). Each core gets
the full 2048-node K-side tensors of its batch with the node axis ROTATED
by h*1024 so the adjacency diagonal lands at the same tile position on
every core (uniform SPMD program); the core computes rows 0..1023 of the
rotated order, which are rows [h*1024, (h+1)*1024) of the original order.

v4 — breadth-first emission (engine queues are strict FIFO; depth-first
emission head-of-line blocks every queue), row-layout LN statistics:
 - All heavy tensors bf16; host pre-transposes x/last/orig; small params
   arrive in two packed mega-tiles (one bf16, one fp32) = 2 DMAs.
 - Head-mix scalars c_h and the Hg-LayerNorm gain fold into q_w/k_w; the
   LN shift becomes an extra contraction row of an augmented [66 x 256]
   weight (rhs rows = [Hg*a ; c ; 1]).
 - Per-node LN stats (Hg, xo, att/soc rows) are computed as [1/2, 512]
   PSUM rows by matmuls against ones/selector weights, scattered into
   [8/16, 128] chunk layout by SBUF-SBUF DMA for the pointwise math, and
   scattered back as broadcast rows fed to K=1 matmuls.
 - xo LayerNorm runs in T layout (feature axis on partitions) with the
   gain applied as a per-partition activation scale; no row-major xo.
 - diag(L)_i = relu(bracket)_ii / rowsum_j relu(bracket)_ij is invariant
   to positive per-row scales, so the 1/sdA row scale of the fused
   pre-relu matrix cancels; only the x3 lhsT rows carry rsS/rsA and the
   stat rows carry {-muA, -rho*muS, sdA}.
 - relu row-sums via accum_out; GCN biases ride the next layer's matmul
   as host-folded b@W rows against a constant ones row.
 - Phase I is software-pipelined two chunks deep over all 8 PSUM banks.
"""

import sys

if '/opt/trn_rl_repo' not in sys.path:
    sys.path.insert(0, '/opt/trn_rl_repo')

from contextlib import ExitStack

import numpy as np
import ml_dtypes

import concourse.bass as bass
import concourse.tile as tile
from concourse import bacc, mybir
from concourse.bass_interp import get_hw_module
from concourse.bass_utils import run_bass_kernel_spmd

FP = mybir.dt.float32
BF = mybir.dt.bfloat16
AL = mybir.AluOpType
AF = mybir.ActivationFunctionType
AX = mybir.AxisListType

B, N, E, G, H = 4, 2048, 64, 64, 4
D = H * G          # 256
HALF = N // 2      # own rows per core
NCH = N // 128     # 16 chunks over all nodes
HCH = HALF // 128  # 8 own chunks
MB = 512
NMB = N // MB      # 4
EPS = 1e-5

# wpack (bf16 [128, WPACK_W]) column layout
W_IDB, W_WZ, W_WR, W_WH = 0, 128, 192, 256
W_QA, W_KA = 320, 576
W_FC1, W_FC2, W_FC3A = 832, 848, 850
W_W1A, W_W2A, W_W3A = 914, 978, 1042
W_SEL, W_ONE = 1106, 1108
WPACK_W = 1280
# fpack (fp32 [128, FPACK_W]) column layout
F_IDF, F_B, F_EPS, F_XG, F_XB3, F_BN = 0, 128, 132, 133, 134, 136
FPACK_W = 528

_CACHE = {}


def _tp(nc, out_ap, in_ap, ident):
    k = in_ap.partition_size()
    nc.tensor.transpose(out_ap, in_ap, ident[0:k, 0:k])


def _leaky(nc, dst):
    nc.vector.scalar_tensor_tensor(dst, dst, 0.01, dst, AL.mult, AL.max)


def _ln_rows(nc, pool, t_in, t_out, g_b, b_b, ngr, tag, epsc):
    """LayerNorm over 64-wide groups: t_in [128, ngr*64] -> t_out."""
    a3 = t_in[:].rearrange("p (g e) -> p g e", e=64)
    o3 = t_out[:].rearrange("p (g e) -> p g e", e=64)
    sm = pool.tile([128, ngr], FP, tag=f"{tag}_sm")
    nc.vector.tensor_reduce(sm[:], a3, AX.X, AL.add)
    sq = pool.tile([128, ngr * 64], FP, tag=f"{tag}_sq")
    nc.scalar.square(sq[:], t_in[:])
    sqs = pool.tile([128, ngr], FP, tag=f"{tag}_sqs")
    nc.vector.tensor_reduce(sqs[:], sq[:].rearrange("p (g e) -> p g e", e=64),
                            AX.X, AL.add)
    mu = pool.tile([128, ngr], FP, tag=f"{tag}_mu")
    nc.vector.tensor_scalar(mu[:], sm[:], 1.0 / 64, None, AL.mult)
    mu2 = pool.tile([128, ngr], FP, tag=f"{tag}_mu2")
    nc.vector.tensor_tensor(mu2[:], mu[:], mu[:], AL.mult)
    var = pool.tile([128, ngr], FP, tag=f"{tag}_var")
    nc.vector.scalar_tensor_tensor(var[:], sqs[:], 1.0 / 64, mu2[:], AL.mult,
                                   AL.subtract)
    sd = pool.tile([128, ngr], FP, tag=f"{tag}_sd")
    nc.scalar.activation(sd[:], var[:], AF.Sqrt, bias=epsc)
    rs = pool.tile([128, ngr], FP, tag=f"{tag}_rs")
    nc.vector.reciprocal(rs[:], sd[:])
    mu_b = mu[:].unsqueeze(2).broadcast_to([128, ngr, 64])
    rs_b = rs[:].unsqueeze(2).broadcast_to([128, ngr, 64])
    g3 = g_b.unsqueeze(1).broadcast_to([128, ngr, 64])
    b3 = b_b.unsqueeze(1).broadcast_to([128, ngr, 64])
    xc = pool.tile([128, ngr * 64], FP, tag=f"{tag}_xc")
    xc3 = xc[:].rearrange("p (g e) -> p g e", e=64)
    nc.vector.tensor_tensor(xc3, a3, mu_b, AL.subtract)
    nc.vector.tensor_tensor(xc3, xc3, rs_b, AL.mult)
    nc.vector.tensor_tensor(xc3, xc3, g3, AL.mult)
    nc.vector.tensor_tensor(o3, xc3, b3, AL.add)


def _stat_land(nc, small, sum_t, sq_t, nch, tag, epsc, inv):
    """[nch,128] sums/sumsq -> (a, c) = (1/sd, -mu/sd), both [nch, 128] fp32."""
    mu = small.tile([nch, 128], FP, tag=f"{tag}_mu", name=f"{tag}_mu")
    nc.vector.tensor_scalar(mu[:], sum_t, inv, None, AL.mult)
    mu2 = small.tile([nch, 128], FP, tag=f"{tag}_mu2", name=f"{tag}_mu2")
    nc.vector.tensor_tensor(mu2[:], mu[:], mu[:], AL.mult)
    var = small.tile([nch, 128], FP, tag=f"{tag}_var", name=f"{tag}_var")
    nc.vector.scalar_tensor_tensor(var[:], sq_t, inv, mu2[:], AL.mult, AL.subtract)
    sd = small.tile([nch, 128], FP, tag=f"{tag}_sd", name=f"{tag}_sd")
    nc.scalar.activation(sd[:], var[:], AF.Sqrt, bias=epsc)
    a = small.tile([nch, 128], FP, tag=f"{tag}_a", name=f"{tag}_a")
    nc.vector.reciprocal(a[:], sd[:])
    c = small.tile([nch, 128], FP, tag=f"{tag}_c", name=f"{tag}_c")
    nc.vector.scalar_tensor_tensor(c[:], mu[:], -1.0, a[:], AL.mult, AL.mult)
    return mu, sd, a, c


def _emit(ctx: ExitStack, tc: tile.TileContext, io: dict):
    nc = tc.nc

    persist = ctx.enter_context(tc.tile_pool(name="persist", bufs=1))
    small = ctx.enter_context(tc.tile_pool(name="small", bufs=1))

    # ---------------- packed params (2 DMAs) ----------------
    wp = persist.tile([128, WPACK_W], BF, tag="wp")
    nc.sync.dma_start(wp[:], io["wpack"][:])
    fp_ = persist.tile([128, FPACK_W], FP, tag="fp_")
    nc.sync.dma_start(fp_[:], io["fpack"][:])

    identb = wp[:, W_IDB:W_IDB + 128]
    wz = wp[:, W_WZ:W_WZ + 64]
    wr = wp[:, W_WR:W_WR + 64]
    wh = wp[:, W_WH:W_WH + 64]
    kA = wp[0:66, W_KA:W_KA + 256]
    fc1s = wp[0:64, W_FC1:W_FC1 + 16]
    fc2s = wp[0:16, W_FC2:W_FC2 + 2]
    fc3s = wp[0:2, W_FC3A:W_FC3A + 64]
    fc3a = wp[0:3, W_FC3A:W_FC3A + 64]
    w1a = wp[0:65, W_W1A:W_W1A + 64]
    w2a = wp[0:65, W_W2A:W_W2A + 64]
    w3a = wp[0:65, W_W3A:W_W3A + 64]
    sel2 = wp[:, W_SEL:W_SEL + 2]
    ones128c = wp[:, W_ONE:W_ONE + 1]
    ones64c = wp[0:64, W_ONE:W_ONE + 1]
    onesr128 = wp[0:1, W_ONE:W_ONE + 128]
    onesr64 = wp[0:1, W_ONE:W_ONE + 64]

    ident = fp_[:, F_IDF:F_IDF + 128]
    fc1b = fp_[0:16, F_B + 0:F_B + 1]
    fc2b = fp_[0:2, F_B + 1:F_B + 2]
    fc3b = fp_[0:64, F_B + 2:F_B + 3]
    epsc128 = fp_[0:128, F_EPS:F_EPS + 1]
    epsc16 = fp_[0:16, F_EPS:F_EPS + 1]
    epsc8 = fp_[0:8, F_EPS:F_EPS + 1]
    xng_c = fp_[0:64, F_XG:F_XG + 1]
    xb3_c = fp_[0:64, F_XB3:F_XB3 + 1]

    # LN parameter rows -> [128, 64] broadcast tiles via gpsimd (small)
    brows = {}
    for k, nm in enumerate(("bng", "bnb", "lng", "lnb")):
        t = persist.tile([128, 64], FP, tag=f"{nm}_b", name=f"{nm}_b")
        nc.gpsimd.partition_broadcast(
            t[:], fp_[0:1, F_BN + 64 * k:F_BN + 64 * (k + 1)])
        brows[nm] = t

    # ---------------- big persistent tiles ----------------
    xT = persist.tile([64, N], BF, tag="xT")
    lastT = persist.tile([64, N], BF, tag="lastT")
    c1 = persist.tile([128, N], BF, tag="c1")      # [x3 ; last]
    c2 = persist.tile([128, N], BF, tag="c2")      # [r*last ; x3]
    hgsq = persist.tile([128, N], BF, tag="hgsq")  # [Hg_raw ; Hg_raw^2]
    HgQ = persist.tile([66, N], BF, tag="HgQ")     # [Hg*a ; c ; 1]
    osq = persist.tile([128, HALF], BF, tag="osq")  # [origT ; origT^2]
    a_row = persist.tile([1, N], BF, tag="a_row")
    kT0 = persist.tile([128, N], BF, tag="kT0")
    kT1 = persist.tile([128, N], BF, tag="kT1")
    k2T0 = persist.tile([128, N], BF, tag="k2T0")
    k2T1 = persist.tile([128, N], BF, tag="k2T1")
    qT0 = persist.tile([128, HALF], BF, tag="qT0")
    qT1 = persist.tile([128, HALF], BF, tag="qT1")
    x3gs = persist.tile([67, N], BF, tag="x3gs")   # [x3*gs ; ga ; gs ; cb]
    x3rA = persist.tile([67, HALF], BF, tag="x3rA")
    ga_b = persist.tile([128, N], BF, tag="ga_b")
    gs_b = persist.tile([64, N], BF, tag="gs_b")
    ga_r = persist.tile([1, N], BF, tag="ga_r")
    gs_r = persist.tile([1, N], BF, tag="gs_r")
    gt_sb = persist.tile([128, 256], BF, tag="gt_sb")
    gb_sb = persist.tile([128, 256], BF, tag="gb_sb")
    gs_f = persist.tile([64, 64], BF, tag="gs_f")
    ks0 = persist.tile([128, 1], BF, tag="ks0")
    ks1 = persist.tile([128, 1], BF, tag="ks1")
    xsb = persist.tile([64, 1], BF, tag="xsb")
    rc32 = persist.tile([128, 4 * HCH], FP, tag="rc32")
    dg8 = persist.tile([128, HCH], FP, tag="dg8")
    dl = persist.tile([128, HCH], FP, tag="dl")
    x1T = persist.tile([16, N], BF, tag="x1T")
    x2a = persist.tile([3, N], BF, tag="x2a")      # [x2 ; 1]
    e01 = persist.tile([128, HALF], BF, tag="e01")
    essb = persist.tile([64, HALF], BF, tag="essb")
    cT_sb = persist.tile([128, NCH], FP, tag="cT_sb")
    lastR = persist.tile([128, HCH * 64], FP, tag="lastR")
    x1aug = persist.tile([65, HALF], BF, tag="x1aug")  # [xo^T + b3 ; 1]
    hca = persist.tile([65, HALF], BF, tag="hca")
    hcb = persist.tile([65, HALF], BF, tag="hcb")
    fin = persist.tile([128, HCH * 64], FP, tag="fin")

    # input loads
    nc.sync.dma_start(xT[:], io["xT"][:])
    nc.sync.dma_start(lastT[:], io["lastT"][:])
    nc.sync.dma_start(c1[64:128, :], io["lastT"][:])
    nc.sync.dma_start(osq[0:64, :], io["origT"][:])
    nc.sync.dma_start(x3gs[64:67, :], io["corr4"][0:3, :])
    nc.sync.dma_start(ga_r[:], io["corr4"][0:1, :])
    nc.sync.dma_start(gs_r[:], io["corr4"][1:2, :])
    # constant-ones rows
    nc.gpsimd.dma_start(HgQ[65:66, :], io["corr4"][3:4, :])
    nc.gpsimd.dma_start(x2a[2:3, :], io["corr4"][3:4, :])
    nc.gpsimd.dma_start(x1aug[64:65, :], io["corr4"][3:4, 0:HALF])
    nc.gpsimd.dma_start(hca[64:65, :], io["corr4"][3:4, 0:HALF])
    nc.gpsimd.dma_start(hcb[64:65, :], io["corr4"][3:4, 0:HALF])

    frontA = ExitStack()
    fps = frontA.enter_context(tc.tile_pool(name="fps", bufs=6, space="PSUM"))
    gw = frontA.enter_context(tc.tile_pool(name="gw", bufs=4))

    MBs = [slice(j * MB, (j + 1) * MB) for j in range(NMB)]
    HBs = [slice(j * MB, (j + 1) * MB) for j in range(2)]

    # ---- ga / gs broadcast tiles via K=1 matmuls (breadth) ----
    gps_ = [fps.tile([128, MB], FP, tag="fp", name=f"gab_{j}") for j in range(NMB)]
    gss_ = [fps.tile([64, MB], FP, tag="fp", name=f"gsb_{j}") for j in range(2)]
    for j in range(NMB):
        nc.tensor.matmul(gps_[j][:], onesr128, ga_r[:, MBs[j]], start=True, stop=True)
    for j in range(2):
        nc.tensor.matmul(gss_[j][:], onesr64, gs_r[:, j * MB:(j + 1) * MB],
                         start=True, stop=True)
    for j in range(NMB):
        (nc.vector.tensor_copy if j % 2 == 0 else nc.scalar.copy)(
            ga_b[:, MBs[j]], gps_[j][:])
    for j in range(2):
        (nc.scalar.copy if j % 2 == 0 else nc.vector.tensor_copy)(
            gs_b[:, slice(j * MB, (j + 1) * MB)], gss_[j][:])
    gss2_ = [fps.tile([64, MB], FP, tag="fp", name=f"gsb2_{j}") for j in range(2)]
    for j in range(2):
        sl = slice((2 + j) * MB, (3 + j) * MB)
        nc.tensor.matmul(gss2_[j][:], onesr64, gs_r[:, sl], start=True, stop=True)
        (nc.vector.tensor_copy if j % 2 == 0 else nc.scalar.copy)(
            gs_b[:, sl], gss2_[j][:])

    # ---- xo stats (input-only dependent, fills the early pipeline) ----
    nc.scalar.square(osq[64:128, :], osq[0:64, :])
    oxp = [fps.tile([2, MB], FP, tag="fp", name=f"oxp_{j}") for j in range(2)]
    for j in range(2):
        nc.tensor.matmul(oxp[j][:], sel2, osq[:, HBs[j]], start=True, stop=True)
    oxs = small.tile([2, HALF], FP, tag="oxs")
    for j in range(2):
        (nc.vector.tensor_copy if j == 0 else nc.scalar.copy)(oxs[:, HBs[j]], oxp[j][:])
    oxs0 = small.tile([HCH, 128], FP, tag="oxs0")
    nc.sync.dma_start(oxs0[:], oxs[0:1, :].rearrange("o (i p) -> o i p", p=128))
    oxs1 = small.tile([HCH, 128], FP, tag="oxs1")
    nc.sync.dma_start(oxs1[:], oxs[1:2, :].rearrange("o (i p) -> o i p", p=128))
    _, _, oa, oc = _stat_land(nc, small, oxs0[:], oxs1[:], HCH, "ox", epsc8, 1.0 / 64)
    oa8 = small.tile([HCH, 128], BF, tag="oa8")
    nc.vector.tensor_copy(oa8[:], oa[:])
    oc8 = small.tile([HCH, 128], BF, tag="oc8")
    nc.scalar.copy(oc8[:], oc[:])
    oar = small.tile([1, HALF], BF, tag="oar")
    nc.sync.dma_start(oar[:].rearrange("o (i p) -> o i p", p=128), oa8[:])
    ocr = small.tile([1, HALF], BF, tag="ocr")
    nc.sync.dma_start(ocr[:].rearrange("o (i p) -> o i p", p=128), oc8[:])

    # ============ hyper fc stack (breadth-first stages) ============
    xacc = small.tile([64, NMB], FP, tag="xacc")
    p1 = [fps.tile([16, MB], FP, tag="fp", name=f"p1_{j}") for j in range(NMB)]
    for j in range(NMB):
        nc.tensor.matmul(p1[j][:], fc1s, xT[:, MBs[j]], start=True, stop=True)
    for j in range(NMB):
        nc.scalar.activation(x1T[:, MBs[j]], p1[j][:], AF.Sigmoid, bias=fc1b)
    p2 = [fps.tile([2, MB], FP, tag="fp", name=f"p2_{j}") for j in range(NMB)]
    for j in range(NMB):
        nc.tensor.matmul(p2[j][:], fc2s, x1T[:, MBs[j]], start=True, stop=True)
    for j in range(NMB):
        nc.scalar.activation(x2a[0:2, MBs[j]], p2[j][:], AF.Sigmoid, bias=fc2b)
    p3 = [fps.tile([64, MB], FP, tag="fp", name=f"p3_{j}") for j in range(NMB)]
    for j in range(NMB):
        nc.tensor.matmul(p3[j][:], fc3s, x2a[0:2, MBs[j]], start=True, stop=True)
    for j in range(NMB):
        nc.scalar.activation(c1[0:64, MBs[j]], p3[j][:], AF.Identity, bias=fc3b,
                             accum_out=xacc[:, j:j + 1])
    for j in range(NMB):
        nc.vector.tensor_copy(c2[64:128, MBs[j]], c1[0:64, MBs[j]])
    xs_f = small.tile([64, 1], FP, tag="xs_f")
    nc.vector.tensor_reduce(xs_f[:], xacc[:], AX.X, AL.add)
    nc.vector.tensor_copy(xsb[:], xs_f[:])

    # ================= GRU gates (breadth-first stages) =================
    zp = [fps.tile([64, MB], FP, tag="fp", name=f"zp_{j}") for j in range(NMB)]
    for j in range(NMB):
        nc.tensor.matmul(zp[j][:], wz, c1[:, MBs[j]], start=True, stop=True)
    zt = [gw.tile([64, MB], BF, tag="zt", name=f"zt_{j}") for j in range(NMB)]
    for j in range(NMB):
        nc.scalar.activation(zt[j][:], zp[j][:], AF.Sigmoid)
    rp = [fps.tile([64, MB], FP, tag="fp", name=f"rp_{j}") for j in range(NMB)]
    for j in range(NMB):
        nc.tensor.matmul(rp[j][:], wr, c1[:, MBs[j]], start=True, stop=True)
    rt = [gw.tile([64, MB], BF, tag="rt", name=f"rt_{j}") for j in range(NMB)]
    for j in range(NMB):
        nc.scalar.activation(rt[j][:], rp[j][:], AF.Sigmoid)
    for j in range(NMB):
        nc.vector.tensor_tensor(c2[0:64, MBs[j]], rt[j][:], lastT[:, MBs[j]], AL.mult)
    hp = [fps.tile([64, MB], FP, tag="fp", name=f"hp_{j}") for j in range(NMB)]
    for j in range(NMB):
        nc.tensor.matmul(hp[j][:], wh, c2[:, MBs[j]], start=True, stop=True)
    ht = [gw.tile([64, MB], BF, tag="ht", name=f"ht_{j}") for j in range(NMB)]
    for j in range(NMB):
        nc.scalar.activation(ht[j][:], hp[j][:], AF.Tanh)
    dt_ = [gw.tile([64, MB], BF, tag="dt", name=f"dt_{j}") for j in range(NMB)]
    for j in range(NMB):
        nc.vector.tensor_tensor(dt_[j][:], ht[j][:], lastT[:, MBs[j]], AL.subtract)
    for j in range(NMB):
        nc.vector.tensor_tensor(dt_[j][:], dt_[j][:], zt[j][:], AL.mult)
    for j in range(NMB):
        nc.vector.tensor_tensor(hgsq[0:64, MBs[j]], dt_[j][:], lastT[:, MBs[j]], AL.add)

    # Hg^2 (scalar) then Hg LN stats rows
    nc.scalar.square(hgsq[64:128, 0:HALF], hgsq[0:64, 0:HALF])
    nc.scalar.square(hgsq[64:128, HALF:N], hgsq[0:64, HALF:N])
    hsp = [fps.tile([2, MB], FP, tag="fp", name=f"hsp_{j}") for j in range(NMB)]
    for j in range(NMB):
        nc.tensor.matmul(hsp[j][:], sel2, hgsq[:, MBs[j]], start=True, stop=True)
    hsum = small.tile([2, N], FP, tag="hsum")
    for j in range(NMB):
        (nc.vector.tensor_copy if j % 2 == 0 else nc.scalar.copy)(
            hsum[:, MBs[j]], hsp[j][:])
    hst0 = small.tile([NCH, 128], FP, tag="hst0")
    nc.sync.dma_start(hst0[:], hsum[0:1, :].rearrange("o (i p) -> o i p", p=128))
    hst1 = small.tile([NCH, 128], FP, tag="hst1")
    nc.sync.dma_start(hst1[:], hsum[1:2, :].rearrange("o (i p) -> o i p", p=128))
    _, _, ha, hc = _stat_land(nc, small, hst0[:], hst1[:], NCH, "hg", epsc16, 1.0 / 64)
    ha16 = small.tile([NCH, 128], BF, tag="ha16")
    nc.vector.tensor_copy(ha16[:], ha[:])
    hc16 = small.tile([NCH, 128], BF, tag="hc16")
    nc.scalar.copy(hc16[:], hc[:])
    nc.sync.dma_start(a_row[:].rearrange("o (i p) -> o i p", p=128), ha16[:])
    nc.sync.dma_start(HgQ[64:65, :].rearrange("o (i p) -> o i p", p=128), hc16[:])
    # c in chunk-column layout for the lastH bias path
    pcc = fps.tile([128, NCH], FP, tag="fp", name="pcc", padded_shape=[128, 512])
    _tp(nc, pcc[:], hc[:], ident)
    nc.scalar.copy(cT_sb[:], pcc[:])

    # HgA = Hg_raw * a (K=1 broadcast matmul + fused multiply from PSUM)
    ab = [fps.tile([64, MB], FP, tag="fp", name=f"ab_{j}") for j in range(NMB)]
    for j in range(NMB):
        nc.tensor.matmul(ab[j][:], onesr64, a_row[:, MBs[j]], start=True, stop=True)
    for j in range(NMB):
        nc.vector.tensor_tensor(HgQ[0:64, MBs[j]], hgsq[0:64, MBs[j]], ab[j][:],
                                AL.mult)

    # ===================== q / k projections =====================
    kacc = small.tile([128, 8], FP, tag="kacc")
    kjobs = []
    for half, dst in ((0, kT0), (1, kT1)):
        for j in range(NMB):
            kjobs.append((dst, slice(W_KA + 128 * half, W_KA + 128 * (half + 1)),
                          MBs[j], kacc[:, 4 * half + j:4 * half + j + 1]))
    qjobs = []
    for half, dst in ((0, qT0), (1, qT1)):
        for j in range(2):
            qjobs.append((dst, slice(W_QA + 128 * half, W_QA + 128 * (half + 1)),
                          HBs[j], None))
    kq_ps = []
    for idx, (dst, wsl, sl, acc) in enumerate(kjobs + qjobs):
        kp = fps.tile([128, MB], FP, tag="fp", name=f"kqp_{idx}")
        nc.tensor.matmul(kp[:], wp[0:66, wsl], HgQ[:, sl], start=True, stop=True)
        kq_ps.append(kp)
    for idx, (dst, wsl, sl, acc) in enumerate(kjobs + qjobs):
        nc.scalar.copy(dst[:, sl], kq_ps[idx][:])
        if acc is not None:
            nc.vector.scalar_tensor_tensor(dst[:, sl], dst[:, sl], 0.01, dst[:, sl],
                                           AL.mult, AL.max, accum_out=acc)
        else:
            _leaky(nc, dst[:, sl])
    ks_f = small.tile([128, 2], FP, tag="ks_f")
    nc.vector.tensor_reduce(ks_f[:], kacc[:].rearrange("p (h j) -> p h j", j=4),
                            AX.X, AL.add)
    nc.vector.tensor_copy(ks0[:], ks_f[:, 0:1])
    nc.vector.tensor_copy(ks1[:], ks_f[:, 1:2])
    # k2 = k * ga ; x3gs rows 0:64 = x3 * gs
    nc.vector.tensor_tensor(k2T0[:], kT0[:], ga_b[:], AL.mult)
    nc.vector.tensor_tensor(k2T1[:], kT1[:], ga_b[:], AL.mult)
    nc.vector.tensor_tensor(x3gs[0:64, :], c1[0:64, :], gs_b[:], AL.mult)

    frontA.close()

    # ===================== Gram matrices =====================
    with tc.tile_pool(name="gpsp", bufs=3, space="PSUM") as gpsp, \
         tc.tile_pool(name="krpp", bufs=3, space="PSUM") as krpp, \
         tc.tile_pool(name="krp", bufs=3) as krp:
        gt_ps = gpsp.tile([128, 256], FP, tag="g", padded_shape=[128, 512])
        gb_ps = gpsp.tile([128, 256], FP, tag="g", padded_shape=[128, 512])
        for mi in range(NCH):
            msl = slice(mi * 128, (mi + 1) * 128)
            krq = krpp.tile([128, 256], FP, tag="kr", padded_shape=[128, 512])
            nc.tensor.matmul(krq[:], HgQ[:, msl], kA, start=True, stop=True)
            kr = krp.tile([128, 256], BF, tag="kr")
            nc.scalar.copy(kr[:], krq[:])
            _leaky(nc, kr[:])
            nc.tensor.matmul(gt_ps[:], kr[:, 0:128], kr[:],
                             start=(mi == 0), stop=(mi == NCH - 1))
            nc.tensor.matmul(gb_ps[:], kr[:, 128:256], kr[:],
                             start=(mi == 0), stop=(mi == NCH - 1))
        nc.vector.tensor_copy(gt_sb[:], gt_ps[:])
        nc.scalar.copy(gb_sb[:], gb_ps[:])
        gs_ps = gpsp.tile([64, 64], FP, tag="g", padded_shape=[64, 512])
        for mi in range(NCH):
            msl = slice(mi * 128, (mi + 1) * 128)
            xrq = krpp.tile([128, 64], FP, tag="kr", padded_shape=[128, 512])
            nc.tensor.matmul(xrq[:], x2a[:, msl], fc3a, start=True, stop=True)
            xr = krp.tile([128, 64], BF, tag="xr")
            (nc.vector.tensor_copy if mi % 2 == 0 else nc.scalar.copy)(xr[:], xrq[:])
            nc.tensor.matmul(gs_ps[:], xr[:], xr[:],
                             start=(mi == 0), stop=(mi == NCH - 1))
        nc.vector.tensor_copy(gs_f[:], gs_ps[:])

    # ============ lastH output (Hg LN rows, own half) ============
    with tc.tile_pool(name="lps", bufs=2, space="PSUM") as lps:
        lpt = []
        for i in range(HCH):
            pt = lps.tile([128, 64], BF, tag="lpt", name=f"lpt_{i}",
                          padded_shape=[128, 1024])
            _tp(nc, pt[:], HgQ[0:64, i * 128:(i + 1) * 128], identb)
            lpt.append(pt)
            nc.scalar.activation(lastR[:, i * 64:(i + 1) * 64], pt[:], AF.Identity,
                                 bias=cT_sb[:, i:i + 1])
    l3 = lastR[:].rearrange("p (g e) -> p g e", e=64)
    lg3 = brows["bng"][:].unsqueeze(1).broadcast_to([128, HCH, 64])
    lb3 = brows["bnb"][:].unsqueeze(1).broadcast_to([128, HCH, 64])
    nc.vector.tensor_tensor(l3, l3, lg3, AL.mult)
    nc.vector.tensor_tensor(l3, l3, lb3, AL.add)
    nc.sync.dma_start(io["lastH"].rearrange("(i p) e -> p i e", p=128),
                      lastR[:].rearrange("p (i e) -> p i e", e=64))

    # ============== own-row stats: S1, T1, S2, T2 rows ==============
    statq = ExitStack()
    ups = statq.enter_context(tc.tile_pool(name="ups", bufs=2, space="PSUM"))
    sps = statq.enter_context(tc.tile_pool(name="sps", bufs=2, space="PSUM"))
    ut0 = ups.tile([128, HALF], FP, tag="ut")
    ut1 = ups.tile([128, HALF], FP, tag="ut")
    for jb in range(2):
        sl = HBs[jb]
        nc.tensor.matmul(ut0[:, sl], gt_sb[:, 0:128], qT0[:, sl], start=True, stop=False)
        nc.tensor.matmul(ut0[:, sl], gb_sb[:, 0:128], qT1[:, sl], start=False, stop=True)
        nc.tensor.matmul(ut1[:, sl], gt_sb[:, 128:256], qT0[:, sl], start=True, stop=False)
        nc.tensor.matmul(ut1[:, sl], gb_sb[:, 128:256], qT1[:, sl], start=False, stop=True)
    e1t = small.tile([128, HALF], BF, tag="e1t")
    nc.vector.tensor_tensor(e01[:], ut0[:], qT0[:], AL.mult)
    nc.vector.tensor_tensor(e1t[:], ut1[:], qT1[:], AL.mult)
    nc.vector.tensor_tensor(e01[:], e01[:], e1t[:], AL.add)
    us = ups.tile([64, HALF], FP, tag="ut")
    for jb in range(2):
        nc.tensor.matmul(us[:, HBs[jb]], gs_f[:], c1[0:64, HBs[jb]], start=True,
                         stop=True)
    nc.vector.tensor_tensor(essb[:], us[:], c1[0:64, 0:HALF], AL.mult)

    s1p = sps.tile([1, HALF], FP, tag="st", padded_shape=[1, 1024])
    t1p = sps.tile([1, HALF], FP, tag="st", padded_shape=[1, 1024])
    for jb in range(2):
        sl = HBs[jb]
        nc.tensor.matmul(s1p[:, sl], ks0[:], qT0[:, sl], start=True, stop=False)
        nc.tensor.matmul(s1p[:, sl], ks1[:], qT1[:, sl], start=False, stop=True)
        nc.tensor.matmul(t1p[:, sl], xsb[:], c1[0:64, sl], start=True, stop=True)
    s2p = sps.tile([1, HALF], FP, tag="st", padded_shape=[1, 1024])
    t2p = sps.tile([1, HALF], FP, tag="st", padded_shape=[1, 1024])
    for jb in range(2):
        nc.tensor.matmul(s2p[:, HBs[jb]], ones128c, e01[:, HBs[jb]], start=True,
                         stop=True)
        nc.tensor.matmul(t2p[:, HBs[jb]], ones64c, essb[:, HBs[jb]], start=True,
                         stop=True)
    s1sb = small.tile([1, HALF], FP, tag="s1sb")
    nc.scalar.copy(s1sb[:], s1p[:])
    t1sb = small.tile([1, HALF], FP, tag="t1sb")
    nc.vector.tensor_copy(t1sb[:], t1p[:])
    s2sb = small.tile([1, HALF], FP, tag="s2sb")
    nc.scalar.copy(s2sb[:], s2p[:])
    t2sb = small.tile([1, HALF], FP, tag="t2sb")
    nc.vector.tensor_copy(t2sb[:], t2p[:])
    s1t = small.tile([HCH, 128], FP, tag="s1t")
    nc.sync.dma_start(s1t[:], s1sb[:].rearrange("o (i p) -> o i p", p=128))
    t1t = small.tile([HCH, 128], FP, tag="t1t")
    nc.sync.dma_start(t1t[:], t1sb[:].rearrange("o (i p) -> o i p", p=128))
    s2t = small.tile([HCH, 128], FP, tag="s2t")
    nc.sync.dma_start(s2t[:], s2sb[:].rearrange("o (i p) -> o i p", p=128))
    t2t = small.tile([HCH, 128], FP, tag="t2t")
    nc.sync.dma_start(t2t[:], t2sb[:].rearrange("o (i p) -> o i p", p=128))

    # ---- xo affine into x1aug (fills the stats-land latency) ----
    oab = [sps.tile([64, MB], FP, tag="st", name=f"oab_{j}") for j in range(2)]
    for j in range(2):
        nc.tensor.matmul(oab[j][:], onesr64, oar[:, HBs[j]], start=True, stop=True)
    ocb = [sps.tile([64, MB], FP, tag="st", name=f"ocb_{j}") for j in range(2)]
    for j in range(2):
        nc.tensor.matmul(ocb[j][:], onesr64, ocr[:, HBs[j]], start=True, stop=True)
    for j in range(2):
        tb = small.tile([64, MB], BF, tag=f"oxt_{j}", name=f"oxt_{j}")
        nc.vector.tensor_tensor(tb[:], osq[0:64, HBs[j]], oab[j][:], AL.mult)
        nc.vector.tensor_tensor(tb[:], tb[:], ocb[j][:], AL.add)
        nc.scalar.activation(x1aug[0:64, HBs[j]], tb[:], AF.Identity,
                             scale=xng_c, bias=xb3_c)

    # ---- own stats land [8, 128] ----
    muA, sdA, rsA_, _cA = _stat_land(nc, small, s1t[:], s2t[:], HCH, "sa",
                                     epsc8, 1.0 / N)
    muS, sdS, rsS_, _cS = _stat_land(nc, small, t1t[:], t2t[:], HCH, "ss",
                                     epsc8, 1.0 / N)
    rho = small.tile([HCH, 128], FP, tag="rho")
    nc.vector.tensor_tensor(rho[:], rsS_[:], sdA[:], AL.mult)
    r64t = small.tile([HCH, 128], BF, tag="r64t")
    nc.vector.tensor_scalar(r64t[:], muA[:], -1.0, None, AL.mult)
    r65t = small.tile([HCH, 128], BF, tag="r65t")
    nc.vector.scalar_tensor_tensor(r65t[:], muS[:], -1.0, rho[:], AL.mult, AL.mult)
    r66t = small.tile([HCH, 128], BF, tag="r66t")
    nc.scalar.copy(r66t[:], sdA[:])
    rho16 = small.tile([HCH, 128], BF, tag="rho16")
    nc.scalar.copy(rho16[:], rho[:])
    nc.sync.dma_start(x3rA[64:65, :].rearrange("o (i p) -> o i p", p=128), r64t[:])
    nc.sync.dma_start(x3rA[65:66, :].rearrange("o (i p) -> o i p", p=128), r65t[:])
    nc.sync.dma_start(x3rA[66:67, :].rearrange("o (i p) -> o i p", p=128), r66t[:])
    rho_row = small.tile([1, HALF], BF, tag="rho_row")
    nc.sync.dma_start(rho_row[:].rearrange("o (i p) -> o i p", p=128), rho16[:])
    for jb in range(2):
        rp_ = sps.tile([64, MB], FP, tag="st", name=f"rhob_{jb}")
        nc.tensor.matmul(rp_[:], onesr64, rho_row[:, HBs[jb]], start=True, stop=True)
        nc.vector.tensor_tensor(x3rA[0:64, HBs[jb]], c1[0:64, HBs[jb]], rp_[:],
                                AL.mult)
    statq.close()

    # =================== phase I: fused adjacency ===================
    with tc.tile_pool(name="zps", bufs=8, space="PSUM") as zps, \
         tc.tile_pool(name="scrv", bufs=2) as scrv, \
         tc.tile_pool(name="scra", bufs=2) as scra:
        ztiles = {}

        def passes12(i):
            csl = slice(i * 128, (i + 1) * 128)
            zpt = [zps.tile([128, MB], FP, tag="zpt", name=f"zp_{i}_{m}")
                   for m in range(NMB)]
            ztiles[i] = zpt
            for mb in range(NMB):
                nc.tensor.matmul(zpt[mb][:], qT0[:, csl],
                                 k2T0[:, mb * MB:(mb + 1) * MB],
                                 start=True, stop=False)
            for mb in range(NMB):
                nc.tensor.matmul(zpt[mb][:], qT1[:, csl],
                                 k2T1[:, mb * MB:(mb + 1) * MB],
                                 start=False, stop=False)

        def pass3(i):
            csl = slice(i * 128, (i + 1) * 128)
            zpt = ztiles[i]
            for mb in range(NMB):
                nc.tensor.matmul(zpt[mb][:], x3rA[:, csl],
                                 x3gs[:, mb * MB:(mb + 1) * MB],
                                 start=False, stop=True)
            for mb in range(NMB):
                acc = rc32[:, 4 * i + mb:4 * i + mb + 1]
                if mb % 2 == 0:
                    scr = scrv.tile([128, MB], FP, tag="scr", name=f"scr_{i}_{mb}")
                    nc.vector.tensor_scalar(scr[:], zpt[mb][:], 0.0, None, AL.max,
                                            AL.add, accum_out=acc)
                else:
                    scr = scra.tile([128, MB], FP, tag="scr2", name=f"scr2_{i}_{mb}")
                    nc.scalar.activation(scr[:], zpt[mb][:], AF.Relu, accum_out=acc)
                if mb == i // 4:
                    off = (i * 128) % MB
                    dsel = scrv.tile([128, 128], FP, tag="dsel", name=f"dsel_{i}")
                    nc.gpsimd.affine_select(
                        out=dsel[:], in_=scr[:, off:off + 128],
                        compare_op=AL.is_equal, fill=0.0, base=0,
                        pattern=[[-1, 128]], channel_multiplier=1)
                    nc.vector.tensor_reduce(dg8[:, i:i + 1], dsel[:], AX.X, AL.add)

        passes12(0)
        passes12(1)
        for i in range(HCH):
            pass3(i)
            if i + 2 < HCH:
                passes12(i + 2)
        rs8 = small.tile([128, HCH], FP, tag="rs8")
        nc.vector.tensor_reduce(rs8[:], rc32[:].rearrange("p (i m) -> p i m", m=4),
                                AX.X, AL.add)
        nc.vector.reciprocal(rs8[:], rs8[:])
        nc.vector.tensor_tensor(dl[:], dg8[:], rs8[:], AL.mult)

    # ======================= GCN tail =======================
    with tc.tile_pool(name="jps", bufs=4, space="PSUM") as jps, \
         tc.tile_pool(name="jw", bufs=2) as jw:
        ptd = jps.tile([HCH, 128], FP, tag="jt", padded_shape=[HCH, 512])
        _tp(nc, ptd[:], dl[:], ident)
        s8d = small.tile([HCH, 128], BF, tag="s8d")
        nc.vector.tensor_copy(s8d[:], ptd[:])
        dl_row = small.tile([1, HALF], BF, tag="dl_row")
        nc.sync.dma_start(dl_row[:].rearrange("o (i p) -> o i p", p=128), s8d[:])
        dls = small.tile([64, HALF], BF, tag="dls")
        for jb in range(2):
            dp = jps.tile([64, MB], FP, tag="jt", name=f"dlsb_{jb}")
            nc.tensor.matmul(dp[:], onesr64, dl_row[:, HBs[jb]], start=True, stop=True)
            nc.scalar.copy(dls[:, HBs[jb]], dp[:])

        for (wt, rhs, dst) in ((w1a, x1aug, hca), (w2a, hca, hcb), (w3a, hcb, None)):
            ph = [jps.tile([64, MB], FP, tag="jt", name=f"ph_{id(wt)}_{jb}")
                  for jb in range(2)]
            for jb in range(2):
                nc.tensor.matmul(ph[jb][:], wt, rhs[:, HBs[jb]], start=True, stop=True)
            if dst is not None:
                for jb in range(2):
                    nc.vector.tensor_tensor(dst[0:64, HBs[jb]], ph[jb][:],
                                            dls[:, HBs[jb]], AL.mult)
            else:
                hn = [jw.tile([64, MB], FP, tag="hn", name=f"hn_{jb}")
                      for jb in range(2)]
                for jb in range(2):
                    nc.vector.tensor_tensor(hn[jb][:], ph[jb][:], dls[:, HBs[jb]],
                                            AL.mult)
                    nc.vector.tensor_tensor(hn[jb][:], hn[jb][:],
                                            x1aug[0:64, HBs[jb]], AL.add)
                for i in range(HCH):
                    jb, ii = i // 4, i % 4
                    pt = jps.tile([128, 64], FP, tag="jt", name=f"fin_{i}")
                    _tp(nc, pt[:], hn[jb][:, ii * 128:(ii + 1) * 128], ident)
                    (nc.vector.tensor_copy if i % 2 == 0 else nc.scalar.copy)(
                        fin[:, i * 64:(i + 1) * 64], pt[:])
        _ln_rows(nc, small, fin, fin, brows["lng"][:], brows["lnb"][:], HCH,
                 "lnf", epsc128)
        nc.sync.dma_start(io["outH"].rearrange("(i p) e -> p i e", p=128),
                          fin[:].rearrange("p (i e) -> p i e", e=64))


def _build():
    if "nc" in _CACHE:
        return _CACHE["nc"]
    nc = bacc.Bacc("TRN2", target_bir_lowering=False, debug=False,
                   enable_asserts=True, num_devices=8)
    io = {}
    io["xT"] = nc.dram_tensor("xT", [G, N], BF, kind="ExternalInput").ap()
    io["lastT"] = nc.dram_tensor("lastT", [G, N], BF, kind="ExternalInput").ap()
    io["origT"] = nc.dram_tensor("origT", [E, HALF], BF, kind="ExternalInput").ap()
    io["corr4"] = nc.dram_tensor("corr4", [4, N], BF, kind="ExternalInput").ap()
    io["wpack"] = nc.dram_tensor("wpack", [128, WPACK_W], BF, kind="ExternalInput").ap()
    io["fpack"] = nc.dram_tensor("fpack", [128, FPACK_W], FP, kind="ExternalInput").ap()
    io["outH"] = nc.dram_tensor("outH", [HALF, E], FP, kind="ExternalOutput").ap()
    io["lastH"] = nc.dram_tensor("lastH", [HALF, G], FP, kind="ExternalOutput").ap()

    with tile.TileContext(nc) as tc:
        with ExitStack() as ctx:
            _emit(ctx, tc, io)
    nc.compile()
    nc.m = get_hw_module(nc.m)
    _CACHE["nc"] = nc
    return nc


def _host_prep(inputs):
    f32 = np.float32
    bf = ml_dtypes.bfloat16
    inp = {k: np.asarray(v, f32) for k, v in inputs.items()}
    ch = 1.0 + inp["mlp_w"].sum(axis=0)
    assert (ch > 0).all(), "head-mixing scale fold requires positive c_h"
    g, b = inp["bn_g"], inp["bn_b"]
    qw_c = inp["q_w"] * np.repeat(ch / np.sqrt(G), G)[None, :]
    Wq = g[:, None] * qw_c
    qA = np.concatenate([Wq, Wq.sum(axis=0)[None], (b @ qw_c)[None]], axis=0)
    Wk = g[:, None] * inp["k_w"]
    kA = np.concatenate([Wk, Wk.sum(axis=0)[None], (b @ inp["k_w"])[None]], axis=0)
    w1 = inp["gcn_w1"]
    w1a = np.concatenate([w1, -(inp["gcn_b3"] @ w1)[None]], axis=0)
    w2a = np.concatenate([inp["gcn_w2"], (inp["gcn_b1"] @ inp["gcn_w2"])[None]], axis=0)
    w3a = np.concatenate([inp["gcn_w3"], (inp["gcn_b2"] @ inp["gcn_w3"])[None]], axis=0)
    fc3a = np.concatenate([inp["fc3_w"], inp["fc3_b"][None, :]], axis=0)

    wpack = np.zeros((128, WPACK_W), f32)
    wpack[0:128, W_IDB:W_IDB + 128] = np.eye(128)
    wpack[0:128, W_WZ:W_WZ + 64] = inp["w_z"]
    wpack[0:128, W_WR:W_WR + 64] = inp["w_r"]
    wpack[0:128, W_WH:W_WH + 64] = inp["w_h"]
    wpack[0:66, W_QA:W_QA + 256] = qA
    wpack[0:66, W_KA:W_KA + 256] = kA
    wpack[0:64, W_FC1:W_FC1 + 16] = inp["fc1_w"]
    wpack[0:16, W_FC2:W_FC2 + 2] = inp["fc2_w"]
    wpack[0:3, W_FC3A:W_FC3A + 64] = fc3a
    wpack[0:65, W_W1A:W_W1A + 64] = w1a
    wpack[0:65, W_W2A:W_W2A + 64] = w2a
    wpack[0:65, W_W3A:W_W3A + 64] = w3a
    wpack[0:64, W_SEL:W_SEL + 1] = 1.0
    wpack[64:128, W_SEL + 1:W_SEL + 2] = 1.0
    wpack[:, W_ONE:W_ONE + 128] = 1.0

    fpack = np.zeros((128, FPACK_W), f32)
    fpack[0:128, F_IDF:F_IDF + 128] = np.eye(128)
    fpack[0:16, F_B + 0] = inp["fc1_b"]
    fpack[0:2, F_B + 1] = inp["fc2_b"]
    fpack[0:64, F_B + 2] = inp["fc3_b"]
    fpack[0:128, F_EPS] = EPS
    fpack[0:64, F_XG] = inp["x_nom_g"]
    fpack[0:64, F_XB3] = inp["x_nom_b"] + inp["gcn_b3"]
    for k, nm in enumerate(("bn_g", "bn_b", "last_nom_g", "last_nom_b")):
        fpack[0, F_BN + 64 * k:F_BN + 64 * (k + 1)] = inp[nm]

    def c(a, dt=bf):
        return np.ascontiguousarray(np.asarray(a, dt))

    shared = {"wpack": c(wpack), "fpack": c(fpack, f32)}
    in_maps = []
    for core in range(8):
        bi, h = core // 2, core % 2
        off = h * HALF
        corr4 = np.stack([
            np.roll(inp["attn_norm_g"], -off),
            np.roll(inp["skip_norm_g"], -off),
            np.roll(inp["attn_norm_b"] + inp["skip_norm_b"], -off),
            np.ones(N, f32),
        ])
        m = dict(shared)
        m["xT"] = c(np.roll(inp["x"][bi], -off, axis=0).T)
        m["lastT"] = c(np.roll(inp["last_G_emb"][bi], -off, axis=0).T)
        m["origT"] = c(inp["orig_x"][bi, off:off + HALF].T)
        m["corr4"] = c(corr4)
        in_maps.append(m)
    return in_maps


def run(inputs, trace=False):
    nc = _build()
    in_maps = _host_prep(inputs)
    res = run_bass_kernel_spmd(nc, in_maps, core_ids=list(range(8)), trace=trace)
    out = np.zeros((B, N, E), np.float32)
    last = np.zeros((B, N, G), np.float32)
    for core in range(8):
        bi, h = core // 2, core % 2
        off = h * HALF
        out[bi, off:off + HALF] = res.results[core]["outH"]
        last[bi, off:off + HALF] = res.results[core]["lastH"]
    return (out, last), res


def kernel(**inputs):
    return run(inputs)[0]


# revision 14
# speedup vs baseline: 1.4255x; 1.0071x over previous
"""Trainium2 Bass kernel for nn_DGCN (gnn_message_passing).

Sharding: 8 shards = (batch b in 0..3, row-half h in 0..1). Each core gets
the full 2048-node K-side tensors of its batch with the node axis ROTATED
by h*1024 so the adjacency diagonal lands at the same tile position on
every core (uniform SPMD program); the core computes rows 0..1023 of the
rotated order, which are rows [h*1024, (h+1)*1024) of the original order.

v4 — breadth-first emission (engine queues are strict FIFO; depth-first
emission head-of-line blocks every queue), row-layout LN statistics:
 - All heavy tensors bf16; host pre-transposes x/last/orig; small params
   arrive in two packed mega-tiles (one bf16, one fp32) = 2 DMAs.
 - Head-mix scalars c_h and the Hg-LayerNorm gain fold into q_w/k_w; the
   LN shift becomes an extra contraction row of an augmented [66 x 256]
   weight (rhs rows = [Hg*a ; c ; 1]).
 - Per-node LN stats (Hg, xo, att/soc rows) are computed as [1/2, 512]
   PSUM rows by matmuls against ones/selector weights, scattered into
   [8/16, 128] chunk layout by SBUF-SBUF DMA for the pointwise math, and
   scattered back as broadcast rows fed to K=1 matmuls.
 - xo LayerNorm runs in T layout (feature axis on partitions) with the
   gain applied as a per-partition activation scale; no row-major xo.
 - diag(L)_i = relu(bracket)_ii / rowsum_j relu(bracket)_ij is invariant
   to positive per-row scales, so the 1/sdA row scale of the fused
   pre-relu matrix cancels; only the x3 lhsT rows carry rsS/rsA and the
   stat rows carry {-muA, -rho*muS, sdA}.
 - relu row-sums via accum_out; GCN biases ride the next layer's matmul
   as host-folded b@W rows against a constant ones row.
 - Phase I is software-pipelined two chunks deep over all 8 PSUM banks.
"""

import sys

if '/opt/trn_rl_repo' not in sys.path:
    sys.path.insert(0, '/opt/trn_rl_repo')

from contextlib import ExitStack

import numpy as np
import ml_dtypes

import concourse.bass as bass
import concourse.tile as tile
from concourse import bacc, mybir
from concourse.bass_interp import get_hw_module
from concourse.bass_utils import run_bass_kernel_spmd

FP = mybir.dt.float32
BF = mybir.dt.bfloat16
AL = mybir.AluOpType
AF = mybir.ActivationFunctionType
AX = mybir.AxisListType

B, N, E, G, H = 4, 2048, 64, 64, 4
D = H * G          # 256
HALF = N // 2      # own rows per core
NCH = N // 128     # 16 chunks over all nodes
HCH = HALF // 128  # 8 own chunks
MB = 512
NMB = N // MB      # 4
EPS = 1e-5

# wpack (bf16 [128, WPACK_W]) column layout
W_IDB, W_WZ, W_WR, W_WH = 0, 128, 192, 256
W_QA, W_KA = 320, 576
W_FC1, W_FC2, W_FC3A = 832, 848, 850
W_W1A, W_W2A, W_W3A = 914, 978, 1042
W_SEL, W_ONE = 1106, 1108
WPACK_W = 1280
# fpack (fp32 [128, FPACK_W]) column layout
F_IDF, F_B, F_EPS, F_XG, F_XB3, F_BN = 0, 128, 132, 133, 134, 136
FPACK_W = 528

_CACHE = {}


def _tp(nc, out_ap, in_ap, ident):
    k = in_ap.partition_size()
    nc.tensor.transpose(out_ap, in_ap, ident[0:k, 0:k])


def _leaky(nc, dst):
    nc.vector.scalar_tensor_tensor(dst, dst, 0.01, dst, AL.mult, AL.max)


def _ln_rows(nc, pool, t_in, t_out, g_b, b_b, ngr, tag, epsc):
    """LayerNorm over 64-wide groups: t_in [128, ngr*64] -> t_out."""
    a3 = t_in[:].rearrange("p (g e) -> p g e", e=64)
    o3 = t_out[:].rearrange("p (g e) -> p g e", e=64)
    sm = pool.tile([128, ngr], FP, tag=f"{tag}_sm")
    nc.vector.tensor_reduce(sm[:], a3, AX.X, AL.add)
    sq = pool.tile([128, ngr * 64], FP, tag=f"{tag}_sq")
    nc.scalar.square(sq[:], t_in[:])
    sqs = pool.tile([128, ngr], FP, tag=f"{tag}_sqs")
    nc.vector.tensor_reduce(sqs[:], sq[:].rearrange("p (g e) -> p g e", e=64),
                            AX.X, AL.add)
    mu = pool.tile([128, ngr], FP, tag=f"{tag}_mu")
    nc.vector.tensor_scalar(mu[:], sm[:], 1.0 / 64, None, AL.mult)
    mu2 = pool.tile([128, ngr], FP, tag=f"{tag}_mu2")
    nc.vector.tensor_tensor(mu2[:], mu[:], mu[:], AL.mult)
    var = pool.tile([128, ngr], FP, tag=f"{tag}_var")
    nc.vector.scalar_tensor_tensor(var[:], sqs[:], 1.0 / 64, mu2[:], AL.mult,
                                   AL.subtract)
    sd = pool.tile([128, ngr], FP, tag=f"{tag}_sd")
    nc.scalar.activation(sd[:], var[:], AF.Sqrt, bias=epsc)
    rs = pool.tile([128, ngr], FP, tag=f"{tag}_rs")
    nc.vector.reciprocal(rs[:], sd[:])
    mu_b = mu[:].unsqueeze(2).broadcast_to([128, ngr, 64])
    rs_b = rs[:].unsqueeze(2).broadcast_to([128, ngr, 64])
    g3 = g_b.unsqueeze(1).broadcast_to([128, ngr, 64])
    b3 = b_b.unsqueeze(1).broadcast_to([128, ngr, 64])
    xc = pool.tile([128, ngr * 64], FP, tag=f"{tag}_xc")
    xc3 = xc[:].rearrange("p (g e) -> p g e", e=64)
    nc.vector.tensor_tensor(xc3, a3, mu_b, AL.subtract)
    nc.vector.tensor_tensor(xc3, xc3, rs_b, AL.mult)
    nc.vector.tensor_tensor(xc3, xc3, g3, AL.mult)
    nc.vector.tensor_tensor(o3, xc3, b3, AL.add)


def _stat_land(nc, small, sum_t, sq_t, nch, tag, epsc, inv):
    """[nch,128] sums/sumsq -> (a, c) = (1/sd, -mu/sd), both [nch, 128] fp32."""
    mu = small.tile([nch, 128], FP, tag=f"{tag}_mu", name=f"{tag}_mu")
    nc.vector.tensor_scalar(mu[:], sum_t, inv, None, AL.mult)
    mu2 = small.tile([nch, 128], FP, tag=f"{tag}_mu2", name=f"{tag}_mu2")
    nc.vector.tensor_tensor(mu2[:], mu[:], mu[:], AL.mult)
    var = small.tile([nch, 128], FP, tag=f"{tag}_var", name=f"{tag}_var")
    nc.vector.scalar_tensor_tensor(var[:], sq_t, inv, mu2[:], AL.mult, AL.subtract)
    sd = small.tile([nch, 128], FP, tag=f"{tag}_sd", name=f"{tag}_sd")
    nc.scalar.activation(sd[:], var[:], AF.Sqrt, bias=epsc)
    a = small.tile([nch, 128], FP, tag=f"{tag}_a", name=f"{tag}_a")
    nc.vector.reciprocal(a[:], sd[:])
    c = small.tile([nch, 128], FP, tag=f"{tag}_c", name=f"{tag}_c")
    nc.vector.scalar_tensor_tensor(c[:], mu[:], -1.0, a[:], AL.mult, AL.mult)
    return mu, sd, a, c


def _emit(ctx: ExitStack, tc: tile.TileContext, io: dict):
    nc = tc.nc

    persist = ctx.enter_context(tc.tile_pool(name="persist", bufs=1))
    small = ctx.enter_context(tc.tile_pool(name="small", bufs=1))

    # ---------------- packed params (2 DMAs) ----------------
    wp = persist.tile([128, WPACK_W], BF, tag="wp")
    nc.sync.dma_start(wp[:], io["wpack"][:])
    fp_ = persist.tile([128, FPACK_W], FP, tag="fp_")
    nc.sync.dma_start(fp_[:], io["fpack"][:])

    identb = wp[:, W_IDB:W_IDB + 128]
    wz = wp[:, W_WZ:W_WZ + 64]
    wr = wp[:, W_WR:W_WR + 64]
    wh = wp[:, W_WH:W_WH + 64]
    kA = wp[0:66, W_KA:W_KA + 256]
    fc1s = wp[0:64, W_FC1:W_FC1 + 16]
    fc2s = wp[0:16, W_FC2:W_FC2 + 2]
    fc3s = wp[0:2, W_FC3A:W_FC3A + 64]
    fc3a = wp[0:3, W_FC3A:W_FC3A + 64]
    w1a = wp[0:65, W_W1A:W_W1A + 64]
    w2a = wp[0:65, W_W2A:W_W2A + 64]
    w3a = wp[0:65, W_W3A:W_W3A + 64]
    sel2 = wp[:, W_SEL:W_SEL + 2]
    ones128c = wp[:, W_ONE:W_ONE + 1]
    ones64c = wp[0:64, W_ONE:W_ONE + 1]
    onesr128 = wp[0:1, W_ONE:W_ONE + 128]
    onesr64 = wp[0:1, W_ONE:W_ONE + 64]

    ident = fp_[:, F_IDF:F_IDF + 128]
    fc1b = fp_[0:16, F_B + 0:F_B + 1]
    fc2b = fp_[0:2, F_B + 1:F_B + 2]
    fc3b = fp_[0:64, F_B + 2:F_B + 3]
    epsc128 = fp_[0:128, F_EPS:F_EPS + 1]
    epsc16 = fp_[0:16, F_EPS:F_EPS + 1]
    epsc8 = fp_[0:8, F_EPS:F_EPS + 1]
    xng_c = fp_[0:64, F_XG:F_XG + 1]
    xb3_c = fp_[0:64, F_XB3:F_XB3 + 1]

    # LN parameter rows -> [128, 64] broadcast tiles via gpsimd (small)
    brows = {}
    for k, nm in enumerate(("bng", "bnb", "lng", "lnb")):
        t = persist.tile([128, 64], FP, tag=f"{nm}_b", name=f"{nm}_b")
        nc.gpsimd.partition_broadcast(
            t[:], fp_[0:1, F_BN + 64 * k:F_BN + 64 * (k + 1)])
        brows[nm] = t

    # ---------------- big persistent tiles ----------------
    xT = persist.tile([64, N], BF, tag="xT")
    lastT = persist.tile([64, N], BF, tag="lastT")
    c1 = persist.tile([128, N], BF, tag="c1")      # [x3 ; last]
    c2 = persist.tile([128, N], BF, tag="c2")      # [r*last ; x3]
    hgsq = persist.tile([128, N], BF, tag="hgsq")  # [Hg_raw ; Hg_raw^2]
    HgQ = persist.tile([66, N], BF, tag="HgQ")     # [Hg*a ; c ; 1]
    osq = persist.tile([128, HALF], BF, tag="osq")  # [origT ; origT^2]
    a_row = persist.tile([1, N], BF, tag="a_row")
    kT0 = persist.tile([128, N], BF, tag="kT0")
    kT1 = persist.tile([128, N], BF, tag="kT1")
    k2T0 = persist.tile([128, N], BF, tag="k2T0")
    k2T1 = persist.tile([128, N], BF, tag="k2T1")
    qT0 = persist.tile([128, HALF], BF, tag="qT0")
    qT1 = persist.tile([128, HALF], BF, tag="qT1")
    x3gs = persist.tile([67, N], BF, tag="x3gs")   # [x3*gs ; ga ; gs ; cb]
    x3rA = persist.tile([67, HALF], BF, tag="x3rA")
    ga_b = persist.tile([128, N], BF, tag="ga_b")
    gs_b = persist.tile([64, N], BF, tag="gs_b")
    ga_r = persist.tile([1, N], BF, tag="ga_r")
    gs_r = persist.tile([1, N], BF, tag="gs_r")
    gt_sb = persist.tile([128, 256], BF, tag="gt_sb")
    gb_sb = persist.tile([128, 256], BF, tag="gb_sb")
    gs_f = persist.tile([64, 64], BF, tag="gs_f")
    ks0 = persist.tile([128, 1], BF, tag="ks0")
    ks1 = persist.tile([128, 1], BF, tag="ks1")
    xsb = persist.tile([64, 1], BF, tag="xsb")
    rc32 = persist.tile([128, 4 * HCH], FP, tag="rc32")
    dg8 = persist.tile([128, HCH], FP, tag="dg8")
    dl = persist.tile([128, HCH], FP, tag="dl")
    x1T = persist.tile([16, N], BF, tag="x1T")
    x2a = persist.tile([3, N], BF, tag="x2a")      # [x2 ; 1]
    e0sb = persist.tile([128, HALF], BF, tag="e0sb")
    e1sb = persist.tile([128, HALF], BF, tag="e1sb")
    essb = persist.tile([64, HALF], BF, tag="essb")
    ph1sb = persist.tile([64, HALF], BF, tag="ph1sb")
    finsq = persist.tile([128, HALF], BF, tag="finsq")
    cT_sb = persist.tile([128, NCH], FP, tag="cT_sb")
    lastR = persist.tile([128, HCH * 64], FP, tag="lastR")
    x1aug = persist.tile([65, HALF], BF, tag="x1aug")  # [xo^T + b3 ; 1]
    hca = persist.tile([65, HALF], BF, tag="hca")
    hcb = persist.tile([65, HALF], BF, tag="hcb")
    fin = persist.tile([128, HCH * 64], FP, tag="fin")

    # input loads
    nc.sync.dma_start(xT[:], io["xT"][:])
    nc.sync.dma_start(lastT[:], io["lastT"][:])
    nc.sync.dma_start(c1[64:128, :], io["lastT"][:])
    nc.sync.dma_start(osq[0:64, :], io["origT"][:])
    nc.sync.dma_start(x3gs[64:67, :], io["corr4"][0:3, :])
    nc.sync.dma_start(ga_r[:], io["corr4"][0:1, :])
    nc.sync.dma_start(gs_r[:], io["corr4"][1:2, :])
    # constant-ones rows
    nc.gpsimd.dma_start(HgQ[65:66, :], io["corr4"][3:4, :])
    nc.gpsimd.dma_start(x2a[2:3, :], io["corr4"][3:4, :])
    nc.gpsimd.dma_start(x1aug[64:65, :], io["corr4"][3:4, 0:HALF])
    nc.gpsimd.dma_start(hca[64:65, :], io["corr4"][3:4, 0:HALF])
    nc.gpsimd.dma_start(hcb[64:65, :], io["corr4"][3:4, 0:HALF])

    frontA = ExitStack()
    fps = frontA.enter_context(tc.tile_pool(name="fps", bufs=6, space="PSUM"))
    gw = frontA.enter_context(tc.tile_pool(name="gw", bufs=4))

    MBs = [slice(j * MB, (j + 1) * MB) for j in range(NMB)]
    HBs = [slice(j * MB, (j + 1) * MB) for j in range(2)]

    # ---- ga / gs broadcast tiles via K=1 matmuls (breadth) ----
    gps_ = [fps.tile([128, MB], FP, tag="fp", name=f"gab_{j}") for j in range(NMB)]
    gss_ = [fps.tile([64, MB], FP, tag="fp", name=f"gsb_{j}") for j in range(2)]
    for j in range(NMB):
        nc.tensor.matmul(gps_[j][:], onesr128, ga_r[:, MBs[j]], start=True, stop=True)
    for j in range(2):
        nc.tensor.matmul(gss_[j][:], onesr64, gs_r[:, j * MB:(j + 1) * MB],
                         start=True, stop=True)
    for j in range(NMB):
        (nc.vector.tensor_copy if j % 2 == 0 else nc.scalar.copy)(
            ga_b[:, MBs[j]], gps_[j][:])
    for j in range(2):
        (nc.scalar.copy if j % 2 == 0 else nc.vector.tensor_copy)(
            gs_b[:, slice(j * MB, (j + 1) * MB)], gss_[j][:])
    gss2_ = [fps.tile([64, MB], FP, tag="fp", name=f"gsb2_{j}") for j in range(2)]
    for j in range(2):
        sl = slice((2 + j) * MB, (3 + j) * MB)
        nc.tensor.matmul(gss2_[j][:], onesr64, gs_r[:, sl], start=True, stop=True)
        (nc.vector.tensor_copy if j % 2 == 0 else nc.scalar.copy)(
            gs_b[:, sl], gss2_[j][:])

    # ---- xo stats (input-only dependent, fills the early pipeline) ----
    nc.scalar.square(osq[64:128, :], osq[0:64, :])
    oxp = [fps.tile([2, MB], FP, tag="fp", name=f"oxp_{j}") for j in range(2)]
    for j in range(2):
        nc.tensor.matmul(oxp[j][:], sel2, osq[:, HBs[j]], start=True, stop=True)
    oxs = small.tile([2, HALF], FP, tag="oxs")
    for j in range(2):
        (nc.vector.tensor_copy if j == 0 else nc.scalar.copy)(oxs[:, HBs[j]], oxp[j][:])
    oxs0 = small.tile([HCH, 128], FP, tag="oxs0")
    nc.sync.dma_start(oxs0[:], oxs[0:1, :].rearrange("o (i p) -> o i p", p=128))
    oxs1 = small.tile([HCH, 128], FP, tag="oxs1")
    nc.sync.dma_start(oxs1[:], oxs[1:2, :].rearrange("o (i p) -> o i p", p=128))
    _, _, oa, oc = _stat_land(nc, small, oxs0[:], oxs1[:], HCH, "ox", epsc8, 1.0 / 64)
    oa8 = small.tile([HCH, 128], BF, tag="oa8")
    nc.vector.tensor_copy(oa8[:], oa[:])
    oc8 = small.tile([HCH, 128], BF, tag="oc8")
    nc.scalar.copy(oc8[:], oc[:])
    oar = small.tile([1, HALF], BF, tag="oar")
    nc.sync.dma_start(oar[:].rearrange("o (i p) -> o i p", p=128), oa8[:])
    ocr = small.tile([1, HALF], BF, tag="ocr")
    nc.sync.dma_start(ocr[:].rearrange("o (i p) -> o i p", p=128), oc8[:])

    # ============ hyper fc stack (breadth-first stages) ============
    xacc = small.tile([64, NMB], FP, tag="xacc")
    p1 = [fps.tile([16, MB], FP, tag="fp", name=f"p1_{j}") for j in range(NMB)]
    for j in range(NMB):
        nc.tensor.matmul(p1[j][:], fc1s, xT[:, MBs[j]], start=True, stop=True)
    for j in range(NMB):
        nc.scalar.activation(x1T[:, MBs[j]], p1[j][:], AF.Sigmoid, bias=fc1b)
    p2 = [fps.tile([2, MB], FP, tag="fp", name=f"p2_{j}") for j in range(NMB)]
    for j in range(NMB):
        nc.tensor.matmul(p2[j][:], fc2s, x1T[:, MBs[j]], start=True, stop=True)
    for j in range(NMB):
        nc.scalar.activation(x2a[0:2, MBs[j]], p2[j][:], AF.Sigmoid, bias=fc2b)
    p3 = [fps.tile([64, MB], FP, tag="fp", name=f"p3_{j}") for j in range(NMB)]
    for j in range(NMB):
        nc.tensor.matmul(p3[j][:], fc3s, x2a[0:2, MBs[j]], start=True, stop=True)
    for j in range(NMB):
        nc.scalar.activation(c1[0:64, MBs[j]], p3[j][:], AF.Identity, bias=fc3b,
                             accum_out=xacc[:, j:j + 1])
    for j in range(NMB):
        nc.vector.tensor_copy(c2[64:128, MBs[j]], c1[0:64, MBs[j]])
    xs_f = small.tile([64, 1], FP, tag="xs_f")
    nc.vector.tensor_reduce(xs_f[:], xacc[:], AX.X, AL.add)
    nc.vector.tensor_copy(xsb[:], xs_f[:])

    # ================= GRU gates (breadth-first stages) =================
    zp = [fps.tile([64, MB], FP, tag="fp", name=f"zp_{j}") for j in range(NMB)]
    for j in range(NMB):
        nc.tensor.matmul(zp[j][:], wz, c1[:, MBs[j]], start=True, stop=True)
    zt = [gw.tile([64, MB], BF, tag="zt", name=f"zt_{j}") for j in range(NMB)]
    for j in range(NMB):
        nc.scalar.activation(zt[j][:], zp[j][:], AF.Sigmoid)
    rp = [fps.tile([64, MB], FP, tag="fp", name=f"rp_{j}") for j in range(NMB)]
    for j in range(NMB):
        nc.tensor.matmul(rp[j][:], wr, c1[:, MBs[j]], start=True, stop=True)
    rt = [gw.tile([64, MB], BF, tag="rt", name=f"rt_{j}") for j in range(NMB)]
    for j in range(NMB):
        nc.scalar.activation(rt[j][:], rp[j][:], AF.Sigmoid)
    for j in range(NMB):
        nc.vector.tensor_tensor(c2[0:64, MBs[j]], rt[j][:], lastT[:, MBs[j]], AL.mult)
    hp = [fps.tile([64, MB], FP, tag="fp", name=f"hp_{j}") for j in range(NMB)]
    for j in range(NMB):
        nc.tensor.matmul(hp[j][:], wh, c2[:, MBs[j]], start=True, stop=True)
    ht = [gw.tile([64, MB], BF, tag="ht", name=f"ht_{j}") for j in range(NMB)]
    for j in range(NMB):
        nc.scalar.activation(ht[j][:], hp[j][:], AF.Tanh)
    dt_ = [gw.tile([64, MB], BF, tag="dt", name=f"dt_{j}") for j in range(NMB)]
    for j in range(NMB):
        nc.vector.tensor_tensor(dt_[j][:], ht[j][:], lastT[:, MBs[j]], AL.subtract)
    for j in range(NMB):
        nc.vector.tensor_tensor(dt_[j][:], dt_[j][:], zt[j][:], AL.mult)
    for j in range(NMB):
        nc.vector.tensor_tensor(hgsq[0:64, MBs[j]], dt_[j][:], lastT[:, MBs[j]], AL.add)

    # Hg^2 (scalar) then Hg LN stats rows
    nc.scalar.square(hgsq[64:128, 0:HALF], hgsq[0:64, 0:HALF])
    nc.scalar.square(hgsq[64:128, HALF:N], hgsq[0:64, HALF:N])
    hsp = [fps.tile([2, MB], FP, tag="fp", name=f"hsp_{j}") for j in range(NMB)]
    for j in range(NMB):
        nc.tensor.matmul(hsp[j][:], sel2, hgsq[:, MBs[j]], start=True, stop=True)
    hsum = small.tile([2, N], FP, tag="hsum")
    for j in range(NMB):
        (nc.vector.tensor_copy if j % 2 == 0 else nc.scalar.copy)(
            hsum[:, MBs[j]], hsp[j][:])
    hst0 = small.tile([NCH, 128], FP, tag="hst0")
    nc.sync.dma_start(hst0[:], hsum[0:1, :].rearrange("o (i p) -> o i p", p=128))
    hst1 = small.tile([NCH, 128], FP, tag="hst1")
    nc.sync.dma_start(hst1[:], hsum[1:2, :].rearrange("o (i p) -> o i p", p=128))
    _, _, ha, hc = _stat_land(nc, small, hst0[:], hst1[:], NCH, "hg", epsc16, 1.0 / 64)
    ha16 = small.tile([NCH, 128], BF, tag="ha16")
    nc.vector.tensor_copy(ha16[:], ha[:])
    hc16 = small.tile([NCH, 128], BF, tag="hc16")
    nc.scalar.copy(hc16[:], hc[:])
    nc.sync.dma_start(a_row[:].rearrange("o (i p) -> o i p", p=128), ha16[:])
    nc.sync.dma_start(HgQ[64:65, :].rearrange("o (i p) -> o i p", p=128), hc16[:])
    # c in chunk-column layout for the lastH bias path
    pcc = fps.tile([128, NCH], FP, tag="fp", name="pcc", padded_shape=[128, 512])
    _tp(nc, pcc[:], hc[:], ident)
    nc.scalar.copy(cT_sb[:], pcc[:])

    # HgA = Hg_raw * a (K=1 broadcast matmul + fused multiply from PSUM)
    ab = [fps.tile([64, MB], FP, tag="fp", name=f"ab_{j}") for j in range(NMB)]
    for j in range(NMB):
        nc.tensor.matmul(ab[j][:], onesr64, a_row[:, MBs[j]], start=True, stop=True)
    for j in range(NMB):
        nc.vector.tensor_tensor(HgQ[0:64, MBs[j]], hgsq[0:64, MBs[j]], ab[j][:],
                                AL.mult)

    # ===================== q / k projections =====================
    kacc = small.tile([128, 8], FP, tag="kacc")
    kjobs = []
    for half, dst in ((0, kT0), (1, kT1)):
        for j in range(NMB):
            kjobs.append((dst, slice(W_KA + 128 * half, W_KA + 128 * (half + 1)),
                          MBs[j], kacc[:, 4 * half + j:4 * half + j + 1]))
    qjobs = []
    for half, dst in ((0, qT0), (1, qT1)):
        for j in range(2):
            qjobs.append((dst, slice(W_QA + 128 * half, W_QA + 128 * (half + 1)),
                          HBs[j], None))
    kq_ps = []
    for idx, (dst, wsl, sl, acc) in enumerate(kjobs + qjobs):
        kp = fps.tile([128, MB], FP, tag="fp", name=f"kqp_{idx}")
        nc.tensor.matmul(kp[:], wp[0:66, wsl], HgQ[:, sl], start=True, stop=True)
        kq_ps.append(kp)
    for idx, (dst, wsl, sl, acc) in enumerate(kjobs + qjobs):
        nc.scalar.copy(dst[:, sl], kq_ps[idx][:])
        if acc is not None:
            nc.vector.scalar_tensor_tensor(dst[:, sl], dst[:, sl], 0.01, dst[:, sl],
                                           AL.mult, AL.max, accum_out=acc)
        else:
            _leaky(nc, dst[:, sl])
    ks_f = small.tile([128, 2], FP, tag="ks_f")
    nc.vector.tensor_reduce(ks_f[:], kacc[:].rearrange("p (h j) -> p h j", j=4),
                            AX.X, AL.add)
    nc.vector.tensor_copy(ks0[:], ks_f[:, 0:1])
    nc.vector.tensor_copy(ks1[:], ks_f[:, 1:2])
    # k2 = k * ga ; x3gs rows 0:64 = x3 * gs
    nc.vector.tensor_tensor(k2T0[:], kT0[:], ga_b[:], AL.mult)
    nc.vector.tensor_tensor(k2T1[:], kT1[:], ga_b[:], AL.mult)
    nc.vector.tensor_tensor(x3gs[0:64, :], c1[0:64, :], gs_b[:], AL.mult)

    frontA.close()

    # ===================== Gram matrices =====================
    with tc.tile_pool(name="gpsp", bufs=3, space="PSUM") as gpsp, \
         tc.tile_pool(name="krpp", bufs=3, space="PSUM") as krpp, \
         tc.tile_pool(name="krp", bufs=3) as krp:
        gt_ps = gpsp.tile([128, 256], FP, tag="g", padded_shape=[128, 512])
        gb_ps = gpsp.tile([128, 256], FP, tag="g", padded_shape=[128, 512])
        for mi in range(NCH):
            msl = slice(mi * 128, (mi + 1) * 128)
            krq = krpp.tile([128, 256], FP, tag="kr", padded_shape=[128, 512])
            nc.tensor.matmul(krq[:], HgQ[:, msl], kA, start=True, stop=True)
            kr = krp.tile([128, 256], BF, tag="kr")
            nc.scalar.copy(kr[:], krq[:])
            _leaky(nc, kr[:])
            nc.tensor.matmul(gt_ps[:], kr[:, 0:128], kr[:],
                             start=(mi == 0), stop=(mi == NCH - 1))
            nc.tensor.matmul(gb_ps[:], kr[:, 128:256], kr[:],
                             start=(mi == 0), stop=(mi == NCH - 1))
        nc.vector.tensor_copy(gt_sb[:], gt_ps[:])
        nc.scalar.copy(gb_sb[:], gb_ps[:])
        gs_ps = gpsp.tile([64, 64], FP, tag="g", padded_shape=[64, 512])
        for mi in range(NCH):
            msl = slice(mi * 128, (mi + 1) * 128)
            xrq = krpp.tile([128, 64], FP, tag="kr", padded_shape=[128, 512])
            nc.tensor.matmul(xrq[:], x2a[:, msl], fc3a, start=True, stop=True)
            xr = krp.tile([128, 64], BF, tag="xr")
            (nc.vector.tensor_copy if mi % 2 == 0 else nc.scalar.copy)(xr[:], xrq[:])
            nc.tensor.matmul(gs_ps[:], xr[:], xr[:],
                             start=(mi == 0), stop=(mi == NCH - 1))
        nc.vector.tensor_copy(gs_f[:], gs_ps[:])

    # ============== own-row stats: S1, T1, S2, T2 rows ==============
    statq = ExitStack()
    ups = statq.enter_context(tc.tile_pool(name="ups", bufs=2, space="PSUM"))
    sps = statq.enter_context(tc.tile_pool(name="sps", bufs=2, space="PSUM"))
    lps = statq.enter_context(tc.tile_pool(name="lps", bufs=2, space="PSUM"))
    # u = G q per 512-half; e = u * q  (separate e0/e1, summed in PSUM below)
    for jb in range(2):
        sl = HBs[jb]
        ut0 = ups.tile([128, MB], FP, tag="ut", name=f"ut0_{jb}")
        nc.tensor.matmul(ut0[:], gt_sb[:, 0:128], qT0[:, sl], start=True, stop=False)
        nc.tensor.matmul(ut0[:], gb_sb[:, 0:128], qT1[:, sl], start=False, stop=True)
        ut1 = ups.tile([128, MB], FP, tag="ut", name=f"ut1_{jb}")
        nc.tensor.matmul(ut1[:], gt_sb[:, 128:256], qT0[:, sl], start=True, stop=False)
        nc.tensor.matmul(ut1[:], gb_sb[:, 128:256], qT1[:, sl], start=False, stop=True)
        nc.vector.tensor_tensor(e0sb[:, sl], ut0[:], qT0[:, sl], AL.mult)
        nc.vector.tensor_tensor(e1sb[:, sl], ut1[:], qT1[:, sl], AL.mult)
    for jb in range(2):
        sl = HBs[jb]
        us = ups.tile([64, MB], FP, tag="ut", name=f"us_{jb}")
        nc.tensor.matmul(us[:], gs_f[:], c1[0:64, sl], start=True, stop=True)
        nc.vector.tensor_tensor(essb[:, sl], us[:], c1[0:64, sl], AL.mult)

    s1sb = small.tile([1, HALF], FP, tag="s1sb")
    t1sb = small.tile([1, HALF], FP, tag="t1sb")
    s2sb = small.tile([1, HALF], FP, tag="s2sb")
    t2sb = small.tile([1, HALF], FP, tag="t2sb")
    for jb in range(2):
        sl = HBs[jb]
        s1p = sps.tile([1, MB], FP, tag="st", name=f"s1p_{jb}", padded_shape=[1, 512])
        nc.tensor.matmul(s1p[:], ks0[:], qT0[:, sl], start=True, stop=False)
        nc.tensor.matmul(s1p[:], ks1[:], qT1[:, sl], start=False, stop=True)
        t1p = sps.tile([1, MB], FP, tag="st", name=f"t1p_{jb}", padded_shape=[1, 512])
        nc.tensor.matmul(t1p[:], xsb[:], c1[0:64, sl], start=True, stop=True)
        nc.scalar.copy(s1sb[:, sl], s1p[:])
        nc.vector.tensor_copy(t1sb[:, sl], t1p[:])
    for jb in range(2):
        sl = HBs[jb]
        s2p = sps.tile([1, MB], FP, tag="st", name=f"s2p_{jb}", padded_shape=[1, 512])
        nc.tensor.matmul(s2p[:], ones128c, e0sb[:, sl], start=True, stop=False)
        nc.tensor.matmul(s2p[:], ones128c, e1sb[:, sl], start=False, stop=True)
        t2p = sps.tile([1, MB], FP, tag="st", name=f"t2p_{jb}", padded_shape=[1, 512])
        nc.tensor.matmul(t2p[:], ones64c, essb[:, sl], start=True, stop=True)
        nc.scalar.copy(s2sb[:, sl], s2p[:])
        nc.vector.tensor_copy(t2sb[:, sl], t2p[:])
    s1t = small.tile([HCH, 128], FP, tag="s1t")
    nc.sync.dma_start(s1t[:], s1sb[:].rearrange("o (i p) -> o i p", p=128))
    t1t = small.tile([HCH, 128], FP, tag="t1t")
    nc.scalar.dma_start(t1t[:], t1sb[:].rearrange("o (i p) -> o i p", p=128))
    s2t = small.tile([HCH, 128], FP, tag="s2t")
    nc.sync.dma_start(s2t[:], s2sb[:].rearrange("o (i p) -> o i p", p=128))
    t2t = small.tile([HCH, 128], FP, tag="t2t")
    nc.scalar.dma_start(t2t[:], t2sb[:].rearrange("o (i p) -> o i p", p=128))

    # ---- fillers for the stats-land latency ----
    # lastH output (Hg LN rows, own half)
    for i in range(HCH):
        pt = lps.tile([128, 64], BF, tag="lpt", name=f"lpt_{i}",
                      padded_shape=[128, 1024])
        _tp(nc, pt[:], HgQ[0:64, i * 128:(i + 1) * 128], identb)
        nc.scalar.activation(lastR[:, i * 64:(i + 1) * 64], pt[:], AF.Identity,
                             bias=cT_sb[:, i:i + 1])
    l3 = lastR[:].rearrange("p (g e) -> p g e", e=64)
    lg3 = brows["bng"][:].unsqueeze(1).broadcast_to([128, HCH, 64])
    lb3 = brows["bnb"][:].unsqueeze(1).broadcast_to([128, HCH, 64])
    nc.vector.tensor_tensor(l3, l3, lg3, AL.mult)
    nc.vector.tensor_tensor(l3, l3, lb3, AL.add)
    nc.sync.dma_start(io["lastH"].rearrange("(i p) e -> p i e", p=128),
                      lastR[:].rearrange("p (i e) -> p i e", e=64))

    # xo affine into x1aug
    oab = [sps.tile([64, MB], FP, tag="st", name=f"oab_{j}") for j in range(2)]
    for j in range(2):
        nc.tensor.matmul(oab[j][:], onesr64, oar[:, HBs[j]], start=True, stop=True)
    ocb = [sps.tile([64, MB], FP, tag="st", name=f"ocb_{j}") for j in range(2)]
    for j in range(2):
        nc.tensor.matmul(ocb[j][:], onesr64, ocr[:, HBs[j]], start=True, stop=True)
    for j in range(2):
        tb = small.tile([64, MB], BF, tag=f"oxt_{j}", name=f"oxt_{j}")
        nc.vector.tensor_tensor(tb[:], osq[0:64, HBs[j]], oab[j][:], AL.mult)
        nc.vector.tensor_tensor(tb[:], tb[:], ocb[j][:], AL.add)
        nc.scalar.activation(x1aug[0:64, HBs[j]], tb[:], AF.Identity,
                             scale=xng_c, bias=xb3_c)

    # GCN layer-1 matmul (dl-independent)
    for jb in range(2):
        ph1 = sps.tile([64, MB], FP, tag="st", name=f"ph1_{jb}")
        nc.tensor.matmul(ph1[:], w1a, x1aug[:, HBs[jb]], start=True, stop=True)
        (nc.scalar.copy if jb == 0 else nc.vector.tensor_copy)(
            ph1sb[:, HBs[jb]], ph1[:])

    # ---- own stats land [8, 128] ----
    muA, sdA, rsA_, _cA = _stat_land(nc, small, s1t[:], s2t[:], HCH, "sa",
                                     epsc8, 1.0 / N)
    muS, sdS, rsS_, _cS = _stat_land(nc, small, t1t[:], t2t[:], HCH, "ss",
                                     epsc8, 1.0 / N)
    rho = small.tile([HCH, 128], FP, tag="rho")
    nc.vector.tensor_tensor(rho[:], rsS_[:], sdA[:], AL.mult)
    r64t = small.tile([HCH, 128], BF, tag="r64t")
    nc.vector.tensor_scalar(r64t[:], muA[:], -1.0, None, AL.mult)
    r65t = small.tile([HCH, 128], BF, tag="r65t")
    nc.vector.scalar_tensor_tensor(r65t[:], muS[:], -1.0, rho[:], AL.mult, AL.mult)
    r66t = small.tile([HCH, 128], BF, tag="r66t")
    nc.scalar.copy(r66t[:], sdA[:])
    rho16 = small.tile([HCH, 128], BF, tag="rho16")
    nc.scalar.copy(rho16[:], rho[:])
    nc.sync.dma_start(x3rA[64:65, :].rearrange("o (i p) -> o i p", p=128), r64t[:])
    nc.scalar.dma_start(x3rA[65:66, :].rearrange("o (i p) -> o i p", p=128), r65t[:])
    nc.sync.dma_start(x3rA[66:67, :].rearrange("o (i p) -> o i p", p=128), r66t[:])
    rho_row = small.tile([1, HALF], BF, tag="rho_row")
    nc.scalar.dma_start(rho_row[:].rearrange("o (i p) -> o i p", p=128), rho16[:])
    for jb in range(2):
        rp_ = sps.tile([64, MB], FP, tag="st", name=f"rhob_{jb}")
        nc.tensor.matmul(rp_[:], onesr64, rho_row[:, HBs[jb]], start=True, stop=True)
        nc.vector.tensor_tensor(x3rA[0:64, HBs[jb]], c1[0:64, HBs[jb]], rp_[:],
                                AL.mult)
    statq.close()

    # =================== phase I: fused adjacency ===================
    with tc.tile_pool(name="zps", bufs=8, space="PSUM") as zps, \
         tc.tile_pool(name="scrv", bufs=2) as scrv, \
         tc.tile_pool(name="scra", bufs=2) as scra:
        ztiles = {}

        def passes12(i):
            csl = slice(i * 128, (i + 1) * 128)
            zpt = [zps.tile([128, MB], FP, tag="zpt", name=f"zp_{i}_{m}")
                   for m in range(NMB)]
            ztiles[i] = zpt
            for mb in range(NMB):
                nc.tensor.matmul(zpt[mb][:], qT0[:, csl],
                                 k2T0[:, mb * MB:(mb + 1) * MB],
                                 start=True, stop=False)
            for mb in range(NMB):
                nc.tensor.matmul(zpt[mb][:], qT1[:, csl],
                                 k2T1[:, mb * MB:(mb + 1) * MB],
                                 start=False, stop=False)

        def pass3(i):
            csl = slice(i * 128, (i + 1) * 128)
            zpt = ztiles[i]
            for mb in range(NMB):
                nc.tensor.matmul(zpt[mb][:], x3rA[:, csl],
                                 x3gs[:, mb * MB:(mb + 1) * MB],
                                 start=False, stop=True)
            for mb in range(NMB):
                acc = rc32[:, 4 * i + mb:4 * i + mb + 1]
                if mb % 2 == 0:
                    scr = scrv.tile([128, MB], FP, tag="scr", name=f"scr_{i}_{mb}")
                    nc.vector.tensor_scalar(scr[:], zpt[mb][:], 0.0, None, AL.max,
                                            AL.add, accum_out=acc)
                else:
                    scr = scra.tile([128, MB], FP, tag="scr2", name=f"scr2_{i}_{mb}")
                    nc.scalar.activation(scr[:], zpt[mb][:], AF.Relu, accum_out=acc)
                if mb == i // 4:
                    off = (i * 128) % MB
                    dsel = scrv.tile([128, 128], FP, tag="dsel", name=f"dsel_{i}")
                    nc.gpsimd.affine_select(
                        out=dsel[:], in_=scr[:, off:off + 128],
                        compare_op=AL.is_equal, fill=0.0, base=0,
                        pattern=[[-1, 128]], channel_multiplier=1)
                    nc.vector.tensor_reduce(dg8[:, i:i + 1], dsel[:], AX.X, AL.add)

        passes12(0)
        passes12(1)
        for i in range(HCH):
            pass3(i)
            if i + 2 < HCH:
                passes12(i + 2)
        rs8 = small.tile([128, HCH], FP, tag="rs8")
        nc.vector.tensor_reduce(rs8[:], rc32[:].rearrange("p (i m) -> p i m", m=4),
                                AX.X, AL.add)
        nc.vector.reciprocal(rs8[:], rs8[:])
        nc.vector.tensor_tensor(dl[:], dg8[:], rs8[:], AL.mult)

    # ======================= GCN tail =======================
    with tc.tile_pool(name="jps", bufs=4, space="PSUM") as jps, \
         tc.tile_pool(name="jw", bufs=2) as jw:
        ptd = jps.tile([HCH, 128], FP, tag="jt", padded_shape=[HCH, 512])
        _tp(nc, ptd[:], dl[:], ident)
        s8d = small.tile([HCH, 128], BF, tag="s8d")
        nc.vector.tensor_copy(s8d[:], ptd[:])
        dl_row = small.tile([1, HALF], BF, tag="dl_row")
        nc.sync.dma_start(dl_row[:].rearrange("o (i p) -> o i p", p=128), s8d[:])
        dls = small.tile([64, HALF], BF, tag="dls")
        for jb in range(2):
            dp = jps.tile([64, MB], FP, tag="jt", name=f"dlsb_{jb}")
            nc.tensor.matmul(dp[:], onesr64, dl_row[:, HBs[jb]], start=True, stop=True)
            nc.scalar.copy(dls[:, HBs[jb]], dp[:])

        # layer 1 from the pre-computed matmul; layers 2-3 live
        for jb in range(2):
            nc.vector.tensor_tensor(hca[0:64, HBs[jb]], ph1sb[:, HBs[jb]],
                                    dls[:, HBs[jb]], AL.mult)
        ph2 = [jps.tile([64, MB], FP, tag="jt", name=f"ph2_{jb}") for jb in range(2)]
        for jb in range(2):
            nc.tensor.matmul(ph2[jb][:], w2a, hca[:, HBs[jb]], start=True, stop=True)
        for jb in range(2):
            nc.vector.tensor_tensor(hcb[0:64, HBs[jb]], ph2[jb][:], dls[:, HBs[jb]],
                                    AL.mult)
        ph3 = [jps.tile([64, MB], FP, tag="jt", name=f"ph3_{jb}") for jb in range(2)]
        for jb in range(2):
            nc.tensor.matmul(ph3[jb][:], w3a, hcb[:, HBs[jb]], start=True, stop=True)
        for jb in range(2):
            nc.vector.tensor_tensor(finsq[0:64, HBs[jb]], ph3[jb][:], dls[:, HBs[jb]],
                                    AL.mult)
            nc.vector.tensor_tensor(finsq[0:64, HBs[jb]], finsq[0:64, HBs[jb]],
                                    x1aug[0:64, HBs[jb]], AL.add)
        # fin LayerNorm stats in T layout
        nc.scalar.square(finsq[64:128, :], finsq[0:64, :])
        fsp = [jps.tile([2, MB], FP, tag="jt", name=f"fsp_{jb}") for jb in range(2)]
        for jb in range(2):
            nc.tensor.matmul(fsp[jb][:], sel2, finsq[:, HBs[jb]], start=True,
                             stop=True)
        fsum = small.tile([2, HALF], FP, tag="fsum")
        for jb in range(2):
            (nc.vector.tensor_copy if jb == 0 else nc.scalar.copy)(
                fsum[:, HBs[jb]], fsp[jb][:])
        fst0 = small.tile([HCH, 128], FP, tag="fst0")
        nc.sync.dma_start(fst0[:], fsum[0:1, :].rearrange("o (i p) -> o i p", p=128))
        fst1 = small.tile([HCH, 128], FP, tag="fst1")
        nc.scalar.dma_start(fst1[:], fsum[1:2, :].rearrange("o (i p) -> o i p", p=128))
        _, _, fa, fc = _stat_land(nc, small, fst0[:], fst1[:], HCH, "fl",
                                  epsc8, 1.0 / 64)
        fap = jps.tile([128, HCH], FP, tag="jt", name="fap", padded_shape=[128, 512])
        _tp(nc, fap[:], fa[:], ident)
        facol = small.tile([128, HCH], FP, tag="facol")
        nc.vector.tensor_copy(facol[:], fap[:])
        fcp = jps.tile([128, HCH], FP, tag="jt", name="fcp", padded_shape=[128, 512])
        _tp(nc, fcp[:], fc[:], ident)
        fccol = small.tile([128, HCH], FP, tag="fccol")
        nc.scalar.copy(fccol[:], fcp[:])
        for i in range(HCH):
            pt = jps.tile([128, 64], BF, tag="jt", name=f"fin_{i}",
                          padded_shape=[128, 1024])
            _tp(nc, pt[:], finsq[0:64, i * 128:(i + 1) * 128], identb)
            if i % 2 == 0:
                nc.vector.tensor_scalar(fin[:, i * 64:(i + 1) * 64], pt[:],
                                        facol[:, i:i + 1], fccol[:, i:i + 1],
                                        AL.mult, AL.add)
            else:
                nc.scalar.activation(fin[:, i * 64:(i + 1) * 64], pt[:], AF.Identity,
                                     scale=facol[:, i:i + 1],
                                     bias=fccol[:, i:i + 1])
        f3 = fin[:].rearrange("p (g e) -> p g e", e=64)
        fg3 = brows["lng"][:].unsqueeze(1).broadcast_to([128, HCH, 64])
        fb3 = brows["lnb"][:].unsqueeze(1).broadcast_to([128, HCH, 64])
        nc.vector.tensor_tensor(f3, f3, fg3, AL.mult)
        nc.vector.tensor_tensor(f3, f3, fb3, AL.add)
        nc.sync.dma_start(io["outH"].rearrange("(i p) e -> p i e", p=128),
                          fin[:].rearrange("p (i e) -> p i e", e=64))


def _build():
    if "nc" in _CACHE:
        return _CACHE["nc"]
    nc = bacc.Bacc("TRN2", target_bir_lowering=False, debug=False,
                   enable_asserts=True, num_devices=8)
    io = {}
    io["xT"] = nc.dram_tensor("xT", [G, N], BF, kind="ExternalInput").ap()
    io["lastT"] = nc.dram_tensor("lastT", [G, N], BF, kind="ExternalInput").ap()
    io["origT"] = nc.dram_tensor("origT", [E, HALF], BF, kind="ExternalInput").ap()
    io["corr4"] = nc.dram_tensor("corr4", [4, N], BF, kind="ExternalInput").ap()
    io["wpack"] = nc.dram_tensor("wpack", [128, WPACK_W], BF, kind="ExternalInput").ap()
    io["fpack"] = nc.dram_tensor("fpack", [128, FPACK_W], FP, kind="ExternalInput").ap()
    io["outH"] = nc.dram_tensor("outH", [HALF, E], FP, kind="ExternalOutput").ap()
    io["lastH"] = nc.dram_tensor("lastH", [HALF, G], FP, kind="ExternalOutput").ap()

    with tile.TileContext(nc) as tc:
        with ExitStack() as ctx:
            _emit(ctx, tc, io)
    nc.compile()
    nc.m = get_hw_module(nc.m)
    _CACHE["nc"] = nc
    return nc


def _host_prep(inputs):
    f32 = np.float32
    bf = ml_dtypes.bfloat16
    inp = {k: np.asarray(v, f32) for k, v in inputs.items()}
    ch = 1.0 + inp["mlp_w"].sum(axis=0)
    assert (ch > 0).all(), "head-mixing scale fold requires positive c_h"
    g, b = inp["bn_g"], inp["bn_b"]
    qw_c = inp["q_w"] * np.repeat(ch / np.sqrt(G), G)[None, :]
    Wq = g[:, None] * qw_c
    qA = np.concatenate([Wq, Wq.sum(axis=0)[None], (b @ qw_c)[None]], axis=0)
    Wk = g[:, None] * inp["k_w"]
    kA = np.concatenate([Wk, Wk.sum(axis=0)[None], (b @ inp["k_w"])[None]], axis=0)
    w1 = inp["gcn_w1"]
    w1a = np.concatenate([w1, -(inp["gcn_b3"] @ w1)[None]], axis=0)
    w2a = np.concatenate([inp["gcn_w2"], (inp["gcn_b1"] @ inp["gcn_w2"])[None]], axis=0)
    w3a = np.concatenate([inp["gcn_w3"], (inp["gcn_b2"] @ inp["gcn_w3"])[None]], axis=0)
    fc3a = np.concatenate([inp["fc3_w"], inp["fc3_b"][None, :]], axis=0)

    wpack = np.zeros((128, WPACK_W), f32)
    wpack[0:128, W_IDB:W_IDB + 128] = np.eye(128)
    wpack[0:128, W_WZ:W_WZ + 64] = inp["w_z"]
    wpack[0:128, W_WR:W_WR + 64] = inp["w_r"]
    wpack[0:128, W_WH:W_WH + 64] = inp["w_h"]
    wpack[0:66, W_QA:W_QA + 256] = qA
    wpack[0:66, W_KA:W_KA + 256] = kA
    wpack[0:64, W_FC1:W_FC1 + 16] = inp["fc1_w"]
    wpack[0:16, W_FC2:W_FC2 + 2] = inp["fc2_w"]
    wpack[0:3, W_FC3A:W_FC3A + 64] = fc3a
    wpack[0:65, W_W1A:W_W1A + 64] = w1a
    wpack[0:65, W_W2A:W_W2A + 64] = w2a
    wpack[0:65, W_W3A:W_W3A + 64] = w3a
    wpack[0:64, W_SEL:W_SEL + 1] = 1.0
    wpack[64:128, W_SEL + 1:W_SEL + 2] = 1.0
    wpack[:, W_ONE:W_ONE + 128] = 1.0

    fpack = np.zeros((128, FPACK_W), f32)
    fpack[0:128, F_IDF:F_IDF + 128] = np.eye(128)
    fpack[0:16, F_B + 0] = inp["fc1_b"]
    fpack[0:2, F_B + 1] = inp["fc2_b"]
    fpack[0:64, F_B + 2] = inp["fc3_b"]
    fpack[0:128, F_EPS] = EPS
    fpack[0:64, F_XG] = inp["x_nom_g"]
    fpack[0:64, F_XB3] = inp["x_nom_b"] + inp["gcn_b3"]
    for k, nm in enumerate(("bn_g", "bn_b", "last_nom_g", "last_nom_b")):
        fpack[0, F_BN + 64 * k:F_BN + 64 * (k + 1)] = inp[nm]

    def c(a, dt=bf):
        return np.ascontiguousarray(np.asarray(a, dt))

    shared = {"wpack": c(wpack), "fpack": c(fpack, f32)}
    in_maps = []
    for core in range(8):
        bi, h = core // 2, core % 2
        off = h * HALF
        corr4 = np.stack([
            np.roll(inp["attn_norm_g"], -off),
            np.roll(inp["skip_norm_g"], -off),
            np.roll(inp["attn_norm_b"] + inp["skip_norm_b"], -off),
            np.ones(N, f32),
        ])
        m = dict(shared)
        m["xT"] = c(np.roll(inp["x"][bi], -off, axis=0).T)
        m["lastT"] = c(np.roll(inp["last_G_emb"][bi], -off, axis=0).T)
        m["origT"] = c(inp["orig_x"][bi, off:off + HALF].T)
        m["corr4"] = c(corr4)
        in_maps.append(m)
    return in_maps


def run(inputs, trace=False):
    nc = _build()
    in_maps = _host_prep(inputs)
    res = run_bass_kernel_spmd(nc, in_maps, core_ids=list(range(8)), trace=trace)
    out = np.zeros((B, N, E), np.float32)
    last = np.zeros((B, N, G), np.float32)
    for core in range(8):
        bi, h = core // 2, core % 2
        off = h * HALF
        out[bi, off:off + HALF] = res.results[core]["outH"]
        last[bi, off:off + HALF] = res.results[core]["lastH"]
    return (out, last), res


def kernel(**inputs):
    return run(inputs)[0]
